# revision 1
# baseline (speedup 1.0000x reference)
"""Trainium2 Bass kernel for nn_Encoder (dense transformer block), 8 NeuronCores.

Strategy (single chip, 8 cores):
  Phase A (head-parallel): core c computes attention for heads {2c, 2c+1}.
    All activations are kept "transposed" (feature dim on SBUF partitions) so
    every matmul consumes naturally-laid-out operands and no fp32 transposes
    are ever needed on device; the host pre-transposes X and all weights.
    softmax(relu(s)) is computed as p = max(exp(s/8), 1) (exp is monotonic),
    and the row sums come for free as a 65th column of the p @ [V | 1] matmul.
  Host gathers per-head ctx blocks (2 MiB/core) between launches.
  Phase B (row-parallel): core c takes 512 of the 4096 token rows:
    ctx @ Wo.T (+X residual), LN1, FFN (ReLU), LN2. LayerNorm reductions run
    over the partition dim via tiny ones-vector matmuls on the PE.

kernel() is self-contained: it compiles both phase programs on first call
(cached in module globals) and runs them via run_bass_kernel_spmd.
"""

import os
import sys

for _p in ("/opt/trn_rl_repo",):
    if _p not in sys.path:
        sys.path.insert(0, _p)

# The Bass SPMD path executes through jax/PJRT on the axon platform; make
# sure a caller-pinned JAX_PLATFORMS=cpu doesn't hide the NeuronCores.
_jp = os.environ.get("JAX_PLATFORMS")
if _jp is not None and "axon" not in _jp:
    os.environ["JAX_PLATFORMS"] = "axon," + _jp

import numpy as np

import concourse.bass as bass
import concourse.mybir as mybir
import concourse.tile as tile
from concourse import bacc
from concourse.bass_utils import run_bass_kernel_spmd

F32 = mybir.dt.float32
F32R = mybir.dt.float32r
AF = mybir.ActivationFunctionType
OP = mybir.AluOpType


def _mm(nc, out, lhsT, rhs, **kw):
    # fp32r: 1-pass FP22 matmul (4x faster than 4-pass true-fp32 for N>=256)
    nc.tensor.matmul(out, lhsT.bitcast(F32R), rhs.bitcast(F32R), **kw)

N_CORES = 8
B, S, D, H, DH, FF = 2, 2048, 1024, 16, 64, 4096
N = B * S            # 4096 token rows
P = 128
QC = N // N_CORES    # 512 rows per core in phase B
HPC = H // N_CORES   # 2 heads per core in phase A
EPS = 1e-5

_CACHE = {}


# --------------------------------------------------------------------------
# Phase A: per-core head-parallel attention.
# Inputs (per core):
#   xt  [D, N]     X^T (full, replicated)
#   wqt [D, 128]   Wq^T columns for this core's two heads
#   wkt [D, 128]   Wk^T columns
#   wvt [D, 128]   Wo^T columns (value projection uses W_o in this model)
# Output:
#   ctx_t [128, N] softmax(relu(qk/8)) @ v, transposed; rows = the two heads'
#                  64-dim blocks stacked, cols = (b, s) token index.
# --------------------------------------------------------------------------
def _build_phase_a():
    nc = bacc.Bacc("TRN2", target_bir_lowering=False, debug=False,
                   num_devices=N_CORES)
    xt = nc.dram_tensor("xt", [D, N], F32R, kind="ExternalInput")
    wqt = nc.dram_tensor("wqt", [D, P], F32R, kind="ExternalInput")
    wkt = nc.dram_tensor("wkt", [D, P], F32R, kind="ExternalInput")
    wvt = nc.dram_tensor("wvt", [D, P], F32R, kind="ExternalInput")
    idm = nc.dram_tensor("idm", [P, DH], F32R, kind="ExternalInput")
    ctx_t = nc.dram_tensor("ctx_t", [P, N], F32, kind="ExternalOutput")

    KD = D // P        # 8 contraction chunks over D
    NQ = N // 512      # 8 qi chunks of 512 over all tokens
    KI = S // P        # 16 ki chunks of 128 per batch

    with tile.TileContext(nc) as tc:
        with tc.tile_pool(name="persist", bufs=1) as persist:
            # Persistent SBUF, split per batch so batch-1 projection writes
            # never serialize against batch-0 attention reads: projected Q^T,
            # K^T, V^T (1 MiB each per batch) and V' (natural layout per
            # ki-chunk: [v_h0(64) | 1 | v_h1(64) | 1]).
            qt_sb = [persist.tile([P, S], F32R, name=f"qt{b_}") for b_ in range(B)]
            kt_sb = [persist.tile([P, S], F32R, name=f"kt{b_}") for b_ in range(B)]
            vt_sb = [persist.tile([P, S], F32R, name=f"vt{b_}") for b_ in range(B)]
            vp_sb = [persist.tile([P, KI, 2 * (DH + 1)], F32R, name=f"vp{b_}")
                     for b_ in range(B)]
            wq_sb = persist.tile([P, KD, P], F32R)
            wk_sb = persist.tile([P, KD, P], F32R)
            wv_sb = persist.tile([P, KD, P], F32R)
            id_sb = persist.tile([P, DH], F32R)

            nc.sync.dma_start(wq_sb[:], wqt.ap().rearrange("(kc p) m -> p kc m", p=P))
            nc.sync.dma_start(wk_sb[:], wkt.ap().rearrange("(kc p) m -> p kc m", p=P))
            nc.sync.dma_start(wv_sb[:], wvt.ap().rearrange("(kc p) m -> p kc m", p=P))
            nc.sync.dma_start(id_sb[:], idm.ap())
            for b_ in range(B):
                # ones columns of V' (columns DH and 2*DH+1)
                nc.vector.memset(vp_sb[b_][:, :, DH:DH + 1].bitcast(F32), 1.0)
                nc.vector.memset(
                    vp_sb[b_][:, :, 2 * DH + 1:2 * DH + 2].bitcast(F32), 1.0)

            # ---------------- fused projections + attention ----------------
            # Projections run in t-layout (N=512 keeps fp32r at 1 cyc/row); V
            # is PE-transposed into natural layout for the ctx matmul. Batch
            # 0's projections form the prologue; batch 1's are interleaved
            # into batch 0's attention chunks to fill the PE slack while the
            # ScalarE exp pass (the bottleneck) runs. The attention itself is
            # software-pipelined at ki-chunk granularity: chunk i+1's score
            # matmuls interleave with chunk i's ctx matmuls.
            with (
                tc.tile_pool(name="xa", bufs=9) as xpool,
                tc.tile_pool(name="accp", bufs=2, space="PSUM") as accp,
                tc.tile_pool(name="slabp", bufs=19) as slabp,
                tc.tile_pool(name="smallp", bufs=2) as smallp,
                tc.tile_pool(name="coutp", bufs=2) as coutp,
                tc.tile_pool(name="pss", bufs=2, space="PSUM") as pss,
                tc.tile_pool(name="psc", bufs=1, space="PSUM") as psc,
            ):

                def proj_chunk(b_, o):
                    """Project one 512-token slice of batch b_ into qt/kt/vt.

                    Three sequential PSUM accumulation chains over a shared
                    single-slot pool tag keep the PSUM footprint at 2 banks.
                    """
                    tiles = []
                    for kc in range(KD):
                        xt_tile = xpool.tile([P, 512], F32R, name="xt_tile")
                        nc.sync.dma_start(
                            xt_tile[:],
                            xt[kc * P:(kc + 1) * P,
                               b_ * S + o * 512:b_ * S + (o + 1) * 512])
                        tiles.append(xt_tile)
                    for w_sb, dst in ((wq_sb, qt_sb[b_]), (wk_sb, kt_sb[b_]),
                                      (wv_sb, vt_sb[b_])):
                        acc = accp.tile([P, 512], F32, name="acc_ps")
                        for kc in range(KD):
                            _mm(nc, acc[:], w_sb[:, kc], tiles[kc][:],
                                start=(kc == 0), stop=(kc == KD - 1))
                        nc.vector.tensor_copy(
                            dst[:, o * 512:(o + 1) * 512], acc[:])

                def transp_chunk(b_, kc2):
                    """PE-transpose one [64,128] V^T block per head into V'."""
                    for hh in range(2):
                        tp = accp.tile([P, DH], F32R, name="acc_ps")
                        nc.tensor.transpose(
                            tp[:, :DH],
                            vt_sb[b_][hh * DH:(hh + 1) * DH,
                                      kc2 * P:(kc2 + 1) * P],
                            id_sb[hh * DH:(hh + 1) * DH, :])
                        nc.vector.tensor_copy(
                            vp_sb[b_][:, kc2,
                                      hh * (DH + 1):hh * (DH + 1) + DH],
                            tp[:, :DH])
                chunks = [(b_, o) for b_ in range(B) for o in range(S // 512)]
                state = {}

                def emit_scores(idx, kc):
                    b_, o = chunks[idx]
                    qs = slice(o * 512, (o + 1) * 512)
                    ks = slice(kc * P, (kc + 1) * P)
                    s_ps = pss.tile([P, 1024], F32, name="s_ps")
                    _mm(nc, s_ps[:, 0:512], kt_sb[b_][0:DH, ks],
                        qt_sb[b_][0:DH, qs], start=True, stop=True)
                    _mm(nc, s_ps[:, 512:1024], kt_sb[b_][DH:2 * DH, ks],
                        qt_sb[b_][DH:2 * DH, qs], start=True, stop=True)
                    slab = slabp.tile([P, 1024], F32R, name="slab")
                    nc.scalar.activation(slab[:], s_ps[:], AF.Exp, scale=0.125)
                    nc.vector.tensor_scalar_max(slab[:], slab[:], 1.0)
                    state[idx]["slabs"].append(slab)

                def emit_ctx(idx, kc):
                    b_, o = chunks[idx]
                    st_, sp_ = kc == 0, kc == KI - 1
                    c0, c1 = state[idx]["c0"], state[idx]["c1"]
                    slab = state[idx]["slabs"][kc]
                    _mm(nc, c0[:], vp_sb[b_][:, kc, 0:DH + 1], slab[:, 0:512],
                        start=st_, stop=sp_)
                    _mm(nc, c1[:], vp_sb[b_][:, kc, DH + 1:2 * DH + 2],
                        slab[:, 512:1024], start=st_, stop=sp_)

                def emit_normalize(idx):
                    b_, o = chunks[idx]
                    qs = slice(b_ * S + o * 512, b_ * S + (o + 1) * 512)
                    c0, c1 = state[idx]["c0"], state[idx]["c1"]
                    inv0 = smallp.tile([1, 512], F32, name="inv0")
                    inv1 = smallp.tile([1, 512], F32, name="inv1")
                    nc.vector.reciprocal(inv0[:], c0[DH:DH + 1, :])
                    nc.vector.reciprocal(inv1[:], c1[DH:DH + 1, :])
                    inv0b = smallp.tile([DH, 512], F32, name="inv0b")
                    inv1b = smallp.tile([DH, 512], F32, name="inv1b")
                    nc.gpsimd.partition_broadcast(inv0b[:], inv0[:])
                    nc.gpsimd.partition_broadcast(inv1b[:], inv1[:])
                    cout0 = coutp.tile([DH, 512], F32, name="cout0")
                    cout1 = coutp.tile([DH, 512], F32, name="cout1")
                    nc.vector.tensor_mul(cout0[:], c0[0:DH, :], inv0b[:])
                    nc.vector.tensor_mul(cout1[:], c1[0:DH, :], inv1b[:])
                    nc.sync.dma_start(ctx_t[0:DH, qs], cout0[:])
                    nc.sync.dma_start(ctx_t[DH:2 * DH, qs], cout1[:])
                    del state[idx]

                NO = S // 512   # 4 proj chunks per batch
                TPO = KI // NO  # 4 transposes per proj chunk
                # prologue: batch-0 projections with chunk-0 score matmuls
                # folded in per o-slice, so the ScalarE exp pass starts after
                # the first projection chunk (~7 us) instead of after all of
                # batch 0 (~28 us).
                state[0] = {
                    "c0": psc.tile([DH + 1, 512], F32, name="c0"),
                    "c1": psc.tile([DH + 1, 512], F32, name="c1"),
                    "slabs": [],
                }
                for o in range(NO):
                    proj_chunk(0, o)
                    for t in range(TPO):
                        transp_chunk(0, o * TPO + t)
                    for kc in range(o * TPO, (o + 1) * TPO):
                        emit_scores(0, kc)
                # attention, with batch-1 projections/transposes interleaved
                # into batch-0's chunks (idx 1..4)
                for idx in range(1, len(chunks)):
                    state[idx] = {
                        "c0": psc.tile([DH + 1, 512], F32, name="c0"),
                        "c1": psc.tile([DH + 1, 512], F32, name="c1"),
                        "slabs": [],
                    }
                    if idx <= NO:
                        proj_chunk(1, idx - 1)
                        for t in range(TPO):
                            transp_chunk(1, (idx - 1) * TPO + t)
                    for kc in range(KI):
                        emit_scores(idx, kc)
                        emit_ctx(idx - 1, kc)
                    emit_normalize(idx - 1)
                last = len(chunks) - 1
                for kc in range(KI):
                    emit_ctx(last, kc)
                emit_normalize(last)
    nc.compile()
    return nc


# --------------------------------------------------------------------------
# Phase B: per-core row-parallel Wo-proj + AddNorm1 + FFN + AddNorm2.
# Inputs (per core, qi = this core's 512 token rows):
#   ct  [D, QC]    ctx^T slice
#   xts [D, QC]    X^T slice (residual 1)
#   wot [D, D]     Wo^T
#   w1t [D, FF]    W1^T
#   w2t [FF, D]    W2^T
#   g1,be1,g2,be2 [P, D//P]  ln params, feature-on-partition layout
#   b1t [P, FF//P], b2t [P, D//P]
# Output: out_t [D, QC]
# --------------------------------------------------------------------------
def _build_phase_b():
    nc = bacc.Bacc("TRN2", target_bir_lowering=False, debug=False,
                   num_devices=N_CORES)
    ct = nc.dram_tensor("ct", [D, QC], F32R, kind="ExternalInput")
    xts = nc.dram_tensor("xts", [D, QC], F32, kind="ExternalInput")
    wot = nc.dram_tensor("wot", [D, D], F32R, kind="ExternalInput")
    w1t = nc.dram_tensor("w1t", [D, FF], F32R, kind="ExternalInput")
    w2t = nc.dram_tensor("w2t", [FF, D], F32R, kind="ExternalInput")
    g1 = nc.dram_tensor("g1", [P, D // P], F32, kind="ExternalInput")
    be1 = nc.dram_tensor("be1", [P, D // P], F32, kind="ExternalInput")
    g2 = nc.dram_tensor("g2", [P, D // P], F32, kind="ExternalInput")
    be2 = nc.dram_tensor("be2", [P, D // P], F32, kind="ExternalInput")
    b1t = nc.dram_tensor("b1t", [P, FF // P], F32, kind="ExternalInput")
    b2t = nc.dram_tensor("b2t", [P, D // P], F32, kind="ExternalInput")
    out_t = nc.dram_tensor("out_t", [D, QC], F32, kind="ExternalOutput")

    KD = D // P     # 8
    KF = FF // P    # 32

    def layernorm(nc, tc, pools, y_sb, g_sb, be_sb, z_sb, ones, tag):
        """t-layout layernorm: y_sb [P, KD, 512] -> z_sb (may alias layout)."""
        smallp, sqp, bcp = pools
        import contextlib
        ctx = contextlib.ExitStack()
        psst = ctx.enter_context(
            tc.tile_pool(name=f"psst_{tag}", bufs=1, space="PSUM"))
        st_ps = psst.tile([1, 1024], F32, name="st_ps")
        for kc in range(KD):
            _mm(nc, st_ps[:, 0:512], ones[:], y_sb[:, kc],
                             start=(kc == 0), stop=(kc == KD - 1))
        for kc in range(KD):
            sq = sqp.tile([P, 512], F32R, name="sq")
            nc.vector.tensor_mul(sq[:], y_sb[:, kc], y_sb[:, kc])
            _mm(nc, st_ps[:, 512:1024], ones[:], sq[:],
                             start=(kc == 0), stop=(kc == KD - 1))
        stats = smallp.tile([1, 1024], F32, name="stats")
        nc.vector.tensor_scalar(out=stats[:], in0=st_ps[:], scalar1=1.0 / D,
                                scalar2=None, op0=OP.mult)
        mean = stats[:, 0:512]
        ex2 = stats[:, 512:1024]
        msq = smallp.tile([1, 512], F32, name="msq")
        nc.vector.tensor_mul(msq[:], mean, mean)
        var = smallp.tile([1, 512], F32, name="var")
        nc.vector.tensor_sub(var[:], ex2, msq[:])
        nc.vector.tensor_scalar_add(var[:], var[:], EPS)
        std = smallp.tile([1, 512], F32, name="std")
        nc.scalar.activation(std[:], var[:], AF.Sqrt)
        rstd = smallp.tile([1, 512], F32, name="rstd")
        nc.vector.reciprocal(rstd[:], std[:])
        ms = smallp.tile([1, 512], F32, name="ms")
        nc.vector.tensor_mul(ms[:], mean, rstd[:])
        rstd_b = bcp.tile([P, 512], F32, name="rstd_b")
        ms_b = bcp.tile([P, 512], F32, name="ms_b")
        nc.gpsimd.partition_broadcast(rstd_b[:], rstd[:])
        nc.gpsimd.partition_broadcast(ms_b[:], ms[:])
        for kc in range(KD):
            t = sqp.tile([P, 512], F32, name="t_ln")
            nc.vector.tensor_mul(t[:], y_sb[:, kc], rstd_b[:])
            nc.vector.tensor_sub(t[:], t[:], ms_b[:])
            nc.vector.tensor_scalar(out=z_sb[:, kc], in0=t[:],
                                    scalar1=g_sb[:, kc:kc + 1],
                                    scalar2=be_sb[:, kc:kc + 1],
                                    op0=OP.mult, op1=OP.add)
        ctx.close()

    with tile.TileContext(nc) as tc:
        with (
            tc.tile_pool(name="persist", bufs=1) as persist,
            tc.tile_pool(name="wp", bufs=6) as wp,
            tc.tile_pool(name="sqp", bufs=3) as sqp,
            tc.tile_pool(name="smallp", bufs=1) as smallp,
            tc.tile_pool(name="bcp", bufs=2) as bcp,
        ):
            ct_sb = persist.tile([P, KD, QC], F32R)
            xts_sb = persist.tile([P, KD, QC], F32)
            y1_sb = persist.tile([P, KD, QC], F32R)
            z1_sb = persist.tile([P, KD, QC], F32R)
            h_sb = persist.tile([P, KF, QC], F32R)
            # y2 reuses y1's slot (y1 dead after LN1); z2 reuses ct's (dead
            # after B1). Tag sharing makes Tile serialize via WAR edges.
            y2_sb = persist.tile([P, KD, QC], F32R, tag="y1_sb")
            z2_sb = persist.tile([P, KD, QC], F32, tag="ct_sb")
            g1_sb = persist.tile([P, KD], F32)
            be1_sb = persist.tile([P, KD], F32)
            g2_sb = persist.tile([P, KD], F32)
            be2_sb = persist.tile([P, KD], F32)
            b1t_sb = persist.tile([P, KF], F32)
            b2t_sb = persist.tile([P, KD], F32)
            ones = persist.tile([P, 1], F32R)

            nc.sync.dma_start(ct_sb[:], ct.ap().rearrange("(kc p) q -> p kc q", p=P))
            nc.sync.dma_start(xts_sb[:], xts.ap().rearrange("(kc p) q -> p kc q", p=P))
            for t_sb, t_dr in ((g1_sb, g1), (be1_sb, be1), (g2_sb, g2),
                               (be2_sb, be2), (b1t_sb, b1t), (b2t_sb, b2t)):
                nc.sync.dma_start(t_sb[:], t_dr.ap())
            nc.vector.memset(ones[:].bitcast(F32), 1.0)

            # ---- B1: att_out = Wo @ ct (+ X residual) ----
            with tc.tile_pool(name="psa", bufs=1, space="PSUM") as psa:
                for mg in range(2):
                    a_ps = [psa.tile([P, 512], F32, name=f"mm_ps{i}")
                            for i in range(4)]
                    for kc in range(KD):
                        w_tile = wp.tile([P, 512], F32R, name="wo_tile")
                        nc.sync.dma_start(
                            w_tile[:],
                            wot[kc * P:(kc + 1) * P, mg * 512:(mg + 1) * 512])
                        for i in range(4):
                            _mm(nc, a_ps[i][:],
                                w_tile[:, i * P:(i + 1) * P], ct_sb[:, kc],
                                start=(kc == 0), stop=(kc == KD - 1))
                    for i in range(4):
                        m = mg * 4 + i
                        nc.vector.tensor_add(y1_sb[:, m], a_ps[i][:], xts_sb[:, m])

                # ---- LN1 ----
                layernorm(nc, tc, (smallp, sqp, bcp), y1_sb, g1_sb, be1_sb,
                          z1_sb, ones, "ln1")

            # ---- FFN1 + FFN2: the first output half of FFN2 (mg0) is
            # interleaved into the FFN1 loop so W2 @ h starts consuming h
            # chunks as soon as they exist; tile sizes stay [128,512] so the
            # DMA instruction count is unchanged. 8 PSUM banks: 4 h + 4 f. ----
            with tc.tile_pool(name="psa2", bufs=1, space="PSUM") as psa2:
                f_ps = [psa2.tile([P, 512], F32, name=f"f_ps{i}")
                        for i in range(4)]
                for fg in range(KF // 4):
                    h_ps = [psa2.tile([P, 512], F32, name=f"h_ps{i}")
                            for i in range(4)]
                    for kc in range(KD):
                        w_tile = wp.tile([P, 512], F32R, name="w1_tile")
                        nc.sync.dma_start(
                            w_tile[:],
                            w1t[kc * P:(kc + 1) * P, fg * 512:(fg + 1) * 512])
                        for i in range(4):
                            _mm(nc, h_ps[i][:],
                                w_tile[:, i * P:(i + 1) * P], z1_sb[:, kc],
                                start=(kc == 0), stop=(kc == KD - 1))
                    for i in range(4):
                        fm = fg * 4 + i
                        nc.scalar.activation(h_sb[:, fm], h_ps[i][:], AF.Relu,
                                             bias=b1t_sb[:, fm:fm + 1])
                    for i in range(4):
                        fk = fg * 4 + i
                        w_tile = wp.tile([P, 512], F32R, name="w2_tile")
                        nc.sync.dma_start(
                            w_tile[:], w2t[fk * P:(fk + 1) * P, 0:512])
                        for j in range(4):
                            _mm(nc, f_ps[j][:],
                                w_tile[:, j * P:(j + 1) * P], h_sb[:, fk],
                                start=(fk == 0), stop=(fk == KF - 1))
                for j in range(4):
                    nc.vector.scalar_tensor_tensor(
                        out=y2_sb[:, j], in0=f_ps[j][:],
                        scalar=b2t_sb[:, j:j + 1], in1=z1_sb[:, j],
                        op0=OP.add, op1=OP.add)

                f_ps2 = [psa2.tile([P, 512], F32, name=f"f_ps{i}")
                         for i in range(4)]
                for fk in range(KF):
                    w_tile = wp.tile([P, 512], F32R, name="w2_tile")
                    nc.sync.dma_start(
                        w_tile[:], w2t[fk * P:(fk + 1) * P, 512:1024])
                    for j in range(4):
                        _mm(nc, f_ps2[j][:],
                            w_tile[:, j * P:(j + 1) * P], h_sb[:, fk],
                            start=(fk == 0), stop=(fk == KF - 1))
                for j in range(4):
                    m = 4 + j
                    nc.vector.scalar_tensor_tensor(
                        out=y2_sb[:, m], in0=f_ps2[j][:],
                        scalar=b2t_sb[:, m:m + 1], in1=z1_sb[:, m],
                        op0=OP.add, op1=OP.add)

            # ---- LN2 ----
            layernorm(nc, tc, (smallp, sqp, bcp), y2_sb, g2_sb, be2_sb,
                      z2_sb, ones, "ln2")

            for kc in range(KD):
                nc.sync.dma_start(out_t[kc * P:(kc + 1) * P, :], z2_sb[:, kc])
    nc.compile()
    return nc


def _get(name, builder):
    if name not in _CACHE:
        _CACHE[name] = builder()
    return _CACHE[name]


def _prep_inputs(X, Wq, Wk, Wo, ln1_g, ln1_b, ln2_g, ln2_b, W1, b1, W2, b2):
    """Host-side sharding/layout. Returns (in_maps_a, in_maps_b_builder, Xt)."""
    f = lambda a: np.ascontiguousarray(np.asarray(a, dtype=np.float32))
    Xt = f(np.asarray(X, np.float32).reshape(N, D).T)        # [D, N]
    WqT, WkT, WoT = f(np.asarray(Wq).T), f(np.asarray(Wk).T), f(np.asarray(Wo).T)
    W1T, W2T = f(np.asarray(W1).T), f(np.asarray(W2).T)      # [D,FF], [FF,D]
    vecP = lambda v, k: f(np.asarray(v).reshape(k, P).T)     # [P, k]
    g1v, be1v = vecP(ln1_g, D // P), vecP(ln1_b, D // P)
    g2v, be2v = vecP(ln2_g, D // P), vecP(ln2_b, D // P)
    b1v, b2v = vecP(b1, FF // P), vecP(b2, D // P)

    idm = np.tile(np.eye(DH, dtype=np.float32), (2, 1))   # [128, 64]
    in_maps_a = [
        {
            "xt": Xt,
            "idm": idm,
            "wqt": f(WqT[:, c * P:(c + 1) * P]),
            "wkt": f(WkT[:, c * P:(c + 1) * P]),
            "wvt": f(WoT[:, c * P:(c + 1) * P]),
        }
        for c in range(N_CORES)
    ]

    def in_maps_b(ct_full):
        return [
            {
                "ct": f(ct_full[:, c * QC:(c + 1) * QC]),
                "xts": f(Xt[:, c * QC:(c + 1) * QC]),
                "wot": WoT, "w1t": W1T, "w2t": W2T,
                "g1": g1v, "be1": be1v, "g2": g2v, "be2": be2v,
                "b1t": b1v, "b2t": b2v,
            }
            for c in range(N_CORES)
        ]

    return in_maps_a, in_maps_b


def kernel(X, Wq, Wk, Wo, ln1_g, ln1_b, ln2_g, ln2_b, W1, b1, W2, b2):
    in_maps_a, in_maps_b = _prep_inputs(
        X, Wq, Wk, Wo, ln1_g, ln1_b, ln2_g, ln2_b, W1, b1, W2, b2)

    nc_a = _get("a", _build_phase_a)
    res_a = run_bass_kernel_spmd(nc_a, in_maps_a, core_ids=list(range(N_CORES)))
    ct_full = np.concatenate(
        [res_a.results[c]["ctx_t"] for c in range(N_CORES)], axis=0)  # [D, N]

    nc_b = _get("b", _build_phase_b)
    res_b = run_bass_kernel_spmd(nc_b, in_maps_b(ct_full),
                                 core_ids=list(range(N_CORES)))
    out_t = np.concatenate(
        [res_b.results[c]["out_t"] for c in range(N_CORES)], axis=1)  # [D, N]
    return np.ascontiguousarray(out_t.T).reshape(B, S, D).astype(np.float32)



# revision 17
# speedup vs baseline: 1.0554x; 1.0554x over previous
"""Trainium2 Bass kernel for nn_Encoder (dense transformer block), 8 NeuronCores.

Strategy (single chip, 8 cores), v3:
  Phase A (head-parallel): core c computes attention for heads {2c, 2c+1}.
    Projections run in t-layout; q/k land in bf16, V is PE-transposed into
    natural [keys, dims] bf16 layout. softmax(relu(s)) is p = max(exp(s/8), 1)
    with the softmax denominator taken from a ones column appended to V.
    The exp pass on the Activation engine is the critical resource (~134 us);
    a queue-based emitter keeps it saturated: score matmuls are emitted as
    early as their projections allow (wavefront), projection matmuls are
    spread between them in small pieces, and the context-accumulation chains
    (65-cycle bf16 matmuls in the fast [q,65] orientation) fill the PE's
    exp-paced slack. ctx leaves phase A in natural [token, dim] layout.
  Phase B (row-parallel): core c takes 512 of the 4096 token rows. It
    PE-transposes the incoming ctx back to t-layout fused with the Wo
    matmuls, then AddNorm1, FFN (ReLU, bf16 weights/activations), AddNorm2.
    All weights stream as a handful of large host-pre-tiled bf16 DMAs on the
    SP queue; LayerNorm statistics are accumulated in halves so their matmuls
    and squares overlap the surrounding GEMMs.
"""

import os
import sys

for _p in ("/opt/trn_rl_repo",):
    if _p not in sys.path:
        sys.path.insert(0, _p)

# The Bass SPMD path executes through jax/PJRT on the axon platform; make
# sure a caller-pinned JAX_PLATFORMS=cpu doesn't hide the NeuronCores.
_jp = os.environ.get("JAX_PLATFORMS")
if _jp is not None and "axon" not in _jp:
    os.environ["JAX_PLATFORMS"] = "axon," + _jp

import ml_dtypes
import numpy as np

import concourse.bass as bass
import concourse.mybir as mybir
import concourse.tile as tile
from concourse import bacc
from concourse.bass_utils import run_bass_kernel_spmd

F32 = mybir.dt.float32
F32R = mybir.dt.float32r
BF16 = mybir.dt.bfloat16
AF = mybir.ActivationFunctionType
OP = mybir.AluOpType
BF = ml_dtypes.bfloat16


def _mm(nc, out, lhsT, rhs, **kw):
    # fp32r: 1-pass FP22 matmul (1 cyc/row when the moving dim is >= 256)
    nc.tensor.matmul(out, lhsT.bitcast(F32R), rhs.bitcast(F32R), **kw)


N_CORES = 8
B, S, D, H, DH, FF = 2, 2048, 1024, 16, 64, 4096
N = B * S            # 4096 token rows
P = 128
QC = N // N_CORES    # 512 rows per core in phase B
KD = D // P          # 8 contraction chunks over D
KI = S // P          # 16 key chunks of 128 per batch
NO = S // 512        # 4 query chunks of 512 per batch
NQ = N // 512        # 8 query chunks overall
KF = FF // P         # 32
EPS = 1e-5

_CACHE = {}


# --------------------------------------------------------------------------
# Phase A: per-core head-parallel attention.
# Inputs (per core):
#   xt8  [NQ, P, KD, 512]  X^T tiled per 512-token chunk (replicated)
#   wqt8/wkt8/wvt8 [P, KD, P]  W^T columns for this core's two heads, tiled
#   id64b [P, DH] bf16 tiled identity (V transposes)
# Output:
#   ctxn [P, NQ, 4, P] f32: natural-layout ctx; token = idx*512 + j*128 + p,
#   col = the two heads' 64-dim blocks concatenated.
# --------------------------------------------------------------------------
def _build_phase_a():
    nc = bacc.Bacc("TRN2", target_bir_lowering=False, debug=False,
                   num_devices=N_CORES)
    xt8 = nc.dram_tensor("xt8", [NQ, P, KD, 512], F32R, kind="ExternalInput")
    wqt8 = nc.dram_tensor("wqt8", [P, KD, P], F32R, kind="ExternalInput")
    wkt8 = nc.dram_tensor("wkt8", [P, KD, P], F32R, kind="ExternalInput")
    wvt8 = nc.dram_tensor("wvt8", [P, KD, P], F32R, kind="ExternalInput")
    id64b = nc.dram_tensor("id64b", [P, DH], BF16, kind="ExternalInput")
    ctxn = nc.dram_tensor("ctxn", [P, NQ, 4, P], F32, kind="ExternalOutput")

    chunks = [(b_, o) for b_ in range(B) for o in range(NO)]

    with tile.TileContext(nc) as tc:
        with tc.tile_pool(name="persist", bufs=1) as persist:
            qt_sb = [persist.tile([P, S], BF16, name=f"qt{b_}") for b_ in range(B)]
            kt_sb = [persist.tile([P, S], BF16, name=f"kt{b_}") for b_ in range(B)]
            vt_sb = [persist.tile([P, S], BF16, name=f"vt{b_}") for b_ in range(B)]
            vp_sb = [persist.tile([P, KI, 2, DH + 1], BF16, name=f"vp{b_}")
                     for b_ in range(B)]
            wq_sb = persist.tile([P, KD, P], F32R)
            wk_sb = persist.tile([P, KD, P], F32R)
            wv_sb = persist.tile([P, KD, P], F32R)
            id64_sb = persist.tile([P, DH], BF16)

            nc.sync.dma_start(wq_sb[:], wqt8.ap())
            nc.sync.dma_start(wk_sb[:], wkt8.ap())
            nc.sync.dma_start(wv_sb[:], wvt8.ap())
            nc.sync.dma_start(id64_sb[:], id64b.ap())
            for b_ in range(B):
                nc.vector.memset(vp_sb[b_][:, :, 0, DH:DH + 1], 1.0)
                nc.vector.memset(vp_sb[b_][:, :, 1, DH:DH + 1], 1.0)

            with (
                tc.tile_pool(name="xpool", bufs=2) as xpool,
                tc.tile_pool(name="accp", bufs=2, space="PSUM") as accp,
                tc.tile_pool(name="slabp", bufs=50) as slabp,
                tc.tile_pool(name="stagep", bufs=2) as stagep,
                tc.tile_pool(name="smallp", bufs=8) as smallp,
                tc.tile_pool(name="pss", bufs=2, space="PSUM") as pss,
                tc.tile_pool(name="psc", bufs=2, space="PSUM") as psc,
            ):
                xt_tiles = {}

                def issue_xt(ci):
                    t = xpool.tile([P, KD, 512], F32R, name="xt_tile")
                    nc.sync.dma_start(t[:], xt8[ci])
                    xt_tiles[ci] = t

                def gen_proj(ci):
                    """Generator: project chunk ci into qt/kt/vt (bf16) and
                    vp, yielding between small matmul pieces."""
                    b_, o = chunks[ci]
                    osl = slice(o * 512, (o + 1) * 512)
                    xt_tile = xt_tiles.pop(ci)
                    for w_sb, dst in ((wq_sb, qt_sb[b_]), (wk_sb, kt_sb[b_]),
                                      (wv_sb, vt_sb[b_])):
                        acc = accp.tile([P, 512], F32, name="acc_ps", tag="acc")
                        for kc in range(KD):
                            _mm(nc, acc[:], w_sb[:, kc], xt_tile[:, kc],
                                start=(kc == 0), stop=(kc == KD - 1))
                            if kc % 2 == 1:
                                yield
                        nc.vector.tensor_copy(dst[:, osl], acc[:])
                        yield
                    for t in range(4):
                        kc2 = o * 4 + t
                        for hh in range(2):
                            tp = accp.tile([P, DH], BF16, name="tp_ps",
                                           tag="acc")
                            nc.tensor.transpose(
                                tp[:, 0:DH],
                                vt_sb[b_][hh * DH:(hh + 1) * DH,
                                          kc2 * P:(kc2 + 1) * P],
                                id64_sb[hh * DH:(hh + 1) * DH, :])
                            nc.vector.tensor_copy(
                                vp_sb[b_][:, kc2, hh, 0:DH], tp[:, 0:DH])
                        yield

                slabs = {i: {} for i in range(NQ)}   # idx -> kc -> slab tile
                stages = {}

                def emit_scores(idx, kc):
                    b_, o = chunks[idx]
                    qs = slice(o * 512, (o + 1) * 512)
                    ks = slice(kc * P, (kc + 1) * P)
                    s_ps = pss.tile([P, 1024], F32, name="s_ps")
                    nc.tensor.matmul(s_ps[:, 0:512], kt_sb[b_][0:DH, ks],
                                     qt_sb[b_][0:DH, qs], start=True, stop=True)
                    nc.tensor.matmul(s_ps[:, 512:1024], kt_sb[b_][DH:2 * DH, ks],
                                     qt_sb[b_][DH:2 * DH, qs],
                                     start=True, stop=True)
                    slab = slabp.tile([P, 1024], BF16, name="slab")
                    nc.scalar.activation(slab[:], s_ps[:], AF.Exp, scale=0.125)
                    nc.vector.tensor_scalar_max(slab[:], slab[:], 1.0)
                    slabs[idx][kc] = slab

                def gen_chains(idx):
                    """Generator: the 8 ctx chains of idx + normalize + DMA,
                    yielding every couple of matmuls."""
                    b_, o = chunks[idx]
                    stage = stagep.tile([P, 4, P], F32, name="stage")
                    for ci in range(8):
                        j, h = ci // 2, ci % 2
                        acc = psc.tile([P, DH + 1], F32, name="ctx_ps")
                        for kc in range(KI):
                            nc.tensor.matmul(
                                acc[:],
                                slabs[idx][kc][:, h * 512 + j * P:
                                               h * 512 + (j + 1) * P],
                                vp_sb[b_][:, kc, h, :],
                                start=(kc == 0), stop=(kc == KI - 1))
                            if kc % 4 == 3:
                                yield
                        inv = smallp.tile([P, 1], F32, name="inv")
                        nc.vector.reciprocal(inv[:], acc[:, DH:DH + 1])
                        nc.vector.tensor_scalar(
                            out=stage[:, j, h * DH:(h + 1) * DH],
                            in0=acc[:, 0:DH], scalar1=inv[:], scalar2=None,
                            op0=OP.mult)
                        yield
                    nc.sync.dma_start(ctxn[:, idx], stage[:])
                    slabs[idx].clear()

                # ---------------- queue-based emitter ----------------
                emitted = set()           # (idx, kc) scores emitted
                score_q = []              # ordered pending scores
                proj_done = [False] * NQ
                chains_done = 0           # count of fully-emitted chain idxs
                chain_gen = None
                chain_idx = 0             # next idx needing chains
                proj_idx = 0              # next proj chunk to drive
                pgen = None

                def update_score_q():
                    for i in range(NQ):
                        bi, _ = chunks[i]
                        if not proj_done[i]:
                            continue
                        if i >= chains_done + 3:
                            continue
                        base = 4 * bi
                        kmax = sum(4 for c in range(base, base + NO)
                                   if proj_done[c])
                        for k in range(kmax):
                            if (i, k) not in emitted and (i, k) not in score_q:
                                score_q.append((i, k))

                issue_xt(0)
                issue_xt(1)
                while (proj_idx < NQ or pgen is not None or score_q
                       or chain_idx < NQ or chain_gen is not None
                       or len(emitted) < NQ * KI):
                    # 1. a slice of chain work (PE filler, no Act dependency)
                    if chain_gen is None and chain_idx < NQ:
                        if len(slabs[chain_idx]) == KI and all(
                                (chain_idx, k) in emitted for k in range(KI)):
                            chain_gen = gen_chains(chain_idx)
                    if chain_gen is not None:
                        for _ in range(3):
                            try:
                                next(chain_gen)
                            except StopIteration:
                                chain_gen = None
                                chain_idx += 1
                                chains_done += 1
                                update_score_q()
                                break
                    # 2. projection pieces
                    if pgen is None and proj_idx < NQ:
                        pgen = gen_proj(proj_idx)
                    if pgen is not None:
                        steps = 1 if score_q else 4
                        for _ in range(steps):
                            try:
                                next(pgen)
                            except StopIteration:
                                proj_done[proj_idx] = True
                                proj_idx += 1
                                if proj_idx + 1 < NQ:
                                    issue_xt(proj_idx + 1)
                                pgen = None
                                update_score_q()
                                break
                    # 3. one score (the Act engine's food)
                    if score_q:
                        i, k = score_q.pop(0)
                        emit_scores(i, k)
                        emitted.add((i, k))
                        update_score_q()
    nc.compile()
    return nc


# --------------------------------------------------------------------------
# Phase B: per-core row-parallel transpose + Wo-proj + AddNorm1 + FFN + AddNorm2.
# Inputs (per core, qi = this core's 512 token rows):
#   ctin [P, 4, KD, P] bf16   natural-layout ctx blocks for these rows
#   wo8  [P, KD, D]    bf16   Wo^T tiled
#   w18  [KD, P, KD, 512] bf16  W1^T tiled per 512-wide ffn-col group
#   w2a8/w2b8 [KD, P, 4, 512] bf16  W2^T tiled, first/second output half
#   xts8 [P, KD, 512] f32     X^T slice (residual 1)
#   consts [P, 72] f32        g1|be1|g2|be2|b1t|b2t feature-on-partition
#   id128b [P, P] bf16
# Output: out8 [P, KD, 512] f32 (t-layout output slice, tiled)
# --------------------------------------------------------------------------
def _build_phase_b():
    nc = bacc.Bacc("TRN2", target_bir_lowering=False, debug=False,
                   num_devices=N_CORES)
    ctin = nc.dram_tensor("ctin", [P, 4, KD, P], BF16, kind="ExternalInput")
    wo8 = nc.dram_tensor("wo8", [P, KD, D], BF16, kind="ExternalInput")
    w18 = nc.dram_tensor("w18", [KD, P, KD, 512], BF16, kind="ExternalInput")
    w2a8 = nc.dram_tensor("w2a8", [KD, P, 4, 512], BF16, kind="ExternalInput")
    w2b8 = nc.dram_tensor("w2b8", [KD, P, 4, 512], BF16, kind="ExternalInput")
    xts8 = nc.dram_tensor("xts8", [P, KD, 512], F32, kind="ExternalInput")
    consts = nc.dram_tensor("consts", [P, 72], F32, kind="ExternalInput")
    id128b = nc.dram_tensor("id128b", [P, P], BF16, kind="ExternalInput")
    out8 = nc.dram_tensor("out8", [P, KD, 512], F32, kind="ExternalOutput")

    with tile.TileContext(nc) as tc:
        with (
            tc.tile_pool(name="persist", bufs=1) as persist,
            tc.tile_pool(name="w1p", bufs=3) as w1p,
            tc.tile_pool(name="w2p", bufs=3) as w2p,
            tc.tile_pool(name="sqp", bufs=3) as sqp,
            tc.tile_pool(name="smallp", bufs=2) as smallp,
            tc.tile_pool(name="bcp", bufs=2) as bcp,
        ):
            ctin_sb = persist.tile([P, 4, KD, P], BF16)
            ct_sb = persist.tile([P, KD, 4, P], BF16)
            wo_sb = persist.tile([P, KD, D], BF16)
            xts_sb = persist.tile([P, KD, 512], F32)
            y1_sb = persist.tile([P, KD, 512], F32R)
            z1_sb = persist.tile([P, KD, 512], BF16)
            h_sb = persist.tile([P, KF, 512], BF16)
            y2_sb = persist.tile([P, KD, 512], F32R, tag="y1_sb")
            z2_sb = persist.tile([P, KD, 512], F32, tag="xts_sb")
            consts_sb = persist.tile([P, 72], F32)
            id128_sb = persist.tile([P, P], BF16)
            ones = persist.tile([P, 1], F32R)

            nc.sync.dma_start(ctin_sb[:], ctin.ap())
            nc.sync.dma_start(id128_sb[:], id128b.ap())
            nc.sync.dma_start(consts_sb[:], consts.ap())
            nc.sync.dma_start(wo_sb[:, 0:4], wo8[:, 0:4])
            nc.sync.dma_start(wo_sb[:, 4:8], wo8[:, 4:8])
            nc.sync.dma_start(xts_sb[:], xts8.ap())
            nc.vector.memset(ones[:].bitcast(F32), 1.0)

            g1 = [consts_sb[:, kc:kc + 1] for kc in range(KD)]
            be1 = [consts_sb[:, 8 + kc:9 + kc] for kc in range(KD)]
            g2 = [consts_sb[:, 16 + kc:17 + kc] for kc in range(KD)]
            be2 = [consts_sb[:, 24 + kc:25 + kc] for kc in range(KD)]
            b1c = [consts_sb[:, 32 + fm:33 + fm] for fm in range(KF)]
            b2c = [consts_sb[:, 64 + kc:65 + kc] for kc in range(KD)]

            def ln_sums(st_sum, st_sq, y_sb, kcs):
                """Partial LN stats for feature chunks kcs of y_sb."""
                for kc in kcs:
                    _mm(nc, st_sum[:], ones[:], y_sb[:, kc],
                        start=(kc == 0), stop=(kc == KD - 1))
                for kc in kcs:
                    sq = sqp.tile([P, 512], F32R, name="sq")
                    nc.vector.tensor_mul(sq[:], y_sb[:, kc], y_sb[:, kc])
                    _mm(nc, st_sq[:], ones[:], sq[:],
                        start=(kc == 0), stop=(kc == KD - 1))

            def ln_finish(st_sum, st_sq, tag):
                """Stats -> (rstd_b, ms_b) broadcast tiles."""
                mean = smallp.tile([1, 512], F32, name="mean")
                ex2 = smallp.tile([1, 512], F32, name="ex2")
                nc.vector.tensor_scalar(out=mean[:], in0=st_sum[:],
                                        scalar1=1.0 / D, scalar2=None,
                                        op0=OP.mult)
                nc.vector.tensor_scalar(out=ex2[:], in0=st_sq[:],
                                        scalar1=1.0 / D, scalar2=None,
                                        op0=OP.mult)
                msq = smallp.tile([1, 512], F32, name="msq")
                nc.vector.tensor_mul(msq[:], mean[:], mean[:])
                var = smallp.tile([1, 512], F32, name="var")
                nc.vector.tensor_sub(var[:], ex2[:], msq[:])
                nc.vector.tensor_scalar_add(var[:], var[:], EPS)
                std = smallp.tile([1, 512], F32, name="std")
                nc.scalar.activation(std[:], var[:], AF.Sqrt)
                rstd = smallp.tile([1, 512], F32, name="rstd")
                nc.vector.reciprocal(rstd[:], std[:])
                ms = smallp.tile([1, 512], F32, name="ms")
                nc.vector.tensor_mul(ms[:], mean[:], rstd[:])
                rstd_b = bcp.tile([P, 512], F32, name="rstd_b")
                ms_b = bcp.tile([P, 512], F32, name="ms_b")
                nc.gpsimd.partition_broadcast(rstd_b[:], rstd[:])
                nc.gpsimd.partition_broadcast(ms_b[:], ms[:])
                return rstd_b, ms_b

            def ln_apply(y_sb, rstd_b, ms_b, g_c, be_c, z_sb, kc):
                t = sqp.tile([P, 512], F32, name="t_ln")
                nc.vector.tensor_mul(t[:], y_sb[:, kc], rstd_b[:])
                nc.vector.tensor_sub(t[:], t[:], ms_b[:])
                nc.vector.tensor_scalar(out=z_sb[:, kc], in0=t[:],
                                        scalar1=g_c[kc], scalar2=be_c[kc],
                                        op0=OP.mult, op1=OP.add)

            # ---- B0+B1: transpose ctx to t-layout, fused with Wo matmuls ----
            with (
                tc.tile_pool(name="tpp", bufs=2, space="PSUM") as tpp,
                tc.tile_pool(name="psa", bufs=1, space="PSUM") as psa,
                tc.tile_pool(name="psst1", bufs=1, space="PSUM") as psst1,
            ):
                st1_sum = psst1.tile([1, 512], F32, name="st1_sum")
                st1_sq = psst1.tile([1, 512], F32, name="st1_sq")
                a_ps = [psa.tile([P, 512], F32, name=f"mm_ps{i}")
                        for i in range(4)]
                for kc in range(KD):
                    tp = tpp.tile([P, 4, P], BF16, name="tp_ps")
                    for jb in range(4):
                        nc.tensor.transpose(tp[:, jb, :],
                                            ctin_sb[:, jb, kc, :],
                                            id128_sb[:])
                    nc.vector.tensor_copy(ct_sb[:, kc], tp[:])
                    for i in range(4):
                        nc.tensor.matmul(
                            a_ps[i][:], wo_sb[:, kc, i * P:(i + 1) * P],
                            ct_sb[:, kc], start=(kc == 0), stop=(kc == KD - 1))
                for i in range(4):
                    nc.vector.tensor_add(y1_sb[:, i], a_ps[i][:], xts_sb[:, i])
                a_ps2 = [psa.tile([P, 512], F32, name=f"mm_ps{i}")
                         for i in range(4)]
                for kc in range(KD):
                    for i in range(4):
                        nc.tensor.matmul(
                            a_ps2[i][:],
                            wo_sb[:, kc, 512 + i * P:512 + (i + 1) * P],
                            ct_sb[:, kc], start=(kc == 0), stop=(kc == KD - 1))
                # first-half LN1 stats run on the PE behind the mg1 matmuls,
                # overlapping the DVE residual adds
                ln_sums(st1_sum, st1_sq, y1_sb, range(0, 4))
                for i in range(4):
                    m = 4 + i
                    nc.vector.tensor_add(y1_sb[:, m], a_ps2[i][:], xts_sb[:, m])
                ln_sums(st1_sum, st1_sq, y1_sb, range(4, 8))
                rstd_b1, ms_b1 = ln_finish(st1_sum, st1_sq, "ln1")
                for kc in range(KD):
                    ln_apply(y1_sb, rstd_b1, ms_b1, g1, be1, z1_sb, kc)

            # ---- FFN1 + FFN2 (first output half interleaved) ----
            with tc.tile_pool(name="psa2", bufs=1, space="PSUM") as psa2:
                f_ps = [psa2.tile([P, 512], F32, name=f"f_ps{i}")
                        for i in range(4)]
                for fg in range(KD):
                    w1_tile = w1p.tile([P, KD, 512], BF16, name="w1_tile")
                    nc.sync.dma_start(w1_tile[:], w18[fg])
                    h_ps = [psa2.tile([P, 512], F32, name=f"h_ps{i}")
                            for i in range(4)]
                    for kc in range(KD):
                        for i in range(4):
                            nc.tensor.matmul(
                                h_ps[i][:], w1_tile[:, kc, i * P:(i + 1) * P],
                                z1_sb[:, kc], start=(kc == 0), stop=(kc == KD - 1))
                    for i in range(4):
                        fm = fg * 4 + i
                        nc.scalar.activation(h_sb[:, fm], h_ps[i][:], AF.Relu,
                                             bias=b1c[fm])
                    w2_tile = w2p.tile([P, 4, 512], BF16, name="w2_tile")
                    nc.sync.dma_start(w2_tile[:], w2a8[fg])
                    for i in range(4):
                        fk = fg * 4 + i
                        for j in range(4):
                            nc.tensor.matmul(
                                f_ps[j][:], w2_tile[:, i, j * P:(j + 1) * P],
                                h_sb[:, fk], start=(fk == 0), stop=(fk == KF - 1))
                # first output half of y2 + its LN2 partial stats; the stats
                # matmuls are emitted behind FFN2b's first group so the PE
                # never waits on the DVE adds
                st2_sum = psa2.tile([1, 512], F32, name="h_ps0", tag="h_ps0")
                st2_sq = psa2.tile([1, 512], F32, name="h_ps1", tag="h_ps1")
                for j in range(4):
                    nc.vector.scalar_tensor_tensor(
                        out=y2_sb[:, j], in0=f_ps[j][:], scalar=b2c[j],
                        in1=z1_sb[:, j], op0=OP.add, op1=OP.add)

                f_ps2 = [psa2.tile([P, 512], F32, name=f"f_ps{i}")
                         for i in range(4)]
                for fg in range(KD):
                    w2_tile = w2p.tile([P, 4, 512], BF16, name="w2_tile")
                    nc.sync.dma_start(w2_tile[:], w2b8[fg])
                    for i in range(4):
                        fk = fg * 4 + i
                        for j in range(4):
                            nc.tensor.matmul(
                                f_ps2[j][:], w2_tile[:, i, j * P:(j + 1) * P],
                                h_sb[:, fk], start=(fk == 0), stop=(fk == KF - 1))
                    if fg == 0:
                        ln_sums(st2_sum, st2_sq, y2_sb, range(0, 4))
                for j in range(4):
                    m = 4 + j
                    nc.vector.scalar_tensor_tensor(
                        out=y2_sb[:, m], in0=f_ps2[j][:], scalar=b2c[m],
                        in1=z1_sb[:, m], op0=OP.add, op1=OP.add)
                ln_sums(st2_sum, st2_sq, y2_sb, range(4, 8))
                rstd_b2, ms_b2 = ln_finish(st2_sum, st2_sq, "ln2")
                for kc in range(KD):
                    ln_apply(y2_sb, rstd_b2, ms_b2, g2, be2, z2_sb, kc)
                    nc.sync.dma_start(out8[:, kc], z2_sb[:, kc])
    nc.compile()
    return nc


def _get(name, builder):
    if name not in _CACHE:
        _CACHE[name] = builder()
    return _CACHE[name]


def _prep_inputs(X, Wq, Wk, Wo, ln1_g, ln1_b, ln2_g, ln2_b, W1, b1, W2, b2):
    f = lambda a: np.ascontiguousarray(a)
    Xt = np.asarray(X, np.float32).reshape(N, D).T          # [D, N]
    WqT = np.asarray(Wq, np.float32).T                      # [D, D]
    WkT = np.asarray(Wk, np.float32).T
    WoT = np.asarray(Wo, np.float32).T
    W1T = np.asarray(W1, np.float32).T                      # [D, FF]
    W2T = np.asarray(W2, np.float32).T                      # [FF, D]
    vecP = lambda v, k: np.asarray(v, np.float32).reshape(k, P).T  # [P, k]

    # xt8: [idx, p, kc, q]
    xt8 = f(Xt.reshape(KD, P, NQ, 512).transpose(2, 1, 0, 3))
    id64b = np.tile(np.eye(DH, dtype=np.float32), (2, 1)).astype(BF)
    w_tile = lambda w, c: f(
        w[:, c * P:(c + 1) * P].reshape(KD, P, P).transpose(1, 0, 2))

    in_maps_a = [
        {
            "xt8": xt8,
            "id64b": id64b,
            "wqt8": w_tile(WqT, c),
            "wkt8": w_tile(WkT, c),
            "wvt8": w_tile(WoT, c),   # value projection uses W_o in this model
        }
        for c in range(N_CORES)
    ]

    wo8 = f(WoT.reshape(KD, P, D).transpose(1, 0, 2).astype(BF))
    w18 = f(W1T.reshape(KD, P, KD, 512).transpose(2, 1, 0, 3).astype(BF))
    w2a8 = f(W2T[:, 0:512].reshape(KD, 4, P, 512).transpose(0, 2, 1, 3)
             .astype(BF))
    w2b8 = f(W2T[:, 512:1024].reshape(KD, 4, P, 512).transpose(0, 2, 1, 3)
             .astype(BF))
    consts = f(np.hstack([vecP(ln1_g, KD), vecP(ln1_b, KD), vecP(ln2_g, KD),
                          vecP(ln2_b, KD), vecP(b1, KF), vecP(b2, KD)]))
    id128b = np.eye(P, dtype=BF)

    def in_maps_b(full_nat):
        maps = []
        for c in range(N_CORES):
            blk = full_nat[c * QC:(c + 1) * QC]             # [512, 1024]
            ctin = f(blk.reshape(4, P, KD, P).transpose(1, 0, 2, 3).astype(BF))
            xts = f(Xt[:, c * QC:(c + 1) * QC].reshape(KD, P, 512)
                    .transpose(1, 0, 2))
            maps.append({
                "ctin": ctin, "xts8": xts, "wo8": wo8, "w18": w18,
                "w2a8": w2a8, "w2b8": w2b8, "consts": consts,
                "id128b": id128b,
            })
        return maps

    return in_maps_a, in_maps_b


def kernel(X, Wq, Wk, Wo, ln1_g, ln1_b, ln2_g, ln2_b, W1, b1, W2, b2):
    in_maps_a, in_maps_b = _prep_inputs(
        X, Wq, Wk, Wo, ln1_g, ln1_b, ln2_g, ln2_b, W1, b1, W2, b2)

    nc_a = _get("a", _build_phase_a)
    res_a = run_bass_kernel_spmd(nc_a, in_maps_a, core_ids=list(range(N_CORES)))
    # ctxn [P, NQ, 4, P] per core -> natural [4096, 128] -> concat cols
    full_nat = np.concatenate(
        [res_a.results[c]["ctxn"].transpose(1, 2, 0, 3).reshape(N, P)
         for c in range(N_CORES)], axis=1)                  # [N, D]

    nc_b = _get("b", _build_phase_b)
    res_b = run_bass_kernel_spmd(nc_b, in_maps_b(full_nat),
                                 core_ids=list(range(N_CORES)))
    # out8 [P, KD, 512] per core -> [D, 512] col block of out^T
    out_t = np.concatenate(
        [res_b.results[c]["out8"].transpose(1, 0, 2).reshape(D, QC)
         for c in range(N_CORES)], axis=1)                  # [D, N]
    return np.ascontiguousarray(out_t.T).reshape(B, S, D).astype(np.float32)


# revision 23
# speedup vs baseline: 1.0906x; 1.0333x over previous
"""Trainium2 Bass kernel for nn_Encoder (dense transformer block), 8 NeuronCores.

Strategy (single chip, 8 cores), v3:
  Phase A (head-parallel): core c computes attention for heads {2c, 2c+1}.
    Projections run in t-layout; q/k land in bf16, V is PE-transposed into
    natural [keys, dims] bf16 layout. softmax(relu(s)) is p = max(exp(s/8), 1)
    with the softmax denominator taken from a ones column appended to V.
    The exp pass on the Activation engine is the critical resource (~134 us);
    a queue-based emitter keeps it saturated: score matmuls are emitted as
    early as their projections allow (wavefront), projection matmuls are
    spread between them in small pieces, and the context-accumulation chains
    (65-cycle bf16 matmuls in the fast [q,65] orientation) fill the PE's
    exp-paced slack. ctx leaves phase A in natural [token, dim] layout.
  Phase B (row-parallel): core c takes 512 of the 4096 token rows. It
    PE-transposes the incoming ctx back to t-layout fused with the Wo
    matmuls, then AddNorm1, FFN (ReLU, bf16 weights/activations), AddNorm2.
    All weights stream as a handful of large host-pre-tiled bf16 DMAs on the
    SP queue; LayerNorm statistics are accumulated in halves so their matmuls
    and squares overlap the surrounding GEMMs.
"""

import os
import sys

for _p in ("/opt/trn_rl_repo",):
    if _p not in sys.path:
        sys.path.insert(0, _p)

# The Bass SPMD path executes through jax/PJRT on the axon platform; make
# sure a caller-pinned JAX_PLATFORMS=cpu doesn't hide the NeuronCores.
_jp = os.environ.get("JAX_PLATFORMS")
if _jp is not None and "axon" not in _jp:
    os.environ["JAX_PLATFORMS"] = "axon," + _jp

import ml_dtypes
import numpy as np

import concourse.bass as bass
import concourse.mybir as mybir
import concourse.tile as tile
from concourse import bacc
from concourse.bass_utils import run_bass_kernel_spmd

F32 = mybir.dt.float32
F32R = mybir.dt.float32r
BF16 = mybir.dt.bfloat16
AF = mybir.ActivationFunctionType
OP = mybir.AluOpType
BF = ml_dtypes.bfloat16


def _mm(nc, out, lhsT, rhs, **kw):
    # fp32r: 1-pass FP22 matmul (1 cyc/row when the moving dim is >= 256)
    nc.tensor.matmul(out, lhsT.bitcast(F32R), rhs.bitcast(F32R), **kw)


N_CORES = 8
B, S, D, H, DH, FF = 2, 2048, 1024, 16, 64, 4096
N = B * S            # 4096 token rows
P = 128
QC = N // N_CORES    # 512 rows per core in phase B
KD = D // P          # 8 contraction chunks over D
KI = S // P          # 16 key chunks of 128 per batch
NO = S // 512        # 4 query chunks of 512 per batch
NQ = N // 512        # 8 query chunks overall
KF = FF // P         # 32
EPS = 1e-5

_CACHE = {}


# --------------------------------------------------------------------------
# Phase A: per-core head-parallel attention.
# Inputs (per core):
#   xt8  [NQ, P, KD, 512]  X^T tiled per 512-token chunk (replicated)
#   wqt8/wkt8/wvt8 [P, KD, P]  W^T columns for this core's two heads, tiled
#   id64b [P, DH] bf16 tiled identity (V transposes)
# Output:
#   ctxn [P, NQ, 4, P] f32: natural-layout ctx; token = idx*512 + j*128 + p,
#   col = the two heads' 64-dim blocks concatenated.
# --------------------------------------------------------------------------
def _build_phase_a():
    nc = bacc.Bacc("TRN2", target_bir_lowering=False, debug=False,
                   num_devices=N_CORES)
    xt8 = nc.dram_tensor("xt8", [NQ, P, KD, 512], F32R, kind="ExternalInput")
    wqt8 = nc.dram_tensor("wqt8", [P, KD, P], F32R, kind="ExternalInput")
    wkt8 = nc.dram_tensor("wkt8", [P, KD, P], F32R, kind="ExternalInput")
    wvt8 = nc.dram_tensor("wvt8", [P, KD, P], F32R, kind="ExternalInput")
    id64b = nc.dram_tensor("id64b", [P, DH], BF16, kind="ExternalInput")
    ctxn = nc.dram_tensor("ctxn", [P, NQ, 4, P], F32, kind="ExternalOutput")

    chunks = [(b_, o) for b_ in range(B) for o in range(NO)]

    with tile.TileContext(nc) as tc:
        with tc.tile_pool(name="persist", bufs=1) as persist:
            qt_sb = [persist.tile([P, S], BF16, name=f"qt{b_}") for b_ in range(B)]
            kt_sb = [persist.tile([P, S], BF16, name=f"kt{b_}") for b_ in range(B)]
            vt_sb = [persist.tile([P, S], BF16, name=f"vt{b_}") for b_ in range(B)]
            vp_sb = [persist.tile([P, KI, 2, DH + 1], BF16, name=f"vp{b_}")
                     for b_ in range(B)]
            wq_sb = persist.tile([P, KD, P], F32R)
            wk_sb = persist.tile([P, KD, P], F32R)
            wv_sb = persist.tile([P, KD, P], F32R)
            id64_sb = persist.tile([P, DH], BF16)

            for b_ in range(B):
                nc.vector.memset(vp_sb[b_][:, :, 0, DH:DH + 1], 1.0)
                nc.vector.memset(vp_sb[b_][:, :, 1, DH:DH + 1], 1.0)

            with (
                tc.tile_pool(name="xpool", bufs=2) as xpool,
                tc.tile_pool(name="accp", bufs=2, space="PSUM") as accp,
                tc.tile_pool(name="slabp", bufs=50) as slabp,
                tc.tile_pool(name="stagep", bufs=2) as stagep,
                tc.tile_pool(name="smallp", bufs=8) as smallp,
                tc.tile_pool(name="pss", bufs=2, space="PSUM") as pss,
                tc.tile_pool(name="psc", bufs=2, space="PSUM") as psc,
            ):
                xt_tiles = {}

                def issue_xt(ci):
                    t = xpool.tile([P, KD, 512], F32R, name="xt_tile")
                    # two half-DMAs so the first projection matmuls can start
                    # as soon as the front half lands (subtile deps)
                    nc.sync.dma_start(t[:, 0:4], xt8[ci, :, 0:4])
                    nc.sync.dma_start(t[:, 4:8], xt8[ci, :, 4:8])
                    xt_tiles[ci] = t

                def gen_proj_qk(ci):
                    """Generator: project chunk ci into qt/kt (bf16)."""
                    b_, o = chunks[ci]
                    osl = slice(o * 512, (o + 1) * 512)
                    xt_tile = xt_tiles[ci]
                    for w_sb, dst in ((wq_sb, qt_sb[b_]), (wk_sb, kt_sb[b_])):
                        acc = accp.tile([P, 512], F32, name="acc_ps", tag="acc")
                        for kc in range(KD):
                            _mm(nc, acc[:], w_sb[:, kc], xt_tile[:, kc],
                                start=(kc == 0), stop=(kc == KD - 1))
                            if kc % 2 == 1:
                                yield
                        nc.vector.tensor_copy(dst[:, osl], acc[:])
                        yield

                def gen_proj_v(ci):
                    """Generator: V projection + natural-layout transposes."""
                    b_, o = chunks[ci]
                    osl = slice(o * 512, (o + 1) * 512)
                    xt_tile = xt_tiles.pop(ci)
                    acc = accp.tile([P, 512], F32, name="acc_ps", tag="acc")
                    for kc in range(KD):
                        _mm(nc, acc[:], wv_sb[:, kc], xt_tile[:, kc],
                            start=(kc == 0), stop=(kc == KD - 1))
                        if kc % 2 == 1:
                            yield
                    nc.vector.tensor_copy(vt_sb[b_][:, osl], acc[:])
                    yield
                    for t in range(4):
                        kc2 = o * 4 + t
                        for hh in range(2):
                            tp = accp.tile([P, DH], BF16, name="tp_ps",
                                           tag="acc")
                            nc.tensor.transpose(
                                tp[:, 0:DH],
                                vt_sb[b_][hh * DH:(hh + 1) * DH,
                                          kc2 * P:(kc2 + 1) * P],
                                id64_sb[hh * DH:(hh + 1) * DH, :])
                            nc.vector.tensor_copy(
                                vp_sb[b_][:, kc2, hh, 0:DH], tp[:, 0:DH])
                        yield

                slabs = {i: {} for i in range(NQ)}   # idx -> kc -> slab tile
                stages = {}

                def emit_scores(idx, kc):
                    b_, o = chunks[idx]
                    qs = slice(o * 512, (o + 1) * 512)
                    ks = slice(kc * P, (kc + 1) * P)
                    s_ps = pss.tile([P, 1024], F32, name="s_ps")
                    nc.tensor.matmul(s_ps[:, 0:512], kt_sb[b_][0:DH, ks],
                                     qt_sb[b_][0:DH, qs], start=True, stop=True)
                    nc.tensor.matmul(s_ps[:, 512:1024], kt_sb[b_][DH:2 * DH, ks],
                                     qt_sb[b_][DH:2 * DH, qs],
                                     start=True, stop=True)
                    slab = slabp.tile([P, 1024], BF16, name="slab")
                    nc.scalar.activation(slab[:], s_ps[:], AF.Exp, scale=0.125)
                    nc.vector.tensor_scalar_max(slab[:], slab[:], 1.0)
                    slabs[idx][kc] = slab

                def gen_chains(idx):
                    """Generator: the 8 ctx chains of idx + normalize + DMA,
                    yielding every couple of matmuls."""
                    b_, o = chunks[idx]
                    stage = stagep.tile([P, 4, P], F32, name="stage")
                    for ci in range(8):
                        j, h = ci // 2, ci % 2
                        acc = psc.tile([P, DH + 1], F32, name="ctx_ps")
                        for kc in range(KI):
                            nc.tensor.matmul(
                                acc[:],
                                slabs[idx][kc][:, h * 512 + j * P:
                                               h * 512 + (j + 1) * P],
                                vp_sb[b_][:, kc, h, :],
                                start=(kc == 0), stop=(kc == KI - 1))
                            if kc % 4 == 3:
                                yield
                        inv = smallp.tile([P, 1], F32, name="inv")
                        nc.vector.reciprocal(inv[:], acc[:, DH:DH + 1])
                        nc.vector.tensor_scalar(
                            out=stage[:, j, h * DH:(h + 1) * DH],
                            in0=acc[:, 0:DH], scalar1=inv[:], scalar2=None,
                            op0=OP.mult)
                        yield
                    nc.sync.dma_start(ctxn[:, idx], stage[:])
                    slabs[idx].clear()

                # ---------------- queue-based emitter ----------------
                emitted = set()           # (idx, kc) scores emitted
                score_q = []              # ordered pending scores
                qk_done = [False] * NQ
                v_done = [False] * NQ
                chains_done = 0           # count of fully-emitted chain idxs
                chain_gen = None
                chain_idx = 0             # next idx needing chains
                qk_idx = 0                # next chunk for q/k projection
                v_idx = 0                 # next chunk for v projection
                qkgen = None
                vgen = None

                def update_score_q():
                    for i in range(NQ):
                        bi, _ = chunks[i]
                        if not qk_done[i]:
                            continue
                        if i >= chains_done + 3:
                            continue
                        base = 4 * bi
                        kmax = sum(4 for c in range(base, base + NO)
                                   if qk_done[c])
                        for k in range(kmax):
                            if (i, k) not in emitted and (i, k) not in score_q:
                                score_q.append((i, k))

                # first xt chunk ahead of the (large) weight loads so the
                # first projection matmuls start as early as possible
                nc.sync.dma_start(xt_w := xpool.tile([P, KD, 512], F32R,
                                                     name="xt_tile"), None) \
                    if False else None
                issue_xt(0)
                nc.sync.dma_start(wq_sb[:], wqt8.ap())
                nc.sync.dma_start(wk_sb[:], wkt8.ap())
                issue_xt(1)
                nc.sync.dma_start(wv_sb[:], wvt8.ap())
                nc.sync.dma_start(id64_sb[:], id64b.ap())
                while (qk_idx < NQ or v_idx < NQ or score_q
                       or chain_idx < NQ or chain_gen is not None):
                    # 1. a slice of chain work (PE filler, no Act dependency)
                    if chain_gen is None and chain_idx < NQ:
                        bci, _ = chunks[chain_idx]
                        if (len(slabs[chain_idx]) == KI
                                and all(v_done[c] for c in
                                        range(4 * bci, 4 * bci + NO))):
                            chain_gen = gen_chains(chain_idx)
                    if chain_gen is not None:
                        for _ in range(3):
                            try:
                                next(chain_gen)
                            except StopIteration:
                                chain_gen = None
                                chain_idx += 1
                                chains_done += 1
                                update_score_q()
                                break
                    # 2. q/k projection pieces (gate scores)
                    if qkgen is None and qk_idx < NQ and qk_idx <= v_idx:
                        qkgen = gen_proj_qk(qk_idx)
                    if qkgen is not None:
                        steps = 1 if score_q else 4
                        for _ in range(steps):
                            try:
                                next(qkgen)
                            except StopIteration:
                                qk_done[qk_idx] = True
                                qk_idx += 1
                                if qk_idx + 1 < NQ:
                                    issue_xt(qk_idx + 1)
                                qkgen = None
                                update_score_q()
                                break
                    # 3. v projection + transposes (gate chains only)
                    if vgen is None and v_idx < NQ and v_idx < qk_idx:
                        vgen = gen_proj_v(v_idx)
                    if vgen is not None:
                        steps = 2 if (score_q and chain_gen is not None) else 4
                        for _ in range(steps):
                            try:
                                next(vgen)
                            except StopIteration:
                                v_done[v_idx] = True
                                v_idx += 1
                                vgen = None
                                break
                    # 4. one score (the Act engine's food)
                    if score_q:
                        i, k = score_q.pop(0)
                        emit_scores(i, k)
                        emitted.add((i, k))
                        update_score_q()
    nc.compile()
    return nc


# --------------------------------------------------------------------------
# Phase B: per-core row-parallel transpose + Wo-proj + AddNorm1 + FFN + AddNorm2.
# Inputs (per core, qi = this core's 512 token rows):
#   ctin [P, 4, KD, P] bf16   natural-layout ctx blocks for these rows
#   wo8  [P, KD, D]    bf16   Wo^T tiled
#   w18  [KD, P, KD, 512] bf16  W1^T tiled per 512-wide ffn-col group
#   w2a8/w2b8 [KD, P, 4, 512] bf16  W2^T tiled, first/second output half
#   xts8 [P, KD, 512] f32     X^T slice (residual 1)
#   consts [P, 72] f32        g1|be1|g2|be2|b1t|b2t feature-on-partition
#   id128b [P, P] bf16
# Output: out8 [P, KD, 512] f32 (t-layout output slice, tiled)
# --------------------------------------------------------------------------
def _build_phase_b():
    nc = bacc.Bacc("TRN2", target_bir_lowering=False, debug=False,
                   num_devices=N_CORES)
    ctin = nc.dram_tensor("ctin", [P, 4, KD, P], BF16, kind="ExternalInput")
    wo8 = nc.dram_tensor("wo8", [P, KD, D], BF16, kind="ExternalInput")
    w18 = nc.dram_tensor("w18", [KD, P, KD, 512], BF16, kind="ExternalInput")
    w2a8 = nc.dram_tensor("w2a8", [KD, P, 4, 512], BF16, kind="ExternalInput")
    w2b8 = nc.dram_tensor("w2b8", [KD, P, 4, 512], BF16, kind="ExternalInput")
    xts8 = nc.dram_tensor("xts8", [P, KD, 512], F32, kind="ExternalInput")
    consts = nc.dram_tensor("consts", [P, 72], F32, kind="ExternalInput")
    id128b = nc.dram_tensor("id128b", [P, P], BF16, kind="ExternalInput")
    out8 = nc.dram_tensor("out8", [P, KD, 512], BF16, kind="ExternalOutput")

    with tile.TileContext(nc) as tc:
        with (
            tc.tile_pool(name="persist", bufs=1) as persist,
            tc.tile_pool(name="w1p", bufs=3) as w1p,
            tc.tile_pool(name="w2p", bufs=3) as w2p,
            tc.tile_pool(name="sqp", bufs=3) as sqp,
            tc.tile_pool(name="smallp", bufs=2) as smallp,
            tc.tile_pool(name="bcp", bufs=2) as bcp,
        ):
            ctin_sb = persist.tile([P, 4, KD, P], BF16)
            ct_sb = persist.tile([P, KD, 4, P], BF16)
            wo_sb = persist.tile([P, KD, D], BF16)
            xts_sb = persist.tile([P, KD, 512], F32)
            y1_sb = persist.tile([P, KD, 512], BF16)
            z1_sb = persist.tile([P, KD, 512], BF16)
            h_sb = persist.tile([P, KF, 512], BF16)
            y2_sb = persist.tile([P, KD, 512], BF16, tag="y1_sb")
            z2_sb = persist.tile([P, KD, 512], BF16, tag="xts_sb")
            consts_sb = persist.tile([P, 72], F32)
            id128_sb = persist.tile([P, P], BF16)
            ones = persist.tile([P, 1], BF16)

            nc.sync.dma_start(ctin_sb[:], ctin.ap())
            nc.sync.dma_start(id128_sb[:], id128b.ap())
            nc.sync.dma_start(consts_sb[:], consts.ap())
            nc.sync.dma_start(wo_sb[:, 0:4], wo8[:, 0:4])
            nc.sync.dma_start(wo_sb[:, 4:8], wo8[:, 4:8])
            nc.sync.dma_start(xts_sb[:], xts8.ap())
            nc.vector.memset(ones[:], 1.0)

            g1 = [consts_sb[:, kc:kc + 1] for kc in range(KD)]
            be1 = [consts_sb[:, 8 + kc:9 + kc] for kc in range(KD)]
            g2 = [consts_sb[:, 16 + kc:17 + kc] for kc in range(KD)]
            be2 = [consts_sb[:, 24 + kc:25 + kc] for kc in range(KD)]
            b1c = [consts_sb[:, 32 + fm:33 + fm] for fm in range(KF)]
            b2c = [consts_sb[:, 64 + kc:65 + kc] for kc in range(KD)]

            def ln_sums(st_sum, st_sq, y_sb, kcs):
                """Partial LN stats for feature chunks kcs of y_sb (bf16)."""
                for kc in kcs:
                    nc.tensor.matmul(st_sum[:], ones[:], y_sb[:, kc],
                                     start=(kc == 0), stop=(kc == KD - 1))
                for kc in kcs:
                    sq = sqp.tile([P, 512], BF16, name="sq")
                    nc.vector.tensor_mul(sq[:], y_sb[:, kc], y_sb[:, kc])
                    nc.tensor.matmul(st_sq[:], ones[:], sq[:],
                                     start=(kc == 0), stop=(kc == KD - 1))

            def ln_finish(st_sum, st_sq, tag):
                """Stats -> (rstd_b, ms_b) broadcast tiles."""
                mean = smallp.tile([1, 512], F32, name="mean")
                ex2 = smallp.tile([1, 512], F32, name="ex2")
                nc.vector.tensor_scalar(out=mean[:], in0=st_sum[:],
                                        scalar1=1.0 / D, scalar2=None,
                                        op0=OP.mult)
                nc.vector.tensor_scalar(out=ex2[:], in0=st_sq[:],
                                        scalar1=1.0 / D, scalar2=None,
                                        op0=OP.mult)
                msq = smallp.tile([1, 512], F32, name="msq")
                nc.vector.tensor_mul(msq[:], mean[:], mean[:])
                var = smallp.tile([1, 512], F32, name="var")
                nc.vector.tensor_sub(var[:], ex2[:], msq[:])
                nc.vector.tensor_scalar_add(var[:], var[:], EPS)
                std = smallp.tile([1, 512], F32, name="std")
                nc.scalar.activation(std[:], var[:], AF.Sqrt)
                rstd = smallp.tile([1, 512], BF16, name="rstd")
                ms = smallp.tile([1, 512], BF16, name="ms")
                with nc.allow_low_precision(reason="bf16 LN scale factors"):
                    nc.vector.reciprocal(rstd[:], std[:])
                nc.vector.tensor_mul(ms[:], mean[:], rstd[:])
                rstd_b = bcp.tile([P, 512], BF16, name="rstd_b")
                ms_b = bcp.tile([P, 512], BF16, name="ms_b")
                nc.gpsimd.partition_broadcast(rstd_b[:], rstd[:])
                nc.gpsimd.partition_broadcast(ms_b[:], ms[:])
                return rstd_b, ms_b

            def ln_apply(y_sb, rstd_b, ms_b, g_c, be_c, z_sb, kc):
                t = sqp.tile([P, 512], BF16, name="t_ln")
                nc.vector.tensor_mul(t[:], y_sb[:, kc], rstd_b[:])
                nc.vector.tensor_sub(t[:], t[:], ms_b[:])
                nc.vector.tensor_scalar(out=z_sb[:, kc], in0=t[:],
                                        scalar1=g_c[kc], scalar2=be_c[kc],
                                        op0=OP.mult, op1=OP.add)

            # ---- B0+B1: transpose ctx to t-layout, fused with Wo matmuls ----
            with (
                tc.tile_pool(name="tpp", bufs=2, space="PSUM") as tpp,
                tc.tile_pool(name="psa", bufs=1, space="PSUM") as psa,
                tc.tile_pool(name="psst1", bufs=1, space="PSUM") as psst1,
            ):
                st1_sum = psst1.tile([1, 512], F32, name="st1_sum")
                st1_sq = psst1.tile([1, 512], F32, name="st1_sq")
                a_ps = [psa.tile([P, 512], F32, name=f"mm_ps{i}")
                        for i in range(4)]
                for kc in range(KD):
                    tp = tpp.tile([P, 4, P], BF16, name="tp_ps")
                    for jb in range(4):
                        nc.tensor.transpose(tp[:, jb, :],
                                            ctin_sb[:, jb, kc, :],
                                            id128_sb[:])
                    nc.vector.tensor_copy(ct_sb[:, kc], tp[:])
                    for i in range(4):
                        nc.tensor.matmul(
                            a_ps[i][:], wo_sb[:, kc, i * P:(i + 1) * P],
                            ct_sb[:, kc], start=(kc == 0), stop=(kc == KD - 1))
                for i in range(4):
                    nc.vector.tensor_add(y1_sb[:, i], a_ps[i][:], xts_sb[:, i])
                a_ps2 = [psa.tile([P, 512], F32, name=f"mm_ps{i}")
                         for i in range(4)]
                for kc in range(KD):
                    for i in range(4):
                        nc.tensor.matmul(
                            a_ps2[i][:],
                            wo_sb[:, kc, 512 + i * P:512 + (i + 1) * P],
                            ct_sb[:, kc], start=(kc == 0), stop=(kc == KD - 1))
                # first-half LN1 stats run on the PE behind the mg1 matmuls,
                # overlapping the DVE residual adds
                ln_sums(st1_sum, st1_sq, y1_sb, range(0, 4))
                for i in range(4):
                    m = 4 + i
                    nc.vector.tensor_add(y1_sb[:, m], a_ps2[i][:], xts_sb[:, m])
                ln_sums(st1_sum, st1_sq, y1_sb, range(4, 8))
                rstd_b1, ms_b1 = ln_finish(st1_sum, st1_sq, "ln1")
                for kc in range(KD):
                    ln_apply(y1_sb, rstd_b1, ms_b1, g1, be1, z1_sb, kc)

            # ---- FFN1 + FFN2 (first output half interleaved) ----
            with tc.tile_pool(name="psa2", bufs=1, space="PSUM") as psa2:
                f_ps = [psa2.tile([P, 512], F32, name=f"f_ps{i}")
                        for i in range(4)]
                for fg in range(KD):
                    w1_tile = w1p.tile([P, KD, 512], BF16, name="w1_tile")
                    nc.sync.dma_start(w1_tile[:], w18[fg])
                    h_ps = [psa2.tile([P, 512], F32, name=f"h_ps{i}")
                            for i in range(4)]
                    for kc in range(KD):
                        for i in range(4):
                            nc.tensor.matmul(
                                h_ps[i][:], w1_tile[:, kc, i * P:(i + 1) * P],
                                z1_sb[:, kc], start=(kc == 0), stop=(kc == KD - 1))
                    for i in range(4):
                        fm = fg * 4 + i
                        nc.scalar.activation(h_sb[:, fm], h_ps[i][:], AF.Relu,
                                             bias=b1c[fm])
                    w2_tile = w2p.tile([P, 4, 512], BF16, name="w2_tile")
                    nc.sync.dma_start(w2_tile[:], w2a8[fg])
                    for i in range(4):
                        fk = fg * 4 + i
                        for j in range(4):
                            nc.tensor.matmul(
                                f_ps[j][:], w2_tile[:, i, j * P:(j + 1) * P],
                                h_sb[:, fk], start=(fk == 0), stop=(fk == KF - 1))
                # first output half of y2 + its LN2 partial stats; the stats
                # matmuls are emitted behind FFN2b's first group so the PE
                # never waits on the DVE adds
                st2_sum = psa2.tile([1, 512], F32, name="h_ps0", tag="h_ps0")
                st2_sq = psa2.tile([1, 512], F32, name="h_ps1", tag="h_ps1")
                for j in range(4):
                    nc.vector.scalar_tensor_tensor(
                        out=y2_sb[:, j], in0=f_ps[j][:], scalar=b2c[j],
                        in1=z1_sb[:, j], op0=OP.add, op1=OP.add)

                f_ps2 = [psa2.tile([P, 512], F32, name=f"f_ps{i}")
                         for i in range(4)]
                for fg in range(KD):
                    w2_tile = w2p.tile([P, 4, 512], BF16, name="w2_tile")
                    nc.sync.dma_start(w2_tile[:], w2b8[fg])
                    for i in range(4):
                        fk = fg * 4 + i
                        for j in range(4):
                            nc.tensor.matmul(
                                f_ps2[j][:], w2_tile[:, i, j * P:(j + 1) * P],
                                h_sb[:, fk], start=(fk == 0), stop=(fk == KF - 1))
                    if fg == 0:
                        ln_sums(st2_sum, st2_sq, y2_sb, range(0, 4))
                for j in range(4):
                    m = 4 + j
                    nc.vector.scalar_tensor_tensor(
                        out=y2_sb[:, m], in0=f_ps2[j][:], scalar=b2c[m],
                        in1=z1_sb[:, m], op0=OP.add, op1=OP.add)
                ln_sums(st2_sum, st2_sq, y2_sb, range(4, 8))
                rstd_b2, ms_b2 = ln_finish(st2_sum, st2_sq, "ln2")
                for kc in range(KD):
                    ln_apply(y2_sb, rstd_b2, ms_b2, g2, be2, z2_sb, kc)
                    nc.sync.dma_start(out8[:, kc], z2_sb[:, kc])
    nc.compile()
    return nc


def _get(name, builder):
    if name not in _CACHE:
        _CACHE[name] = builder()
    return _CACHE[name]


def _prep_inputs(X, Wq, Wk, Wo, ln1_g, ln1_b, ln2_g, ln2_b, W1, b1, W2, b2):
    f = lambda a: np.ascontiguousarray(a)
    Xt = np.asarray(X, np.float32).reshape(N, D).T          # [D, N]
    WqT = np.asarray(Wq, np.float32).T                      # [D, D]
    WkT = np.asarray(Wk, np.float32).T
    WoT = np.asarray(Wo, np.float32).T
    W1T = np.asarray(W1, np.float32).T                      # [D, FF]
    W2T = np.asarray(W2, np.float32).T                      # [FF, D]
    vecP = lambda v, k: np.asarray(v, np.float32).reshape(k, P).T  # [P, k]

    # xt8: [idx, p, kc, q]
    xt8 = f(Xt.reshape(KD, P, NQ, 512).transpose(2, 1, 0, 3))
    id64b = np.tile(np.eye(DH, dtype=np.float32), (2, 1)).astype(BF)
    w_tile = lambda w, c: f(
        w[:, c * P:(c + 1) * P].reshape(KD, P, P).transpose(1, 0, 2))

    in_maps_a = [
        {
            "xt8": xt8,
            "id64b": id64b,
            "wqt8": w_tile(WqT, c),
            "wkt8": w_tile(WkT, c),
            "wvt8": w_tile(WoT, c),   # value projection uses W_o in this model
        }
        for c in range(N_CORES)
    ]

    wo8 = f(WoT.reshape(KD, P, D).transpose(1, 0, 2).astype(BF))
    w18 = f(W1T.reshape(KD, P, KD, 512).transpose(2, 1, 0, 3).astype(BF))
    w2a8 = f(W2T[:, 0:512].reshape(KD, 4, P, 512).transpose(0, 2, 1, 3)
             .astype(BF))
    w2b8 = f(W2T[:, 512:1024].reshape(KD, 4, P, 512).transpose(0, 2, 1, 3)
             .astype(BF))
    consts = f(np.hstack([vecP(ln1_g, KD), vecP(ln1_b, KD), vecP(ln2_g, KD),
                          vecP(ln2_b, KD), vecP(b1, KF), vecP(b2, KD)]))
    id128b = np.eye(P, dtype=BF)

    def in_maps_b(full_nat):
        maps = []
        for c in range(N_CORES):
            blk = full_nat[c * QC:(c + 1) * QC]             # [512, 1024]
            ctin = f(blk.reshape(4, P, KD, P).transpose(1, 0, 2, 3).astype(BF))
            xts = f(Xt[:, c * QC:(c + 1) * QC].reshape(KD, P, 512)
                    .transpose(1, 0, 2))
            maps.append({
                "ctin": ctin, "xts8": xts, "wo8": wo8, "w18": w18,
                "w2a8": w2a8, "w2b8": w2b8, "consts": consts,
                "id128b": id128b,
            })
        return maps

    return in_maps_a, in_maps_b


def kernel(X, Wq, Wk, Wo, ln1_g, ln1_b, ln2_g, ln2_b, W1, b1, W2, b2):
    in_maps_a, in_maps_b = _prep_inputs(
        X, Wq, Wk, Wo, ln1_g, ln1_b, ln2_g, ln2_b, W1, b1, W2, b2)

    nc_a = _get("a", _build_phase_a)
    res_a = run_bass_kernel_spmd(nc_a, in_maps_a, core_ids=list(range(N_CORES)))
    # ctxn [P, NQ, 4, P] per core -> natural [4096, 128] -> concat cols
    full_nat = np.concatenate(
        [res_a.results[c]["ctxn"].transpose(1, 2, 0, 3).reshape(N, P)
         for c in range(N_CORES)], axis=1)                  # [N, D]

    nc_b = _get("b", _build_phase_b)
    res_b = run_bass_kernel_spmd(nc_b, in_maps_b(full_nat),
                                 core_ids=list(range(N_CORES)))
    # out8 [P, KD, 512] per core -> [D, 512] col block of out^T
    out_t = np.concatenate(
        [res_b.results[c]["out8"].astype(np.float32).transpose(1, 0, 2)
         .reshape(D, QC) for c in range(N_CORES)], axis=1)  # [D, N]
    return np.ascontiguousarray(out_t.T).reshape(B, S, D).astype(np.float32)


# revision 30
# speedup vs baseline: 1.0985x; 1.0072x over previous
"""Trainium2 Bass kernel for nn_Encoder (dense transformer block), 8 NeuronCores.

Strategy (single chip, 8 cores), v3:
  Phase A (head-parallel): core c computes attention for heads {2c, 2c+1}.
    Projections run in t-layout; q/k land in bf16, V is PE-transposed into
    natural [keys, dims] bf16 layout. softmax(relu(s)) is p = max(exp(s/8), 1)
    with the softmax denominator taken from a ones column appended to V.
    The exp pass on the Activation engine is the critical resource (~134 us);
    a queue-based emitter keeps it saturated: score matmuls are emitted as
    early as their projections allow (wavefront), projection matmuls are
    spread between them in small pieces, and the context-accumulation chains
    (65-cycle bf16 matmuls in the fast [q,65] orientation) fill the PE's
    exp-paced slack. ctx leaves phase A in natural [token, dim] layout.
  Phase B (row-parallel): core c takes 512 of the 4096 token rows. It
    PE-transposes the incoming ctx back to t-layout fused with the Wo
    matmuls, then AddNorm1, FFN (ReLU, bf16 weights/activations), AddNorm2.
    All weights stream as a handful of large host-pre-tiled bf16 DMAs on the
    SP queue; LayerNorm statistics are accumulated in halves so their matmuls
    and squares overlap the surrounding GEMMs.
"""

import os
import sys

for _p in ("/opt/trn_rl_repo",):
    if _p not in sys.path:
        sys.path.insert(0, _p)

# The Bass SPMD path executes through jax/PJRT on the axon platform; make
# sure a caller-pinned JAX_PLATFORMS=cpu doesn't hide the NeuronCores.
_jp = os.environ.get("JAX_PLATFORMS")
if _jp is not None and "axon" not in _jp:
    os.environ["JAX_PLATFORMS"] = "axon," + _jp

import ml_dtypes
import numpy as np

import concourse.bass as bass
import concourse.mybir as mybir
import concourse.tile as tile
from concourse import bacc
from concourse.bass_utils import run_bass_kernel_spmd

F32 = mybir.dt.float32
F32R = mybir.dt.float32r
BF16 = mybir.dt.bfloat16
AF = mybir.ActivationFunctionType
OP = mybir.AluOpType
BF = ml_dtypes.bfloat16


def _mm(nc, out, lhsT, rhs, **kw):
    # fp32r: 1-pass FP22 matmul (1 cyc/row when the moving dim is >= 256)
    nc.tensor.matmul(out, lhsT.bitcast(F32R), rhs.bitcast(F32R), **kw)


N_CORES = 8
B, S, D, H, DH, FF = 2, 2048, 1024, 16, 64, 4096
N = B * S            # 4096 token rows
P = 128
QC = N // N_CORES    # 512 rows per core in phase B
KD = D // P          # 8 contraction chunks over D
KI = S // P          # 16 key chunks of 128 per batch
NO = S // 512        # 4 query chunks of 512 per batch
NQ = N // 512        # 8 query chunks overall
KF = FF // P         # 32
EPS = 1e-5

_CACHE = {}


# --------------------------------------------------------------------------
# Phase A: per-core head-parallel attention.
# Inputs (per core):
#   xt8  [NQ, P, KD, 512]  X^T tiled per 512-token chunk (replicated)
#   wqt8/wkt8/wvt8 [P, KD, P]  W^T columns for this core's two heads, tiled
#   id64b [P, DH] bf16 tiled identity (V transposes)
# Output:
#   ctxn [P, NQ, 4, P] f32: natural-layout ctx; token = idx*512 + j*128 + p,
#   col = the two heads' 64-dim blocks concatenated.
# --------------------------------------------------------------------------
def _build_phase_a():
    nc = bacc.Bacc("TRN2", target_bir_lowering=False, debug=False,
                   num_devices=N_CORES)
    xt8 = nc.dram_tensor("xt8", [NQ, P, KD, 512], F32R, kind="ExternalInput")
    wqt8 = nc.dram_tensor("wqt8", [P, KD, P], F32R, kind="ExternalInput")
    wkt8 = nc.dram_tensor("wkt8", [P, KD, P], F32R, kind="ExternalInput")
    wvt8 = nc.dram_tensor("wvt8", [P, KD, P], F32R, kind="ExternalInput")
    id64b = nc.dram_tensor("id64b", [P, DH], BF16, kind="ExternalInput")
    ctxn = nc.dram_tensor("ctxn", [P, NQ, 4, P], F32, kind="ExternalOutput")

    chunks = [(b_, o) for b_ in range(B) for o in range(NO)]

    with tile.TileContext(nc) as tc:
        with tc.tile_pool(name="persist", bufs=1) as persist:
            qt_sb = [persist.tile([P, S], BF16, name=f"qt{b_}") for b_ in range(B)]
            kt_sb = [persist.tile([P, S], BF16, name=f"kt{b_}") for b_ in range(B)]
            vt_sb = [persist.tile([P, S], BF16, name=f"vt{b_}") for b_ in range(B)]
            vp_sb = [persist.tile([P, KI, 2, DH + 1], BF16, name=f"vp{b_}")
                     for b_ in range(B)]
            wq_sb = persist.tile([P, KD, P], F32R)
            wk_sb = persist.tile([P, KD, P], F32R)
            wv_sb = persist.tile([P, KD, P], F32R)
            id64_sb = persist.tile([P, DH], BF16)

            for b_ in range(B):
                nc.vector.memset(vp_sb[b_][:, :, 0, DH:DH + 1], 1.0)
                nc.vector.memset(vp_sb[b_][:, :, 1, DH:DH + 1], 1.0)

            with (
                tc.tile_pool(name="xpool", bufs=2) as xpool,
                tc.tile_pool(name="accp", bufs=2, space="PSUM") as accp,
                tc.tile_pool(name="slabp", bufs=50) as slabp,
                tc.tile_pool(name="stagep", bufs=2) as stagep,
                tc.tile_pool(name="smallp", bufs=8) as smallp,
                tc.tile_pool(name="pss", bufs=2, space="PSUM") as pss,
                tc.tile_pool(name="psc", bufs=2, space="PSUM") as psc,
            ):
                xt_tiles = {}

                def issue_xt(ci):
                    t = xpool.tile([P, KD, 512], F32R, name="xt_tile")
                    # two half-DMAs so the first projection matmuls can start
                    # as soon as the front half lands (subtile deps)
                    nc.sync.dma_start(t[:, 0:4], xt8[ci, :, 0:4])
                    nc.sync.dma_start(t[:, 4:8], xt8[ci, :, 4:8])
                    xt_tiles[ci] = t

                def gen_proj_qk(ci):
                    """Generator: project chunk ci into qt/kt (bf16)."""
                    b_, o = chunks[ci]
                    osl = slice(o * 512, (o + 1) * 512)
                    xt_tile = xt_tiles[ci]
                    for w_sb, dst in ((wq_sb, qt_sb[b_]), (wk_sb, kt_sb[b_])):
                        acc = accp.tile([P, 512], F32, name="acc_ps", tag="acc")
                        for kc in range(KD):
                            _mm(nc, acc[:], w_sb[:, kc], xt_tile[:, kc],
                                start=(kc == 0), stop=(kc == KD - 1))
                            if kc % 2 == 1:
                                yield
                        nc.vector.tensor_copy(dst[:, osl], acc[:])
                        yield

                def gen_proj_v(ci):
                    """Generator: V projection + natural-layout transposes."""
                    b_, o = chunks[ci]
                    osl = slice(o * 512, (o + 1) * 512)
                    xt_tile = xt_tiles.pop(ci)
                    acc = accp.tile([P, 512], F32, name="acc_ps", tag="acc")
                    for kc in range(KD):
                        _mm(nc, acc[:], wv_sb[:, kc], xt_tile[:, kc],
                            start=(kc == 0), stop=(kc == KD - 1))
                        if kc % 2 == 1:
                            yield
                    nc.vector.tensor_copy(vt_sb[b_][:, osl], acc[:])
                    yield
                    for t in range(4):
                        kc2 = o * 4 + t
                        for hh in range(2):
                            tp = accp.tile([P, DH], BF16, name="tp_ps",
                                           tag="acc")
                            nc.tensor.transpose(
                                tp[:, 0:DH],
                                vt_sb[b_][hh * DH:(hh + 1) * DH,
                                          kc2 * P:(kc2 + 1) * P],
                                id64_sb[hh * DH:(hh + 1) * DH, :])
                            nc.vector.tensor_copy(
                                vp_sb[b_][:, kc2, hh, 0:DH], tp[:, 0:DH])
                        yield

                slabs = {i: {} for i in range(NQ)}   # idx -> kc -> slab tile
                stages = {}

                def emit_scores(idx, kc):
                    b_, o = chunks[idx]
                    qs = slice(o * 512, (o + 1) * 512)
                    ks = slice(kc * P, (kc + 1) * P)
                    s_ps = pss.tile([P, 1024], F32, name="s_ps")
                    nc.tensor.matmul(s_ps[:, 0:512], kt_sb[b_][0:DH, ks],
                                     qt_sb[b_][0:DH, qs], start=True, stop=True)
                    nc.tensor.matmul(s_ps[:, 512:1024], kt_sb[b_][DH:2 * DH, ks],
                                     qt_sb[b_][DH:2 * DH, qs],
                                     start=True, stop=True)
                    slab = slabp.tile([P, 1024], BF16, name="slab")
                    nc.scalar.activation(slab[:], s_ps[:], AF.Exp, scale=0.125)
                    nc.vector.tensor_scalar_max(slab[:], slab[:], 1.0)
                    slabs[idx][kc] = slab

                def gen_chains(idx):
                    """Generator: the 8 ctx chains of idx + normalize + DMA,
                    yielding every couple of matmuls."""
                    b_, o = chunks[idx]
                    stage = stagep.tile([P, 4, P], F32, name="stage")
                    for ci in range(8):
                        j, h = ci // 2, ci % 2
                        acc = psc.tile([P, DH + 1], F32, name="ctx_ps")
                        for kc in range(KI):
                            nc.tensor.matmul(
                                acc[:],
                                slabs[idx][kc][:, h * 512 + j * P:
                                               h * 512 + (j + 1) * P],
                                vp_sb[b_][:, kc, h, :],
                                start=(kc == 0), stop=(kc == KI - 1))
                            if kc % 4 == 3:
                                yield
                        inv = smallp.tile([P, 1], F32, name="inv")
                        nc.vector.reciprocal(inv[:], acc[:, DH:DH + 1])
                        nc.vector.tensor_scalar(
                            out=stage[:, j, h * DH:(h + 1) * DH],
                            in0=acc[:, 0:DH], scalar1=inv[:], scalar2=None,
                            op0=OP.mult)
                        yield
                    nc.sync.dma_start(ctxn[:, idx], stage[:])
                    slabs[idx].clear()

                # ---------------- queue-based emitter ----------------
                emitted = set()           # (idx, kc) scores emitted
                score_q = []              # ordered pending scores
                qk_done = [False] * NQ
                v_done = [False] * NQ
                chains_done = 0           # count of fully-emitted chain idxs
                chain_gen = None
                chain_idx = 0             # next idx needing chains
                qk_idx = 0                # next chunk for q/k projection
                v_idx = 0                 # next chunk for v projection
                qkgen = None
                vgen = None

                def update_score_q():
                    for i in range(NQ):
                        bi, _ = chunks[i]
                        if not qk_done[i]:
                            continue
                        if i >= chains_done + 3:
                            continue
                        base = 4 * bi
                        kmax = sum(4 for c in range(base, base + NO)
                                   if qk_done[c])
                        for k in range(kmax):
                            if (i, k) not in emitted and (i, k) not in score_q:
                                score_q.append((i, k))

                # DMA order tuned so the first q-projection matmuls can
                # start at ~4.5us: wq, then the first xt half, then wk etc.
                nc.sync.dma_start(wq_sb[:], wqt8.ap())
                issue_xt(0)
                nc.sync.dma_start(wk_sb[:], wkt8.ap())
                nc.sync.dma_start(wv_sb[:], wvt8.ap())
                issue_xt(1)
                nc.sync.dma_start(id64_sb[:], id64b.ap())
                while (qk_idx < NQ or v_idx < NQ or score_q
                       or chain_idx < NQ or chain_gen is not None):
                    # 1. a slice of chain work (PE filler, no Act dependency)
                    if chain_gen is None and chain_idx < NQ:
                        bci, _ = chunks[chain_idx]
                        if (len(slabs[chain_idx]) == KI
                                and all(v_done[c] for c in
                                        range(4 * bci, 4 * bci + NO))):
                            chain_gen = gen_chains(chain_idx)
                    if chain_gen is not None:
                        for _ in range(3):
                            try:
                                next(chain_gen)
                            except StopIteration:
                                chain_gen = None
                                chain_idx += 1
                                chains_done += 1
                                update_score_q()
                                break
                    # 2. q/k projection pieces (gate scores)
                    if qkgen is None and qk_idx < NQ and qk_idx <= v_idx:
                        qkgen = gen_proj_qk(qk_idx)
                    if qkgen is not None:
                        steps = 1 if score_q else 4
                        for _ in range(steps):
                            try:
                                next(qkgen)
                            except StopIteration:
                                qk_done[qk_idx] = True
                                qk_idx += 1
                                if qk_idx + 1 < NQ:
                                    issue_xt(qk_idx + 1)
                                qkgen = None
                                update_score_q()
                                break
                    # 3. v projection + transposes (gate chains only)
                    if vgen is None and v_idx < NQ and v_idx < qk_idx:
                        vgen = gen_proj_v(v_idx)
                    if vgen is not None:
                        # boost only when idle or when chains are starved on v
                        chain_starved = (
                            chain_gen is None and chain_idx < NQ
                            and len(slabs[chain_idx]) == KI)
                        steps = 2 if (chain_starved or not score_q) else 1
                        for _ in range(steps):
                            try:
                                next(vgen)
                            except StopIteration:
                                v_done[v_idx] = True
                                v_idx += 1
                                vgen = None
                                break
                    # 4. one score (the Act engine's food)
                    if score_q:
                        i, k = score_q.pop(0)
                        emit_scores(i, k)
                        emitted.add((i, k))
                        update_score_q()
    nc.compile()
    return nc


# --------------------------------------------------------------------------
# Phase B: per-core row-parallel transpose + Wo-proj + AddNorm1 + FFN + AddNorm2.
# Inputs (per core, qi = this core's 512 token rows):
#   ctin [P, 4, KD, P] bf16   natural-layout ctx blocks for these rows
#   wo8  [P, KD, D]    bf16   Wo^T tiled
#   w18  [KD, P, KD, 512] bf16  W1^T tiled per 512-wide ffn-col group
#   w2a8/w2b8 [KD, P, 4, 512] bf16  W2^T tiled, first/second output half
#   xts8 [P, KD, 512] f32     X^T slice (residual 1)
#   consts [P, 72] f32        g1|be1|g2|be2|b1t|b2t feature-on-partition
#   id128b [P, P] bf16
# Output: out8 [P, KD, 512] f32 (t-layout output slice, tiled)
# --------------------------------------------------------------------------
def _build_phase_b():
    nc = bacc.Bacc("TRN2", target_bir_lowering=False, debug=False,
                   num_devices=N_CORES)
    ctin = nc.dram_tensor("ctin", [P, KD, 4, P], BF16, kind="ExternalInput")
    wo8 = nc.dram_tensor("wo8", [P, KD, D], BF16, kind="ExternalInput")
    w18 = nc.dram_tensor("w18", [KD, P, KD, 512], BF16, kind="ExternalInput")
    w2a8 = nc.dram_tensor("w2a8", [KD, P, 4, 512], BF16, kind="ExternalInput")
    w2b8 = nc.dram_tensor("w2b8", [KD, P, 4, 512], BF16, kind="ExternalInput")
    xts8 = nc.dram_tensor("xts8", [P, KD, 512], F32, kind="ExternalInput")
    consts = nc.dram_tensor("consts", [P, 72], F32, kind="ExternalInput")
    id128b = nc.dram_tensor("id128b", [P, P], BF16, kind="ExternalInput")
    out8 = nc.dram_tensor("out8", [P, KD, 512], BF16, kind="ExternalOutput")

    with tile.TileContext(nc) as tc:
        with (
            tc.tile_pool(name="persist", bufs=1) as persist,
            tc.tile_pool(name="w1p", bufs=3) as w1p,
            tc.tile_pool(name="w2p", bufs=3) as w2p,
            tc.tile_pool(name="sqp", bufs=3) as sqp,
            tc.tile_pool(name="smallp", bufs=2) as smallp,
            tc.tile_pool(name="bcp", bufs=2) as bcp,
        ):
            ctin_sb = persist.tile([P, KD, 4, P], BF16)
            ct_sb = persist.tile([P, KD, 4, P], BF16)
            wo_sb = persist.tile([P, KD, D], BF16)
            xts_sb = persist.tile([P, KD, 512], F32)
            y1_sb = persist.tile([P, KD, 512], BF16)
            z1_sb = persist.tile([P, KD, 512], BF16)
            h_sb = persist.tile([P, KF, 512], BF16)
            w2b_sb = persist.tile([P, KD, 4, 512], BF16)
            y2_sb = persist.tile([P, KD, 512], BF16, tag="y1_sb")
            z2_sb = persist.tile([P, KD, 512], BF16, tag="xts_sb")
            consts_sb = persist.tile([P, 72], F32)
            id128_sb = persist.tile([P, P], BF16)
            ones = persist.tile([P, 1], BF16)

            nc.sync.dma_start(ctin_sb[:, 0:2], ctin[:, 0:2])
            nc.sync.dma_start(id128_sb[:], id128b.ap())
            nc.sync.dma_start(wo_sb[:, 0:2], wo8[:, 0:2])
            nc.sync.dma_start(ctin_sb[:, 2:8], ctin[:, 2:8])
            nc.sync.dma_start(wo_sb[:, 2:8], wo8[:, 2:8])
            nc.sync.dma_start(consts_sb[:], consts.ap())
            nc.sync.dma_start(xts_sb[:], xts8.ap())
            nc.vector.memset(ones[:], 1.0)

            g1 = [consts_sb[:, kc:kc + 1] for kc in range(KD)]
            be1 = [consts_sb[:, 8 + kc:9 + kc] for kc in range(KD)]
            g2 = [consts_sb[:, 16 + kc:17 + kc] for kc in range(KD)]
            be2 = [consts_sb[:, 24 + kc:25 + kc] for kc in range(KD)]
            b1c = [consts_sb[:, 32 + fm:33 + fm] for fm in range(KF)]
            b2c = [consts_sb[:, 64 + kc:65 + kc] for kc in range(KD)]

            def ln_finish(st_sum, st_sq, tag):
                """Stats -> (rstd_b, ms_b) broadcast tiles."""
                mean = smallp.tile([1, 512], F32, name="mean")
                ex2 = smallp.tile([1, 512], F32, name="ex2")
                nc.vector.tensor_scalar(out=mean[:], in0=st_sum[:],
                                        scalar1=1.0 / D, scalar2=None,
                                        op0=OP.mult)
                nc.vector.tensor_scalar(out=ex2[:], in0=st_sq[:],
                                        scalar1=1.0 / D, scalar2=None,
                                        op0=OP.mult)
                msq = smallp.tile([1, 512], F32, name="msq")
                nc.vector.tensor_mul(msq[:], mean[:], mean[:])
                var = smallp.tile([1, 512], F32, name="var")
                nc.vector.tensor_sub(var[:], ex2[:], msq[:])
                nc.vector.tensor_scalar_add(var[:], var[:], EPS)
                std = smallp.tile([1, 512], F32, name="std")
                nc.scalar.activation(std[:], var[:], AF.Sqrt)
                rstd = smallp.tile([1, 512], BF16, name="rstd")
                ms = smallp.tile([1, 512], BF16, name="ms")
                with nc.allow_low_precision(reason="bf16 LN scale factors"):
                    nc.vector.reciprocal(rstd[:], std[:])
                nc.vector.tensor_mul(ms[:], mean[:], rstd[:])
                rstd_b = bcp.tile([P, 512], BF16, name="rstd_b")
                ms_b = bcp.tile([P, 512], BF16, name="ms_b")
                nc.gpsimd.partition_broadcast(rstd_b[:], rstd[:])
                nc.gpsimd.partition_broadcast(ms_b[:], ms[:])
                return rstd_b, ms_b

            def ln_apply(y_sb, rstd_b, ms_b, g_c, be_c, z_sb, kc, eng=None):
                eng = eng or nc.vector
                t = sqp.tile([P, 512], BF16, name="t_ln")
                eng.tensor_mul(t[:], y_sb[:, kc], rstd_b[:])
                eng.tensor_sub(t[:], t[:], ms_b[:])
                eng.tensor_scalar(out=z_sb[:, kc], in0=t[:],
                                  scalar1=g_c[kc], scalar2=be_c[kc],
                                  op0=OP.mult, op1=OP.add)

            # ---- B0+B1: transpose ctx to t-layout, fused with Wo matmuls ----
            with (
                tc.tile_pool(name="tpp", bufs=2, space="PSUM") as tpp,
                tc.tile_pool(name="psa", bufs=1, space="PSUM") as psa,
                tc.tile_pool(name="psst1", bufs=1, space="PSUM") as psst1,
            ):
                st1_sum = psst1.tile([1, 512], F32, name="st1_sum")
                st1_sq = psst1.tile([1, 512], F32, name="st1_sq")
                # chain-major Wo: each output chain stops early so its
                # residual add + square overlap the following chains
                a_ps = [psa.tile([P, 512], F32, name=f"mm_ps{i}")
                        for i in range(4)]
                sqs = {}

                def y1_add_sq(m, ps):
                    nc.vector.tensor_add(y1_sb[:, m], ps[:], xts_sb[:, m])
                    sq = sqp.tile([P, 512], BF16, name="sq", bufs=9)
                    nc.vector.tensor_mul(sq[:], y1_sb[:, m], y1_sb[:, m])
                    sqs[m] = sq

                for kc in range(KD):
                    tp = tpp.tile([P, 4, P], BF16, name="tp_ps")
                    for jb in range(4):
                        nc.tensor.transpose(tp[:, jb, :],
                                            ctin_sb[:, kc, jb, :],
                                            id128_sb[:])
                    nc.vector.tensor_copy(ct_sb[:, kc], tp[:])
                    nc.tensor.matmul(a_ps[0][:], wo_sb[:, kc, 0:P],
                                     ct_sb[:, kc],
                                     start=(kc == 0), stop=(kc == KD - 1))
                y1_add_sq(0, a_ps[0])
                for i in range(1, 4):
                    for kc in range(KD):
                        nc.tensor.matmul(
                            a_ps[i][:], wo_sb[:, kc, i * P:(i + 1) * P],
                            ct_sb[:, kc], start=(kc == 0), stop=(kc == KD - 1))
                    y1_add_sq(i, a_ps[i])
                a_ps2 = [psa.tile([P, 512], F32, name=f"mm_ps{i}")
                         for i in range(4)]
                for i in range(4):
                    for kc in range(KD):
                        nc.tensor.matmul(
                            a_ps2[i][:],
                            wo_sb[:, kc, 512 + i * P:512 + (i + 1) * P],
                            ct_sb[:, kc], start=(kc == 0), stop=(kc == KD - 1))
                    y1_add_sq(4 + i, a_ps2[i])
                for kc in range(KD):
                    nc.tensor.matmul(st1_sum[:], ones[:], y1_sb[:, kc],
                                     start=(kc == 0), stop=(kc == KD - 1))
                for kc in range(KD):
                    nc.tensor.matmul(st1_sq[:], ones[:], sqs[kc][:],
                                     start=(kc == 0), stop=(kc == KD - 1))
                sqs.clear()
                rstd_b1, ms_b1 = ln_finish(st1_sum, st1_sq, "ln1")
                for kc in range(KD):
                    ln_apply(y1_sb, rstd_b1, ms_b1, g1, be1, z1_sb, kc)

            # ---- FFN1 + FFN2 (first output half interleaved) ----
            with tc.tile_pool(name="psa2", bufs=1, space="PSUM") as psa2:
                f_ps = [psa2.tile([P, 512], F32, name=f"f_ps{i}")
                        for i in range(4)]
                for fg in range(KD):
                    w1_tile = w1p.tile([P, KD, 512], BF16, name="w1_tile")
                    nc.sync.dma_start(w1_tile[:], w18[fg])
                    h_ps = [psa2.tile([P, 512], F32, name=f"h_ps{i}")
                            for i in range(4)]
                    for kc in range(KD):
                        for i in range(4):
                            nc.tensor.matmul(
                                h_ps[i][:], w1_tile[:, kc, i * P:(i + 1) * P],
                                z1_sb[:, kc], start=(kc == 0), stop=(kc == KD - 1))
                    for i in range(4):
                        fm = fg * 4 + i
                        nc.scalar.activation(h_sb[:, fm], h_ps[i][:], AF.Relu,
                                             bias=b1c[fm])
                    w2_tile = w2p.tile([P, 4, 512], BF16, name="w2_tile")
                    nc.sync.dma_start(w2_tile[:], w2a8[fg])
                    nc.sync.dma_start(w2b_sb[:, fg], w2b8[fg])
                    for i in range(4):
                        fk = fg * 4 + i
                        for j in range(4):
                            nc.tensor.matmul(
                                f_ps[j][:], w2_tile[:, i, j * P:(j + 1) * P],
                                h_sb[:, fk], start=(fk == 0), stop=(fk == KF - 1))
                # y2 first half + squares; LN2 stats for it run behind
                # FFN2b's first chain
                st2_sum = psa2.tile([1, 512], F32, name="h_ps0", tag="h_ps0")
                st2_sq = psa2.tile([1, 512], F32, name="h_ps1", tag="h_ps1")
                sqs2 = {}

                def y2_add_sq(m, ps):
                    nc.vector.scalar_tensor_tensor(
                        out=y2_sb[:, m], in0=ps[:], scalar=b2c[m],
                        in1=z1_sb[:, m], op0=OP.add, op1=OP.add)
                    sq = sqp.tile([P, 512], BF16, name="sq", bufs=9)
                    nc.vector.tensor_mul(sq[:], y2_sb[:, m], y2_sb[:, m])
                    sqs2[m] = sq

                for j in range(4):
                    y2_add_sq(j, f_ps[j])

                # chain-major FFN2 second half: chain j finishes early so its
                # add/square/stat matmuls overlap chain j+1
                f_ps2 = [psa2.tile([P, 512], F32, name=f"f_ps{i}")
                         for i in range(4)]
                for j in range(4):
                    for fg in range(KD):
                        for i in range(4):
                            nc.tensor.matmul(
                                f_ps2[j][:],
                                w2b_sb[:, fg, i, j * P:(j + 1) * P],
                                h_sb[:, fg * 4 + i],
                                start=(fg == 0 and i == 0),
                                stop=(fg == KD - 1 and i == 3))
                    m = 4 + j
                    y2_add_sq(m, f_ps2[j])
                    if j == 0:
                        for kc in range(4):
                            nc.tensor.matmul(
                                st2_sum[:], ones[:], y2_sb[:, kc],
                                start=(kc == 0), stop=False)
                            nc.tensor.matmul(
                                st2_sq[:], ones[:], sqs2[kc][:],
                                start=(kc == 0), stop=False)
                    else:
                        mm1 = 4 + j - 1
                        nc.tensor.matmul(st2_sum[:], ones[:], y2_sb[:, mm1],
                                         start=False, stop=False)
                        nc.tensor.matmul(st2_sq[:], ones[:], sqs2[mm1][:],
                                         start=False, stop=False)
                nc.tensor.matmul(st2_sum[:], ones[:], y2_sb[:, 7],
                                 start=False, stop=True)
                nc.tensor.matmul(st2_sq[:], ones[:], sqs2[7][:],
                                 start=False, stop=True)
                rstd_b2, ms_b2 = ln_finish(st2_sum, st2_sq, "ln2")
                for kc in range(KD):
                    ln_apply(y2_sb, rstd_b2, ms_b2, g2, be2, z2_sb, kc)
                    nc.sync.dma_start(out8[:, kc], z2_sb[:, kc])
    nc.compile()
    return nc


def _get(name, builder):
    if name not in _CACHE:
        _CACHE[name] = builder()
    return _CACHE[name]


def _prep_inputs(X, Wq, Wk, Wo, ln1_g, ln1_b, ln2_g, ln2_b, W1, b1, W2, b2):
    f = lambda a: np.ascontiguousarray(a)
    Xt = np.asarray(X, np.float32).reshape(N, D).T          # [D, N]
    WqT = np.asarray(Wq, np.float32).T                      # [D, D]
    WkT = np.asarray(Wk, np.float32).T
    WoT = np.asarray(Wo, np.float32).T
    W1T = np.asarray(W1, np.float32).T                      # [D, FF]
    W2T = np.asarray(W2, np.float32).T                      # [FF, D]
    vecP = lambda v, k: np.asarray(v, np.float32).reshape(k, P).T  # [P, k]

    # xt8: [idx, p, kc, q]
    xt8 = f(Xt.reshape(KD, P, NQ, 512).transpose(2, 1, 0, 3))
    id64b = np.tile(np.eye(DH, dtype=np.float32), (2, 1)).astype(BF)
    w_tile = lambda w, c: f(
        w[:, c * P:(c + 1) * P].reshape(KD, P, P).transpose(1, 0, 2))

    in_maps_a = [
        {
            "xt8": xt8,
            "id64b": id64b,
            "wqt8": w_tile(WqT, c),
            "wkt8": w_tile(WkT, c),
            "wvt8": w_tile(WoT, c),   # value projection uses W_o in this model
        }
        for c in range(N_CORES)
    ]

    wo8 = f(WoT.reshape(KD, P, D).transpose(1, 0, 2).astype(BF))
    w18 = f(W1T.reshape(KD, P, KD, 512).transpose(2, 1, 0, 3).astype(BF))
    w2a8 = f(W2T[:, 0:512].reshape(KD, 4, P, 512).transpose(0, 2, 1, 3)
             .astype(BF))
    w2b8 = f(W2T[:, 512:1024].reshape(KD, 4, P, 512).transpose(0, 2, 1, 3)
             .astype(BF))
    consts = f(np.hstack([vecP(ln1_g, KD), vecP(ln1_b, KD), vecP(ln2_g, KD),
                          vecP(ln2_b, KD), vecP(b1, KF), vecP(b2, KD)]))
    id128b = np.eye(P, dtype=BF)

    def in_maps_b(full_nat):
        maps = []
        for c in range(N_CORES):
            blk = full_nat[c * QC:(c + 1) * QC]             # [512, 1024]
            ctin = f(blk.reshape(4, P, KD, P).transpose(1, 2, 0, 3).astype(BF))
            xts = f(Xt[:, c * QC:(c + 1) * QC].reshape(KD, P, 512)
                    .transpose(1, 0, 2))
            maps.append({
                "ctin": ctin, "xts8": xts, "wo8": wo8, "w18": w18,
                "w2a8": w2a8, "w2b8": w2b8, "consts": consts,
                "id128b": id128b,
            })
        return maps

    return in_maps_a, in_maps_b


def kernel(X, Wq, Wk, Wo, ln1_g, ln1_b, ln2_g, ln2_b, W1, b1, W2, b2):
    in_maps_a, in_maps_b = _prep_inputs(
        X, Wq, Wk, Wo, ln1_g, ln1_b, ln2_g, ln2_b, W1, b1, W2, b2)

    nc_a = _get("a", _build_phase_a)
    res_a = run_bass_kernel_spmd(nc_a, in_maps_a, core_ids=list(range(N_CORES)))
    # ctxn [P, NQ, 4, P] per core -> natural [4096, 128] -> concat cols
    full_nat = np.concatenate(
        [res_a.results[c]["ctxn"].transpose(1, 2, 0, 3).reshape(N, P)
         for c in range(N_CORES)], axis=1)                  # [N, D]

    nc_b = _get("b", _build_phase_b)
    res_b = run_bass_kernel_spmd(nc_b, in_maps_b(full_nat),
                                 core_ids=list(range(N_CORES)))
    # out8 [P, KD, 512] per core -> [D, 512] col block of out^T
    out_t = np.concatenate(
        [res_b.results[c]["out8"].astype(np.float32).transpose(1, 0, 2)
         .reshape(D, QC) for c in range(N_CORES)], axis=1)  # [D, N]
    return np.ascontiguousarray(out_t.T).reshape(B, S, D).astype(np.float32)


# revision 33
# speedup vs baseline: 1.1243x; 1.0235x over previous
"""Trainium2 Bass kernel for nn_Encoder (dense transformer block), 8 NeuronCores.

Strategy (single chip, 8 cores), v3:
  Phase A (head-parallel): core c computes attention for heads {2c, 2c+1}.
    Projections run in t-layout; q/k land in bf16, V is PE-transposed into
    natural [keys, dims] bf16 layout. softmax(relu(s)) is p = max(exp(s/8), 1)
    with the softmax denominator taken from a ones column appended to V.
    The exp pass on the Activation engine is the critical resource (~134 us);
    a queue-based emitter keeps it saturated: score matmuls are emitted as
    early as their projections allow (wavefront), projection matmuls are
    spread between them in small pieces, and the context-accumulation chains
    (65-cycle bf16 matmuls in the fast [q,65] orientation) fill the PE's
    exp-paced slack. ctx leaves phase A in natural [token, dim] layout.
  Phase B (row-parallel): core c takes 512 of the 4096 token rows. It
    PE-transposes the incoming ctx back to t-layout fused with the Wo
    matmuls, then AddNorm1, FFN (ReLU, bf16 weights/activations), AddNorm2.
    All weights stream as a handful of large host-pre-tiled bf16 DMAs on the
    SP queue; LayerNorm statistics are accumulated in halves so their matmuls
    and squares overlap the surrounding GEMMs.
"""

import os
import sys

for _p in ("/opt/trn_rl_repo",):
    if _p not in sys.path:
        sys.path.insert(0, _p)

# The Bass SPMD path executes through jax/PJRT on the axon platform; make
# sure a caller-pinned JAX_PLATFORMS=cpu doesn't hide the NeuronCores.
_jp = os.environ.get("JAX_PLATFORMS")
if _jp is not None and "axon" not in _jp:
    os.environ["JAX_PLATFORMS"] = "axon," + _jp

import ml_dtypes
import numpy as np

import concourse.bass as bass
import concourse.mybir as mybir
import concourse.tile as tile
from concourse import bacc
from concourse.bass_utils import run_bass_kernel_spmd

F32 = mybir.dt.float32
F32R = mybir.dt.float32r
BF16 = mybir.dt.bfloat16
AF = mybir.ActivationFunctionType
OP = mybir.AluOpType
BF = ml_dtypes.bfloat16


def _mm(nc, out, lhsT, rhs, **kw):
    # fp32r: 1-pass FP22 matmul (1 cyc/row when the moving dim is >= 256)
    nc.tensor.matmul(out, lhsT.bitcast(F32R), rhs.bitcast(F32R), **kw)


N_CORES = 8
B, S, D, H, DH, FF = 2, 2048, 1024, 16, 64, 4096
N = B * S            # 4096 token rows
P = 128
QC = N // N_CORES    # 512 rows per core in phase B
KD = D // P          # 8 contraction chunks over D
KI = S // P          # 16 key chunks of 128 per batch
NO = S // 512        # 4 query chunks of 512 per batch
NQ = N // 512        # 8 query chunks overall
KF = FF // P         # 32
EPS = 1e-5

_CACHE = {}


# --------------------------------------------------------------------------
# Phase A: per-core head-parallel attention.
# Inputs (per core):
#   xt8  [NQ, P, KD, 512]  X^T tiled per 512-token chunk (replicated)
#   wqt8/wkt8/wvt8 [P, KD, P]  W^T columns for this core's two heads, tiled
#   id64b [P, DH] bf16 tiled identity (V transposes)
# Output:
#   ctxn [P, NQ, 4, P] f32: natural-layout ctx; token = idx*512 + j*128 + p,
#   col = the two heads' 64-dim blocks concatenated.
# --------------------------------------------------------------------------
def _build_phase_a():
    nc = bacc.Bacc("TRN2", target_bir_lowering=False, debug=False,
                   num_devices=N_CORES)
    xt8 = nc.dram_tensor("xt8", [NQ, P, KD, 512], F32R, kind="ExternalInput")
    wqt8 = nc.dram_tensor("wqt8", [P, KD, P], F32R, kind="ExternalInput")
    wkt8 = nc.dram_tensor("wkt8", [P, KD, P], F32R, kind="ExternalInput")
    wvt8 = nc.dram_tensor("wvt8", [P, KD, P], F32R, kind="ExternalInput")
    id64b = nc.dram_tensor("id64b", [P, DH], BF16, kind="ExternalInput")
    ctxn = nc.dram_tensor("ctxn", [P, NQ, 4, P], F32, kind="ExternalOutput")

    chunks = [(b_, o) for b_ in range(B) for o in range(NO)]

    with tile.TileContext(nc) as tc:
        with tc.tile_pool(name="persist", bufs=1) as persist:
            qt_sb = [persist.tile([P, S], BF16, name=f"qt{b_}") for b_ in range(B)]
            kt_sb = [persist.tile([P, S], BF16, name=f"kt{b_}") for b_ in range(B)]
            vt_sb = [persist.tile([P, S], BF16, name=f"vt{b_}") for b_ in range(B)]
            vp_sb = [persist.tile([P, KI, 2, DH + 1], BF16, name=f"vp{b_}")
                     for b_ in range(B)]
            wq_sb = persist.tile([P, KD, P], F32R)
            wk_sb = persist.tile([P, KD, P], F32R)
            wv_sb = persist.tile([P, KD, P], F32R)
            id64_sb = persist.tile([P, DH], BF16)

            for b_ in range(B):
                nc.vector.memset(vp_sb[b_][:, :, 0, DH:DH + 1], 1.0)
                nc.vector.memset(vp_sb[b_][:, :, 1, DH:DH + 1], 1.0)

            with (
                tc.tile_pool(name="xpool", bufs=2) as xpool,
                tc.tile_pool(name="accp", bufs=2, space="PSUM") as accp,
                tc.tile_pool(name="slabp", bufs=50) as slabp,
                tc.tile_pool(name="stagep", bufs=2) as stagep,
                tc.tile_pool(name="smallp", bufs=8) as smallp,
                tc.tile_pool(name="pss", bufs=2, space="PSUM") as pss,
                tc.tile_pool(name="psc", bufs=2, space="PSUM") as psc,
            ):
                xt_tiles = {}

                def issue_xt(ci):
                    t = xpool.tile([P, KD, 512], F32R, name="xt_tile")
                    # two half-DMAs so the first projection matmuls can start
                    # as soon as the front half lands (subtile deps)
                    nc.sync.dma_start(t[:, 0:4], xt8[ci, :, 0:4])
                    nc.sync.dma_start(t[:, 4:8], xt8[ci, :, 4:8])
                    xt_tiles[ci] = t

                def gen_proj_qk(ci):
                    """Generator: project chunk ci into qt/kt (bf16). For the
                    first chunk the q and k chains interleave so both finish
                    (and the first scores emit) as early as possible."""
                    b_, o = chunks[ci]
                    osl = slice(o * 512, (o + 1) * 512)
                    xt_tile = xt_tiles[ci]
                    if ci == 0:
                        accq = accp.tile([P, 512], F32, name="acc_ps",
                                         tag="acc")
                        acck = accp.tile([P, 512], F32, name="acc_ps",
                                         tag="acc")
                        for kc in range(KD):
                            _mm(nc, accq[:], wq_sb[:, kc], xt_tile[:, kc],
                                start=(kc == 0), stop=(kc == KD - 1))
                            _mm(nc, acck[:], wk_sb[:, kc], xt_tile[:, kc],
                                start=(kc == 0), stop=(kc == KD - 1))
                            if kc % 2 == 1:
                                yield
                        nc.vector.tensor_copy(qt_sb[b_][:, osl], accq[:])
                        nc.vector.tensor_copy(kt_sb[b_][:, osl], acck[:])
                        yield
                        return
                    for w_sb, dst in ((wq_sb, qt_sb[b_]), (wk_sb, kt_sb[b_])):
                        acc = accp.tile([P, 512], F32, name="acc_ps", tag="acc")
                        for kc in range(KD):
                            _mm(nc, acc[:], w_sb[:, kc], xt_tile[:, kc],
                                start=(kc == 0), stop=(kc == KD - 1))
                            if kc % 2 == 1:
                                yield
                        nc.vector.tensor_copy(dst[:, osl], acc[:])
                        yield

                def gen_proj_v(ci):
                    """Generator: V projection + natural-layout transposes."""
                    b_, o = chunks[ci]
                    osl = slice(o * 512, (o + 1) * 512)
                    xt_tile = xt_tiles.pop(ci)
                    acc = accp.tile([P, 512], F32, name="acc_ps", tag="acc")
                    for kc in range(KD):
                        _mm(nc, acc[:], wv_sb[:, kc], xt_tile[:, kc],
                            start=(kc == 0), stop=(kc == KD - 1))
                        if kc % 2 == 1:
                            yield
                    nc.vector.tensor_copy(vt_sb[b_][:, osl], acc[:])
                    yield
                    for t in range(4):
                        kc2 = o * 4 + t
                        for hh in range(2):
                            tp = accp.tile([P, DH], BF16, name="tp_ps",
                                           tag="acc")
                            nc.tensor.transpose(
                                tp[:, 0:DH],
                                vt_sb[b_][hh * DH:(hh + 1) * DH,
                                          kc2 * P:(kc2 + 1) * P],
                                id64_sb[hh * DH:(hh + 1) * DH, :])
                            nc.vector.tensor_copy(
                                vp_sb[b_][:, kc2, hh, 0:DH], tp[:, 0:DH])
                        yield

                slabs = {i: {} for i in range(NQ)}   # idx -> kc -> slab tile
                stages = {}

                def emit_scores(idx, kc):
                    b_, o = chunks[idx]
                    qs = slice(o * 512, (o + 1) * 512)
                    ks = slice(kc * P, (kc + 1) * P)
                    s_ps = pss.tile([P, 1024], F32, name="s_ps")
                    nc.tensor.matmul(s_ps[:, 0:512], kt_sb[b_][0:DH, ks],
                                     qt_sb[b_][0:DH, qs], start=True, stop=True)
                    nc.tensor.matmul(s_ps[:, 512:1024], kt_sb[b_][DH:2 * DH, ks],
                                     qt_sb[b_][DH:2 * DH, qs],
                                     start=True, stop=True)
                    slab = slabp.tile([P, 1024], BF16, name="slab")
                    nc.scalar.activation(slab[:], s_ps[:], AF.Exp, scale=0.125)
                    nc.vector.tensor_scalar_max(slab[:], slab[:], 1.0)
                    slabs[idx][kc] = slab

                def gen_chains(idx):
                    """Generator: the 8 ctx chains of idx + normalize + DMA,
                    yielding every couple of matmuls."""
                    b_, o = chunks[idx]
                    stage = stagep.tile([P, 4, P], F32, name="stage")
                    for ci in range(8):
                        j, h = ci // 2, ci % 2
                        acc = psc.tile([P, DH + 1], F32, name="ctx_ps")
                        for kc in range(KI):
                            nc.tensor.matmul(
                                acc[:],
                                slabs[idx][kc][:, h * 512 + j * P:
                                               h * 512 + (j + 1) * P],
                                vp_sb[b_][:, kc, h, :],
                                start=(kc == 0), stop=(kc == KI - 1))
                            if kc % 4 == 3:
                                yield
                        inv = smallp.tile([P, 1], F32, name="inv")
                        nc.vector.reciprocal(inv[:], acc[:, DH:DH + 1])
                        nc.vector.tensor_scalar(
                            out=stage[:, j, h * DH:(h + 1) * DH],
                            in0=acc[:, 0:DH], scalar1=inv[:], scalar2=None,
                            op0=OP.mult)
                        yield
                    nc.sync.dma_start(ctxn[:, idx], stage[:])
                    slabs[idx].clear()

                # ---------------- queue-based emitter ----------------
                emitted = set()           # (idx, kc) scores emitted
                score_q = []              # ordered pending scores
                qk_done = [False] * NQ
                v_done = [False] * NQ
                chains_done = 0           # count of fully-emitted chain idxs
                chain_gen = None
                chain_idx = 0             # next idx needing chains
                qk_idx = 0                # next chunk for q/k projection
                v_idx = 0                 # next chunk for v projection
                qkgen = None
                vgen = None

                def update_score_q():
                    for i in range(NQ):
                        bi, _ = chunks[i]
                        if not qk_done[i]:
                            continue
                        if i >= chains_done + 3:
                            continue
                        base = 4 * bi
                        kmax = sum(4 for c in range(base, base + NO)
                                   if qk_done[c])
                        for k in range(kmax):
                            if (i, k) not in emitted and (i, k) not in score_q:
                                score_q.append((i, k))

                # DMA order tuned so the first q-projection matmuls can
                # start at ~4.5us: wq, then the first xt half, then wk etc.
                nc.sync.dma_start(wq_sb[:], wqt8.ap())
                issue_xt(0)
                nc.sync.dma_start(wk_sb[:], wkt8.ap())
                nc.sync.dma_start(wv_sb[:], wvt8.ap())
                issue_xt(1)
                nc.sync.dma_start(id64_sb[:], id64b.ap())
                while (qk_idx < NQ or v_idx < NQ or score_q
                       or chain_idx < NQ or chain_gen is not None):
                    # 1. a slice of chain work (PE filler, no Act dependency)
                    if chain_gen is None and chain_idx < NQ:
                        bci, _ = chunks[chain_idx]
                        if (len(slabs[chain_idx]) == KI
                                and all(v_done[c] for c in
                                        range(4 * bci, 4 * bci + NO))):
                            chain_gen = gen_chains(chain_idx)
                    if chain_gen is not None:
                        for _ in range(3 if score_q else 8):
                            try:
                                next(chain_gen)
                            except StopIteration:
                                chain_gen = None
                                chain_idx += 1
                                chains_done += 1
                                update_score_q()
                                break
                    # 2. q/k projection pieces (gate scores)
                    if qkgen is None and qk_idx < NQ and qk_idx <= v_idx:
                        qkgen = gen_proj_qk(qk_idx)
                    if qkgen is not None:
                        steps = 1 if score_q else 4
                        for _ in range(steps):
                            try:
                                next(qkgen)
                            except StopIteration:
                                qk_done[qk_idx] = True
                                qk_idx += 1
                                if qk_idx + 1 < NQ:
                                    issue_xt(qk_idx + 1)
                                qkgen = None
                                update_score_q()
                                break
                    # 3. v projection + transposes (gate chains only)
                    if vgen is None and v_idx < NQ and v_idx < qk_idx:
                        vgen = gen_proj_v(v_idx)
                    if vgen is not None:
                        # boost only when idle or when chains are starved on v
                        chain_starved = (
                            chain_gen is None and chain_idx < NQ
                            and len(slabs[chain_idx]) == KI)
                        steps = 2 if (chain_starved or not score_q
                                      or v_idx < qk_idx - 1) else 1
                        for _ in range(steps):
                            try:
                                next(vgen)
                            except StopIteration:
                                v_done[v_idx] = True
                                v_idx += 1
                                vgen = None
                                break
                    # 4. one score (the Act engine's food)
                    if score_q:
                        i, k = score_q.pop(0)
                        emit_scores(i, k)
                        emitted.add((i, k))
                        update_score_q()
    nc.compile()
    return nc


# --------------------------------------------------------------------------
# Phase B: per-core row-parallel transpose + Wo-proj + AddNorm1 + FFN + AddNorm2.
# Inputs (per core, qi = this core's 512 token rows):
#   ctin [P, 4, KD, P] bf16   natural-layout ctx blocks for these rows
#   wo8  [P, KD, D]    bf16   Wo^T tiled
#   w18  [KD, P, KD, 512] bf16  W1^T tiled per 512-wide ffn-col group
#   w2a8/w2b8 [KD, P, 4, 512] bf16  W2^T tiled, first/second output half
#   xts8 [P, KD, 512] f32     X^T slice (residual 1)
#   consts [P, 72] f32        g1|be1|g2|be2|b1t|b2t feature-on-partition
#   id128b [P, P] bf16
# Output: out8 [P, KD, 512] f32 (t-layout output slice, tiled)
# --------------------------------------------------------------------------
def _build_phase_b():
    nc = bacc.Bacc("TRN2", target_bir_lowering=False, debug=False,
                   num_devices=N_CORES)
    ctin = nc.dram_tensor("ctin", [P, KD, 4, P], BF16, kind="ExternalInput")
    wo8 = nc.dram_tensor("wo8", [P, KD, D], BF16, kind="ExternalInput")
    w18 = nc.dram_tensor("w18", [KD, P, KD, 512], BF16, kind="ExternalInput")
    w2a8 = nc.dram_tensor("w2a8", [KD, P, 4, 512], BF16, kind="ExternalInput")
    w2b8 = nc.dram_tensor("w2b8", [KD, P, 4, 512], BF16, kind="ExternalInput")
    xts8 = nc.dram_tensor("xts8", [P, KD, 512], F32, kind="ExternalInput")
    consts = nc.dram_tensor("consts", [P, 72], F32, kind="ExternalInput")
    id128b = nc.dram_tensor("id128b", [P, P], BF16, kind="ExternalInput")
    out8 = nc.dram_tensor("out8", [P, KD, 512], BF16, kind="ExternalOutput")

    with tile.TileContext(nc) as tc:
        with (
            tc.tile_pool(name="persist", bufs=1) as persist,
            tc.tile_pool(name="w1p", bufs=3) as w1p,
            tc.tile_pool(name="w2p", bufs=3) as w2p,
            tc.tile_pool(name="sqp", bufs=3) as sqp,
            tc.tile_pool(name="smallp", bufs=2) as smallp,
            tc.tile_pool(name="bcp", bufs=2) as bcp,
        ):
            ctin_sb = persist.tile([P, KD, 4, P], BF16)
            ct_sb = persist.tile([P, KD, 4, P], BF16)
            wo_sb = persist.tile([P, KD, D], BF16)
            xts_sb = persist.tile([P, KD, 512], F32)
            y1_sb = persist.tile([P, KD, 512], BF16)
            z1_sb = persist.tile([P, KD, 512], BF16)
            h_sb = persist.tile([P, KF, 512], BF16)
            w2b_sb = persist.tile([P, KD, 4, 512], BF16)
            y2_sb = persist.tile([P, KD, 512], BF16, tag="y1_sb")
            z2_sb = persist.tile([P, KD, 512], BF16, tag="xts_sb")
            consts_sb = persist.tile([P, 72], F32)
            id128_sb = persist.tile([P, P], BF16)
            ones = persist.tile([P, 1], BF16)

            nc.sync.dma_start(ctin_sb[:, 0:2], ctin[:, 0:2])
            nc.sync.dma_start(id128_sb[:], id128b.ap())
            nc.sync.dma_start(wo_sb[:, 0:2], wo8[:, 0:2])
            nc.sync.dma_start(ctin_sb[:, 2:8], ctin[:, 2:8])
            nc.sync.dma_start(wo_sb[:, 2:8], wo8[:, 2:8])
            nc.sync.dma_start(consts_sb[:], consts.ap())
            nc.sync.dma_start(xts_sb[:], xts8.ap())
            nc.vector.memset(ones[:], 1.0)

            g1 = [consts_sb[:, kc:kc + 1] for kc in range(KD)]
            be1 = [consts_sb[:, 8 + kc:9 + kc] for kc in range(KD)]
            g2 = [consts_sb[:, 16 + kc:17 + kc] for kc in range(KD)]
            be2 = [consts_sb[:, 24 + kc:25 + kc] for kc in range(KD)]
            b1c = [consts_sb[:, 32 + fm:33 + fm] for fm in range(KF)]
            b2c = [consts_sb[:, 64 + kc:65 + kc] for kc in range(KD)]

            def ln_finish(st_sum, st_sq, tag, w=512):
                """Stats (over w tokens) -> (rstd_b, ms_b) broadcast tiles."""
                mean = smallp.tile([1, w], F32, name="mean")
                ex2 = smallp.tile([1, w], F32, name="ex2")
                nc.vector.tensor_scalar(out=mean[:], in0=st_sum,
                                        scalar1=1.0 / D, scalar2=None,
                                        op0=OP.mult)
                nc.vector.tensor_scalar(out=ex2[:], in0=st_sq,
                                        scalar1=1.0 / D, scalar2=None,
                                        op0=OP.mult)
                msq = smallp.tile([1, w], F32, name="msq")
                nc.vector.tensor_mul(msq[:], mean[:], mean[:])
                var = smallp.tile([1, w], F32, name="var")
                nc.vector.tensor_sub(var[:], ex2[:], msq[:])
                nc.vector.tensor_scalar_add(var[:], var[:], EPS)
                std = smallp.tile([1, w], F32, name="std")
                nc.scalar.activation(std[:], var[:], AF.Sqrt)
                rstd = smallp.tile([1, w], BF16, name="rstd")
                ms = smallp.tile([1, w], BF16, name="ms")
                with nc.allow_low_precision(reason="bf16 LN scale factors"):
                    nc.vector.reciprocal(rstd[:], std[:])
                nc.vector.tensor_mul(ms[:], mean[:], rstd[:])
                rstd_b = bcp.tile([P, w], BF16, name="rstd_b", bufs=3)
                ms_b = bcp.tile([P, w], BF16, name="ms_b", bufs=3)
                nc.gpsimd.partition_broadcast(rstd_b[:], rstd[:])
                nc.gpsimd.partition_broadcast(ms_b[:], ms[:])
                return rstd_b, ms_b

            def ln_apply(y_sb, rstd_b, ms_b, g_c, be_c, z_sb, kc, cols=None):
                cols = cols or slice(0, 512)
                w = cols.stop - cols.start
                t = sqp.tile([P, 512], BF16, name="t_ln")
                nc.vector.tensor_mul(t[:, 0:w], y_sb[:, kc, cols], rstd_b[:])
                nc.vector.tensor_sub(t[:, 0:w], t[:, 0:w], ms_b[:])
                nc.vector.tensor_scalar(out=z_sb[:, kc, cols], in0=t[:, 0:w],
                                        scalar1=g_c[kc], scalar2=be_c[kc],
                                        op0=OP.mult, op1=OP.add)

            # ---- B0+B1: transpose ctx to t-layout, fused with Wo matmuls ----
            with (
                tc.tile_pool(name="tpp", bufs=2, space="PSUM") as tpp,
                tc.tile_pool(name="psa", bufs=1, space="PSUM") as psa,
                tc.tile_pool(name="psst1", bufs=1, space="PSUM") as psst1,
            ):
                st1_sum = psst1.tile([1, 512], F32, name="st1_sum")
                st1_sq = psst1.tile([1, 512], F32, name="st1_sq")
                # chain-major Wo: each output chain stops early so its
                # residual add + square overlap the following chains
                a_ps = [psa.tile([P, 512], F32, name=f"mm_ps{i}")
                        for i in range(4)]
                sqs = {}

                def y1_add_sq(m, ps):
                    nc.vector.tensor_add(y1_sb[:, m], ps[:], xts_sb[:, m])
                    sq = sqp.tile([P, 512], BF16, name="sq", bufs=9)
                    nc.vector.tensor_mul(sq[:], y1_sb[:, m], y1_sb[:, m])
                    sqs[m] = sq

                for kc in range(KD):
                    tp = tpp.tile([P, 4, P], BF16, name="tp_ps")
                    for jb in range(4):
                        nc.tensor.transpose(tp[:, jb, :],
                                            ctin_sb[:, kc, jb, :],
                                            id128_sb[:])
                    nc.vector.tensor_copy(ct_sb[:, kc], tp[:])
                    nc.tensor.matmul(a_ps[0][:], wo_sb[:, kc, 0:P],
                                     ct_sb[:, kc],
                                     start=(kc == 0), stop=(kc == KD - 1))
                y1_add_sq(0, a_ps[0])
                for i in range(1, 4):
                    for kc in range(KD):
                        nc.tensor.matmul(
                            a_ps[i][:], wo_sb[:, kc, i * P:(i + 1) * P],
                            ct_sb[:, kc], start=(kc == 0), stop=(kc == KD - 1))
                    y1_add_sq(i, a_ps[i])
                a_ps2 = [psa.tile([P, 512], F32, name=f"mm_ps{i}")
                         for i in range(4)]
                for i in range(4):
                    for kc in range(KD):
                        nc.tensor.matmul(
                            a_ps2[i][:],
                            wo_sb[:, kc, 512 + i * P:512 + (i + 1) * P],
                            ct_sb[:, kc], start=(kc == 0), stop=(kc == KD - 1))
                    y1_add_sq(4 + i, a_ps2[i])
                for kc in range(KD):
                    nc.tensor.matmul(st1_sum[:], ones[:], y1_sb[:, kc],
                                     start=(kc == 0), stop=(kc == KD - 1))
                for kc in range(KD):
                    nc.tensor.matmul(st1_sq[:], ones[:], sqs[kc][:],
                                     start=(kc == 0), stop=(kc == KD - 1))
                sqs.clear()
                rstd_b1, ms_b1 = ln_finish(st1_sum[:], st1_sq[:], "ln1")
                for kc in range(KD):
                    ln_apply(y1_sb, rstd_b1, ms_b1, g1, be1, z1_sb, kc)

            # ---- FFN1 + FFN2 (first output half interleaved) ----
            with tc.tile_pool(name="psa2", bufs=1, space="PSUM") as psa2:
                f_ps = [psa2.tile([P, 512], F32, name=f"f_ps{i}")
                        for i in range(4)]
                for fg in range(KD):
                    w1_tile = w1p.tile([P, KD, 512], BF16, name="w1_tile")
                    nc.sync.dma_start(w1_tile[:], w18[fg])
                    h_ps = [psa2.tile([P, 512], F32, name=f"h_ps{i}")
                            for i in range(4)]
                    for kc in range(KD):
                        for i in range(4):
                            nc.tensor.matmul(
                                h_ps[i][:], w1_tile[:, kc, i * P:(i + 1) * P],
                                z1_sb[:, kc], start=(kc == 0), stop=(kc == KD - 1))
                    for i in range(4):
                        fm = fg * 4 + i
                        nc.scalar.activation(h_sb[:, fm], h_ps[i][:], AF.Relu,
                                             bias=b1c[fm])
                    w2_tile = w2p.tile([P, 4, 512], BF16, name="w2_tile")
                    nc.sync.dma_start(w2_tile[:], w2a8[fg])
                    nc.sync.dma_start(w2b_sb[:, fg], w2b8[fg])
                    for i in range(4):
                        fk = fg * 4 + i
                        for j in range(4):
                            nc.tensor.matmul(
                                f_ps[j][:], w2_tile[:, i, j * P:(j + 1) * P],
                                h_sb[:, fk], start=(fk == 0), stop=(fk == KF - 1))
                # y2 first half + squares; LN2 stats for it run behind
                # FFN2b's first chain
                st2_sum = psa2.tile([1, 512], F32, name="h_ps0", tag="h_ps0")
                st2_sq = psa2.tile([1, 512], F32, name="h_ps1", tag="h_ps1")
                sqs2 = {}

                def y2_add_sq(m, ps):
                    nc.vector.scalar_tensor_tensor(
                        out=y2_sb[:, m], in0=ps[:], scalar=b2c[m],
                        in1=z1_sb[:, m], op0=OP.add, op1=OP.add)
                    sq = sqp.tile([P, 512], BF16, name="sq", bufs=9)
                    nc.vector.tensor_mul(sq[:], y2_sb[:, m], y2_sb[:, m])
                    sqs2[m] = sq

                for j in range(4):
                    y2_add_sq(j, f_ps[j])

                # token-halved FFN2 second half: half 0's chains, stats,
                # and LN2 applies complete while half 1's chains run, so only
                # half 1's (narrower) LN2 epilogue remains on the tail
                f_ps2 = [psa2.tile([P, 512], F32, name=f"f_ps{i}")
                         for i in range(4)]
                for half in range(2):
                    hs = slice(half * 256, (half + 1) * 256)
                    for j in range(4):
                        m = 4 + j
                        for fg in range(KD):
                            for i in range(4):
                                nc.tensor.matmul(
                                    f_ps2[j][:, hs],
                                    w2b_sb[:, fg, i, j * P:(j + 1) * P],
                                    h_sb[:, fg * 4 + i, hs],
                                    start=(fg == 0 and i == 0),
                                    stop=(fg == KD - 1 and i == 3))
                        nc.vector.scalar_tensor_tensor(
                            out=y2_sb[:, m, hs], in0=f_ps2[j][:, hs],
                            scalar=b2c[m], in1=z1_sb[:, m, hs],
                            op0=OP.add, op1=OP.add)
                        sq = sqp.tile([P, 512], BF16, name="sq", bufs=9)
                        nc.vector.tensor_mul(sq[:, hs], y2_sb[:, m, hs],
                                             y2_sb[:, m, hs])
                        sqs2[(m, half)] = sq
                        if j == 0:
                            for kc in range(4):
                                nc.tensor.matmul(
                                    st2_sum[:, hs], ones[:],
                                    y2_sb[:, kc, hs],
                                    start=(kc == 0), stop=False)
                                nc.tensor.matmul(
                                    st2_sq[:, hs], ones[:],
                                    sqs2[kc][:, hs],
                                    start=(kc == 0), stop=False)
                        else:
                            mm1 = 4 + j - 1
                            nc.tensor.matmul(
                                st2_sum[:, hs], ones[:], y2_sb[:, mm1, hs],
                                start=False, stop=False)
                            nc.tensor.matmul(
                                st2_sq[:, hs], ones[:],
                                sqs2[(mm1, half)][:, hs],
                                start=False, stop=False)
                    nc.tensor.matmul(st2_sum[:, hs], ones[:],
                                     y2_sb[:, 7, hs], start=False, stop=True)
                    nc.tensor.matmul(st2_sq[:, hs], ones[:],
                                     sqs2[(7, half)][:, hs],
                                     start=False, stop=True)
                    rstd_h, ms_h = ln_finish(st2_sum[:, hs], st2_sq[:, hs],
                                             f"ln2h{half}", w=256)
                    for kc in range(KD):
                        ln_apply(y2_sb, rstd_h, ms_h, g2, be2, z2_sb, kc,
                                 cols=hs)
                        nc.sync.dma_start(out8[:, kc, hs], z2_sb[:, kc, hs])
    nc.compile()
    return nc


def _get(name, builder):
    if name not in _CACHE:
        _CACHE[name] = builder()
    return _CACHE[name]


def _prep_inputs(X, Wq, Wk, Wo, ln1_g, ln1_b, ln2_g, ln2_b, W1, b1, W2, b2):
    f = lambda a: np.ascontiguousarray(a)
    Xt = np.asarray(X, np.float32).reshape(N, D).T          # [D, N]
    WqT = np.asarray(Wq, np.float32).T                      # [D, D]
    WkT = np.asarray(Wk, np.float32).T
    WoT = np.asarray(Wo, np.float32).T
    W1T = np.asarray(W1, np.float32).T                      # [D, FF]
    W2T = np.asarray(W2, np.float32).T                      # [FF, D]
    vecP = lambda v, k: np.asarray(v, np.float32).reshape(k, P).T  # [P, k]

    # xt8: [idx, p, kc, q]
    xt8 = f(Xt.reshape(KD, P, NQ, 512).transpose(2, 1, 0, 3))
    id64b = np.tile(np.eye(DH, dtype=np.float32), (2, 1)).astype(BF)
    w_tile = lambda w, c: f(
        w[:, c * P:(c + 1) * P].reshape(KD, P, P).transpose(1, 0, 2))

    in_maps_a = [
        {
            "xt8": xt8,
            "id64b": id64b,
            "wqt8": w_tile(WqT, c),
            "wkt8": w_tile(WkT, c),
            "wvt8": w_tile(WoT, c),   # value projection uses W_o in this model
        }
        for c in range(N_CORES)
    ]

    wo8 = f(WoT.reshape(KD, P, D).transpose(1, 0, 2).astype(BF))
    w18 = f(W1T.reshape(KD, P, KD, 512).transpose(2, 1, 0, 3).astype(BF))
    w2a8 = f(W2T[:, 0:512].reshape(KD, 4, P, 512).transpose(0, 2, 1, 3)
             .astype(BF))
    w2b8 = f(W2T[:, 512:1024].reshape(KD, 4, P, 512).transpose(0, 2, 1, 3)
             .astype(BF))
    consts = f(np.hstack([vecP(ln1_g, KD), vecP(ln1_b, KD), vecP(ln2_g, KD),
                          vecP(ln2_b, KD), vecP(b1, KF), vecP(b2, KD)]))
    id128b = np.eye(P, dtype=BF)

    def in_maps_b(full_nat):
        maps = []
        for c in range(N_CORES):
            blk = full_nat[c * QC:(c + 1) * QC]             # [512, 1024]
            ctin = f(blk.reshape(4, P, KD, P).transpose(1, 2, 0, 3).astype(BF))
            xts = f(Xt[:, c * QC:(c + 1) * QC].reshape(KD, P, 512)
                    .transpose(1, 0, 2))
            maps.append({
                "ctin": ctin, "xts8": xts, "wo8": wo8, "w18": w18,
                "w2a8": w2a8, "w2b8": w2b8, "consts": consts,
                "id128b": id128b,
            })
        return maps

    return in_maps_a, in_maps_b


def kernel(X, Wq, Wk, Wo, ln1_g, ln1_b, ln2_g, ln2_b, W1, b1, W2, b2):
    in_maps_a, in_maps_b = _prep_inputs(
        X, Wq, Wk, Wo, ln1_g, ln1_b, ln2_g, ln2_b, W1, b1, W2, b2)

    nc_a = _get("a", _build_phase_a)
    res_a = run_bass_kernel_spmd(nc_a, in_maps_a, core_ids=list(range(N_CORES)))
    # ctxn [P, NQ, 4, P] per core -> natural [4096, 128] -> concat cols
    full_nat = np.concatenate(
        [res_a.results[c]["ctxn"].transpose(1, 2, 0, 3).reshape(N, P)
         for c in range(N_CORES)], axis=1)                  # [N, D]

    nc_b = _get("b", _build_phase_b)
    res_b = run_bass_kernel_spmd(nc_b, in_maps_b(full_nat),
                                 core_ids=list(range(N_CORES)))
    # out8 [P, KD, 512] per core -> [D, 512] col block of out^T
    out_t = np.concatenate(
        [res_b.results[c]["out8"].astype(np.float32).transpose(1, 0, 2)
         .reshape(D, QC) for c in range(N_CORES)], axis=1)  # [D, N]
    return np.ascontiguousarray(out_t.T).reshape(B, S, D).astype(np.float32)


# revision 38
# speedup vs baseline: 1.1567x; 1.0288x over previous
"""Trainium2 Bass kernel for nn_Encoder (dense transformer block), 8 NeuronCores.

Strategy (single chip, 8 cores), v3:
  Phase A (head-parallel): core c computes attention for heads {2c, 2c+1}.
    Projections run in t-layout; q/k land in bf16, V is PE-transposed into
    natural [keys, dims] bf16 layout. softmax(relu(s)) is p = max(exp(s/8), 1)
    with the softmax denominator taken from a ones column appended to V.
    The exp pass on the Activation engine is the critical resource (~134 us);
    a queue-based emitter keeps it saturated: score matmuls are emitted as
    early as their projections allow (wavefront), projection matmuls are
    spread between them in small pieces, and the context-accumulation chains
    (65-cycle bf16 matmuls in the fast [q,65] orientation) fill the PE's
    exp-paced slack. ctx leaves phase A in natural [token, dim] layout.
  Phase B (row-parallel): core c takes 512 of the 4096 token rows. It
    PE-transposes the incoming ctx back to t-layout fused with the Wo
    matmuls, then AddNorm1, FFN (ReLU, bf16 weights/activations), AddNorm2.
    All weights stream as a handful of large host-pre-tiled bf16 DMAs on the
    SP queue; LayerNorm statistics are accumulated in halves so their matmuls
    and squares overlap the surrounding GEMMs.
"""

import os
import sys

for _p in ("/opt/trn_rl_repo",):
    if _p not in sys.path:
        sys.path.insert(0, _p)

# The Bass SPMD path executes through jax/PJRT on the axon platform; make
# sure a caller-pinned JAX_PLATFORMS=cpu doesn't hide the NeuronCores.
_jp = os.environ.get("JAX_PLATFORMS")
if _jp is not None and "axon" not in _jp:
    os.environ["JAX_PLATFORMS"] = "axon," + _jp

import ml_dtypes
import numpy as np

import concourse.bass as bass
import concourse.mybir as mybir
import concourse.tile as tile
from concourse import bacc
from concourse.bass_utils import run_bass_kernel_spmd

F32 = mybir.dt.float32
F32R = mybir.dt.float32r
BF16 = mybir.dt.bfloat16
AF = mybir.ActivationFunctionType
OP = mybir.AluOpType
BF = ml_dtypes.bfloat16


def _mm(nc, out, lhsT, rhs, **kw):
    # fp32r: 1-pass FP22 matmul (1 cyc/row when the moving dim is >= 256)
    nc.tensor.matmul(out, lhsT.bitcast(F32R), rhs.bitcast(F32R), **kw)


N_CORES = 8
B, S, D, H, DH, FF = 2, 2048, 1024, 16, 64, 4096
N = B * S            # 4096 token rows
P = 128
QC = N // N_CORES    # 512 rows per core in phase B
KD = D // P          # 8 contraction chunks over D
KI = S // P          # 16 key chunks of 128 per batch
NO = S // 512        # 4 query chunks of 512 per batch
NQ = N // 512        # 8 query chunks overall
KF = FF // P         # 32
EPS = 1e-5

_CACHE = {}


# --------------------------------------------------------------------------
# Phase A: per-core head-parallel attention.
# Inputs (per core):
#   xt8  [NQ, P, KD, 512]  X^T tiled per 512-token chunk (replicated)
#   wqt8/wkt8/wvt8 [P, KD, P]  W^T columns for this core's two heads, tiled
#   id64b [P, DH] bf16 tiled identity (V transposes)
# Output:
#   ctxn [P, NQ, 4, P] f32: natural-layout ctx; token = idx*512 + j*128 + p,
#   col = the two heads' 64-dim blocks concatenated.
# --------------------------------------------------------------------------
def _build_phase_a():
    nc = bacc.Bacc("TRN2", target_bir_lowering=False, debug=False,
                   num_devices=N_CORES)
    xt8 = nc.dram_tensor("xt8", [NQ, P, KD, 512], F32R, kind="ExternalInput")
    wqt8 = nc.dram_tensor("wqt8", [P, KD, P], F32R, kind="ExternalInput")
    wkt8 = nc.dram_tensor("wkt8", [P, KD, P], F32R, kind="ExternalInput")
    wvt8 = nc.dram_tensor("wvt8", [P, KD, P], F32R, kind="ExternalInput")
    id64b = nc.dram_tensor("id64b", [P, DH], BF16, kind="ExternalInput")
    ctxn = nc.dram_tensor("ctxn", [P, NQ, 4, P], F32, kind="ExternalOutput")

    chunks = [(b_, o) for b_ in range(B) for o in range(NO)]

    with tile.TileContext(nc) as tc:
        with tc.tile_pool(name="persist", bufs=1) as persist:
            qt_sb = [persist.tile([P, S], BF16, name=f"qt{b_}") for b_ in range(B)]
            kt_sb = [persist.tile([P, S], BF16, name=f"kt{b_}") for b_ in range(B)]
            vt_sb = [persist.tile([P, S], BF16, name=f"vt{b_}") for b_ in range(B)]
            vp_sb = [persist.tile([P, KI, 2, DH + 1], BF16, name=f"vp{b_}")
                     for b_ in range(B)]
            wq_sb = persist.tile([P, KD, P], F32R)
            wk_sb = persist.tile([P, KD, P], F32R)
            wv_sb = persist.tile([P, KD, P], F32R)
            id64_sb = persist.tile([P, DH], BF16)

            for b_ in range(B):
                nc.vector.memset(vp_sb[b_][:, :, 0, DH:DH + 1], 1.0)
                nc.vector.memset(vp_sb[b_][:, :, 1, DH:DH + 1], 1.0)

            with (
                tc.tile_pool(name="xpool", bufs=2) as xpool,
                tc.tile_pool(name="accp", bufs=2, space="PSUM") as accp,
                tc.tile_pool(name="slabp", bufs=50) as slabp,
                tc.tile_pool(name="stagep", bufs=2) as stagep,
                tc.tile_pool(name="smallp", bufs=8) as smallp,
                tc.tile_pool(name="pss", bufs=2, space="PSUM") as pss,
                tc.tile_pool(name="psc", bufs=2, space="PSUM") as psc,
            ):
                xt_tiles = {}

                def issue_xt(ci):
                    t = xpool.tile([P, KD, 512], F32R, name="xt_tile")
                    # two half-DMAs so the first projection matmuls can start
                    # as soon as the front half lands (subtile deps)
                    nc.sync.dma_start(t[:, 0:4], xt8[ci, :, 0:4])
                    nc.sync.dma_start(t[:, 4:8], xt8[ci, :, 4:8])
                    xt_tiles[ci] = t

                def gen_proj_qk(ci):
                    """Generator: project chunk ci into qt/kt (bf16). For the
                    first chunk the q and k chains interleave so both finish
                    (and the first scores emit) as early as possible."""
                    b_, o = chunks[ci]
                    osl = slice(o * 512, (o + 1) * 512)
                    xt_tile = xt_tiles[ci]
                    if ci == 0:
                        accq = accp.tile([P, 512], F32, name="acc_ps",
                                         tag="acc")
                        acck = accp.tile([P, 512], F32, name="acc_ps",
                                         tag="acc")
                        for kc in range(KD):
                            _mm(nc, accq[:], wq_sb[:, kc], xt_tile[:, kc],
                                start=(kc == 0), stop=(kc == KD - 1))
                            _mm(nc, acck[:], wk_sb[:, kc], xt_tile[:, kc],
                                start=(kc == 0), stop=(kc == KD - 1))
                            if kc % 2 == 1:
                                yield
                        nc.vector.tensor_copy(qt_sb[b_][:, osl], accq[:])
                        nc.vector.tensor_copy(kt_sb[b_][:, osl], acck[:])
                        yield
                        return
                    for w_sb, dst in ((wq_sb, qt_sb[b_]), (wk_sb, kt_sb[b_])):
                        acc = accp.tile([P, 512], F32, name="acc_ps", tag="acc")
                        for kc in range(KD):
                            _mm(nc, acc[:], w_sb[:, kc], xt_tile[:, kc],
                                start=(kc == 0), stop=(kc == KD - 1))
                            if kc % 2 == 1:
                                yield
                        nc.vector.tensor_copy(dst[:, osl], acc[:])
                        yield

                def gen_proj_v(ci):
                    """Generator: V projection + natural-layout transposes."""
                    b_, o = chunks[ci]
                    osl = slice(o * 512, (o + 1) * 512)
                    xt_tile = xt_tiles.pop(ci)
                    acc = accp.tile([P, 512], F32, name="acc_ps", tag="acc")
                    for kc in range(KD):
                        _mm(nc, acc[:], wv_sb[:, kc], xt_tile[:, kc],
                            start=(kc == 0), stop=(kc == KD - 1))
                        if kc % 2 == 1:
                            yield
                    nc.vector.tensor_copy(vt_sb[b_][:, osl], acc[:])
                    yield
                    for t in range(4):
                        kc2 = o * 4 + t
                        for hh in range(2):
                            tp = accp.tile([P, DH], BF16, name="tp_ps",
                                           tag="acc")
                            nc.tensor.transpose(
                                tp[:, 0:DH],
                                vt_sb[b_][hh * DH:(hh + 1) * DH,
                                          kc2 * P:(kc2 + 1) * P],
                                id64_sb[hh * DH:(hh + 1) * DH, :])
                            nc.vector.tensor_copy(
                                vp_sb[b_][:, kc2, hh, 0:DH], tp[:, 0:DH])
                        yield

                slabs = {i: {} for i in range(NQ)}   # idx -> kc -> slab tile
                stages = {}

                def emit_scores(idx, kc):
                    b_, o = chunks[idx]
                    qs = slice(o * 512, (o + 1) * 512)
                    ks = slice(kc * P, (kc + 1) * P)
                    s_ps = pss.tile([P, 1024], F32, name="s_ps")
                    nc.tensor.matmul(s_ps[:, 0:512], kt_sb[b_][0:DH, ks],
                                     qt_sb[b_][0:DH, qs], start=True, stop=True)
                    nc.tensor.matmul(s_ps[:, 512:1024], kt_sb[b_][DH:2 * DH, ks],
                                     qt_sb[b_][DH:2 * DH, qs],
                                     start=True, stop=True)
                    slab = slabp.tile([P, 1024], BF16, name="slab")
                    nc.scalar.activation(slab[:], s_ps[:], AF.Exp, scale=0.125)
                    nc.vector.tensor_scalar_max(slab[:], slab[:], 1.0)
                    slabs[idx][kc] = slab

                def gen_chains(idx):
                    """Generator: the 8 ctx chains of idx + normalize + DMA,
                    yielding every couple of matmuls."""
                    b_, o = chunks[idx]
                    stage = stagep.tile([P, 4, P], F32, name="stage")
                    for ci in range(8):
                        j, h = ci // 2, ci % 2
                        acc = psc.tile([P, DH + 1], F32, name="ctx_ps")
                        for kc in range(KI):
                            nc.tensor.matmul(
                                acc[:],
                                slabs[idx][kc][:, h * 512 + j * P:
                                               h * 512 + (j + 1) * P],
                                vp_sb[b_][:, kc, h, :],
                                start=(kc == 0), stop=(kc == KI - 1))
                            if kc % 4 == 3:
                                yield
                        inv = smallp.tile([P, 1], F32, name="inv")
                        nc.vector.reciprocal(inv[:], acc[:, DH:DH + 1])
                        nc.vector.tensor_scalar(
                            out=stage[:, j, h * DH:(h + 1) * DH],
                            in0=acc[:, 0:DH], scalar1=inv[:], scalar2=None,
                            op0=OP.mult)
                        yield
                    nc.sync.dma_start(ctxn[:, idx], stage[:])
                    slabs[idx].clear()

                # ---------------- queue-based emitter ----------------
                emitted = set()           # (idx, kc) scores emitted
                score_q = []              # ordered pending scores
                qk_done = [False] * NQ
                v_done = [False] * NQ
                chains_done = 0           # count of fully-emitted chain idxs
                chain_gen = None
                chain_idx = 0             # next idx needing chains
                qk_idx = 0                # next chunk for q/k projection
                v_idx = 0                 # next chunk for v projection
                qkgen = None
                vgen = None

                def update_score_q():
                    for i in range(NQ):
                        bi, _ = chunks[i]
                        if not qk_done[i]:
                            continue
                        if i >= chains_done + 3:
                            continue
                        base = 4 * bi
                        kmax = sum(4 for c in range(base, base + NO)
                                   if qk_done[c])
                        for k in range(kmax):
                            if (i, k) not in emitted and (i, k) not in score_q:
                                score_q.append((i, k))

                # DMA order tuned so the first q-projection matmuls can
                # start at ~4.5us: wq, then the first xt half, then wk etc.
                nc.sync.dma_start(wq_sb[:], wqt8.ap())
                issue_xt(0)
                nc.sync.dma_start(wk_sb[:], wkt8.ap())
                nc.sync.dma_start(wv_sb[:], wvt8.ap())
                issue_xt(1)
                nc.sync.dma_start(id64_sb[:], id64b.ap())
                while (qk_idx < NQ or v_idx < NQ or score_q
                       or chain_idx < NQ or chain_gen is not None):
                    # 1. a slice of chain work (PE filler, no Act dependency)
                    if chain_gen is None and chain_idx < NQ:
                        bci, _ = chunks[chain_idx]
                        if (len(slabs[chain_idx]) == KI
                                and all(v_done[c] for c in
                                        range(4 * bci, 4 * bci + NO))):
                            chain_gen = gen_chains(chain_idx)
                    if chain_gen is not None:
                        for _ in range(3 if score_q else 8):
                            try:
                                next(chain_gen)
                            except StopIteration:
                                chain_gen = None
                                chain_idx += 1
                                chains_done += 1
                                update_score_q()
                                break
                    # 2. q/k projection pieces (gate scores)
                    if qkgen is None and qk_idx < NQ and qk_idx <= v_idx:
                        qkgen = gen_proj_qk(qk_idx)
                    if qkgen is not None:
                        steps = 1 if score_q else 4
                        for _ in range(steps):
                            try:
                                next(qkgen)
                            except StopIteration:
                                qk_done[qk_idx] = True
                                qk_idx += 1
                                if qk_idx + 1 < NQ:
                                    issue_xt(qk_idx + 1)
                                qkgen = None
                                update_score_q()
                                break
                    # 3. v projection + transposes (gate chains only)
                    if vgen is None and v_idx < NQ and v_idx < qk_idx:
                        vgen = gen_proj_v(v_idx)
                    if vgen is not None:
                        # boost only when idle or when chains are starved on v
                        chain_starved = (
                            chain_gen is None and chain_idx < NQ
                            and len(slabs[chain_idx]) == KI)
                        steps = 2 if (chain_starved or not score_q
                                      or v_idx < qk_idx - 1) else 1
                        for _ in range(steps):
                            try:
                                next(vgen)
                            except StopIteration:
                                v_done[v_idx] = True
                                v_idx += 1
                                vgen = None
                                break
                    # 4. one score (the Act engine's food)
                    if score_q:
                        i, k = score_q.pop(0)
                        emit_scores(i, k)
                        emitted.add((i, k))
                        update_score_q()
    nc.compile()
    return nc


# --------------------------------------------------------------------------
# Phase B: per-core row-parallel transpose + Wo-proj + AddNorm1 + FFN + AddNorm2.
# Inputs (per core, qi = this core's 512 token rows):
#   ctin [P, 4, KD, P] bf16   natural-layout ctx blocks for these rows
#   wo8  [P, KD, D]    bf16   Wo^T tiled
#   w18  [KD, P, KD, 512] bf16  W1^T tiled per 512-wide ffn-col group
#   w2a8/w2b8 [KD, P, 4, 512] bf16  W2^T tiled, first/second output half
#   xts8 [P, KD, 512] f32     X^T slice (residual 1)
#   consts [P, 72] f32        g1|be1|g2|be2|b1t|b2t feature-on-partition
#   id128b [P, P] bf16
# Output: out8 [P, KD, 512] f32 (t-layout output slice, tiled)
# --------------------------------------------------------------------------
def _build_phase_b():
    nc = bacc.Bacc("TRN2", target_bir_lowering=False, debug=False,
                   num_devices=N_CORES)
    ctin = nc.dram_tensor("ctin", [P, KD, 4, P], BF16, kind="ExternalInput")
    wo8 = nc.dram_tensor("wo8", [P, KD, D], BF16, kind="ExternalInput")
    w18 = nc.dram_tensor("w18", [KD, P, KD, 512], BF16, kind="ExternalInput")
    w2a8 = nc.dram_tensor("w2a8", [KD, P, 4, 512], BF16, kind="ExternalInput")
    w2b8 = nc.dram_tensor("w2b8", [KD, P, 4, 512], BF16, kind="ExternalInput")
    xts8 = nc.dram_tensor("xts8", [P, KD, 512], F32, kind="ExternalInput")
    consts = nc.dram_tensor("consts", [P, 72], F32, kind="ExternalInput")
    id128b = nc.dram_tensor("id128b", [P, P], BF16, kind="ExternalInput")
    out8 = nc.dram_tensor("out8", [P, KD, 512], BF16, kind="ExternalOutput")

    with tile.TileContext(nc) as tc:
        with (
            tc.tile_pool(name="persist", bufs=1) as persist,
            tc.tile_pool(name="w1p", bufs=3) as w1p,
            tc.tile_pool(name="w2p", bufs=3) as w2p,
            tc.tile_pool(name="sqp", bufs=3) as sqp,
            tc.tile_pool(name="smallp", bufs=2) as smallp,
            tc.tile_pool(name="bcp", bufs=2) as bcp,
        ):
            ctin_sb = persist.tile([P, KD, 4, P], BF16)
            ct_sb = persist.tile([P, KD, 4, P], BF16)
            wo_sb = persist.tile([P, KD, D], BF16)
            xts_sb = persist.tile([P, KD, 512], F32)
            y1_sb = persist.tile([P, KD, 512], BF16)
            z1_sb = persist.tile([P, KD, 512], BF16)
            h_sb = persist.tile([P, KF, 512], BF16)
            w2b_sb = persist.tile([P, KD, 4, 512], BF16)
            y2_sb = persist.tile([P, KD, 512], BF16, tag="y1_sb")
            z2_sb = persist.tile([P, KD, 512], BF16, tag="xts_sb")
            consts_sb = persist.tile([P, 72], F32)
            id128_sb = persist.tile([P, P], BF16)
            ones = persist.tile([P, 1], BF16)

            nc.sync.dma_start(ctin_sb[:, 0:2], ctin[:, 0:2])
            nc.sync.dma_start(id128_sb[:], id128b.ap())
            nc.sync.dma_start(wo_sb[:, 0:2], wo8[:, 0:2])
            nc.sync.dma_start(ctin_sb[:, 2:8], ctin[:, 2:8])
            nc.sync.dma_start(wo_sb[:, 2:8], wo8[:, 2:8])
            nc.sync.dma_start(consts_sb[:], consts.ap())
            nc.sync.dma_start(xts_sb[:], xts8.ap())
            nc.vector.memset(ones[:], 1.0)

            g1 = [consts_sb[:, kc:kc + 1] for kc in range(KD)]
            be1 = [consts_sb[:, 8 + kc:9 + kc] for kc in range(KD)]
            g2 = [consts_sb[:, 16 + kc:17 + kc] for kc in range(KD)]
            be2 = [consts_sb[:, 24 + kc:25 + kc] for kc in range(KD)]
            b1c = [consts_sb[:, 32 + fm:33 + fm] for fm in range(KF)]
            b2c = [consts_sb[:, 64 + kc:65 + kc] for kc in range(KD)]

            def ln_finish(st_sum, st_sq, tag, w=512):
                """Stats (over w tokens) -> (rstd_b, ms_b) broadcast tiles."""
                mean = smallp.tile([1, w], F32, name="mean")
                ex2 = smallp.tile([1, w], F32, name="ex2")
                nc.vector.tensor_scalar(out=mean[:], in0=st_sum,
                                        scalar1=1.0 / D, scalar2=None,
                                        op0=OP.mult)
                nc.vector.tensor_scalar(out=ex2[:], in0=st_sq,
                                        scalar1=1.0 / D, scalar2=None,
                                        op0=OP.mult)
                msq = smallp.tile([1, w], F32, name="msq")
                nc.vector.tensor_mul(msq[:], mean[:], mean[:])
                var = smallp.tile([1, w], F32, name="var")
                nc.vector.tensor_sub(var[:], ex2[:], msq[:])
                nc.vector.tensor_scalar_add(var[:], var[:], EPS)
                std = smallp.tile([1, w], F32, name="std")
                nc.scalar.activation(std[:], var[:], AF.Sqrt)
                rstd = smallp.tile([1, w], BF16, name="rstd")
                ms = smallp.tile([1, w], BF16, name="ms")
                with nc.allow_low_precision(reason="bf16 LN scale factors"):
                    nc.vector.reciprocal(rstd[:], std[:])
                nc.vector.tensor_mul(ms[:], mean[:], rstd[:])
                rstd_b = bcp.tile([P, w], BF16, name="rstd_b", bufs=3)
                ms_b = bcp.tile([P, w], BF16, name="ms_b", bufs=3)
                nc.gpsimd.partition_broadcast(rstd_b[:], rstd[:])
                nc.gpsimd.partition_broadcast(ms_b[:], ms[:])
                return rstd_b, ms_b

            def ln_apply(y_sb, rstd_b, ms_b, g_c, be_c, z_sb, kc, cols=None):
                cols = cols or slice(0, 512)
                w = cols.stop - cols.start
                t = sqp.tile([P, 512], BF16, name="t_ln")
                nc.vector.tensor_mul(t[:, 0:w], y_sb[:, kc, cols], rstd_b[:])
                nc.vector.tensor_sub(t[:, 0:w], t[:, 0:w], ms_b[:])
                nc.vector.tensor_scalar(out=z_sb[:, kc, cols], in0=t[:, 0:w],
                                        scalar1=g_c[kc], scalar2=be_c[kc],
                                        op0=OP.mult, op1=OP.add)

            # ---- B0+B1: transpose ctx to t-layout, fused with Wo matmuls ----
            with (
                tc.tile_pool(name="tpp", bufs=2, space="PSUM") as tpp,
                tc.tile_pool(name="psa", bufs=1, space="PSUM") as psa,
                tc.tile_pool(name="psst1", bufs=1, space="PSUM") as psst1,
            ):
                st1_sum = psst1.tile([1, 512], F32, name="st1_sum")
                st1_sq = psst1.tile([1, 512], F32, name="st1_sq")
                # chain-major Wo: each output chain stops early so its
                # residual add + square overlap the following chains
                a_ps = [psa.tile([P, 512], F32, name=f"mm_ps{i}")
                        for i in range(4)]
                sqs = {}

                def y1_add_sq(m, ps):
                    nc.vector.tensor_add(y1_sb[:, m], ps[:], xts_sb[:, m])
                    sq = sqp.tile([P, 512], BF16, name="sq", bufs=14)
                    nc.vector.tensor_mul(sq[:], y1_sb[:, m], y1_sb[:, m])
                    sqs[m] = sq

                for kc in range(KD):
                    tp = tpp.tile([P, 4, P], BF16, name="tp_ps")
                    for jb in range(4):
                        nc.tensor.transpose(tp[:, jb, :],
                                            ctin_sb[:, kc, jb, :],
                                            id128_sb[:])
                    nc.vector.tensor_copy(ct_sb[:, kc], tp[:])
                    nc.tensor.matmul(a_ps[0][:], wo_sb[:, kc, 0:P],
                                     ct_sb[:, kc],
                                     start=(kc == 0), stop=(kc == KD - 1))
                y1_add_sq(0, a_ps[0])
                for i in range(1, 4):
                    for kc in range(KD):
                        nc.tensor.matmul(
                            a_ps[i][:], wo_sb[:, kc, i * P:(i + 1) * P],
                            ct_sb[:, kc], start=(kc == 0), stop=(kc == KD - 1))
                    y1_add_sq(i, a_ps[i])
                # mg1 token-halved: half 0's LN1 stats/broadcast hide under
                # half 1's chains, and FFN1's first group (also token-halved,
                # below) starts right after the half-0 applies
                a_ps2 = [psa.tile([P, 512], F32, name=f"mm_ps{i}")
                         for i in range(4)]
                ln1_bh = {}
                for half in range(2):
                    hs = slice(half * 256, (half + 1) * 256)
                    jbs = slice(half * 2, half * 2 + 2)
                    for i in range(4):
                        m = 4 + i
                        for kc in range(KD):
                            nc.tensor.matmul(
                                a_ps2[i][:, hs],
                                wo_sb[:, kc, 512 + i * P:512 + (i + 1) * P],
                                ct_sb[:, kc, jbs, :],
                                start=(kc == 0), stop=(kc == KD - 1))
                        nc.vector.tensor_add(y1_sb[:, m, hs],
                                             a_ps2[i][:, hs],
                                             xts_sb[:, m, hs])
                        sq = sqp.tile([P, 512], BF16, name="sq", bufs=14)
                        nc.vector.tensor_mul(sq[:, hs], y1_sb[:, m, hs],
                                             y1_sb[:, m, hs])
                        sqs[(m, half)] = sq
                    for kc in range(KD):
                        nc.tensor.matmul(st1_sum[:, hs], ones[:],
                                         y1_sb[:, kc, hs],
                                         start=(kc == 0), stop=(kc == KD - 1))
                    for kc in range(KD):
                        s = sqs[kc] if kc < 4 else sqs[(kc, half)]
                        nc.tensor.matmul(st1_sq[:, hs], ones[:], s[:, hs],
                                         start=(kc == 0), stop=(kc == KD - 1))
                    ln1_bh[half] = ln_finish(st1_sum[:, hs], st1_sq[:, hs],
                                             f"ln1h{half}", w=256)
                sqs.clear()
                for half in range(2):
                    hs = slice(half * 256, (half + 1) * 256)
                    rstd_h, ms_h = ln1_bh[half]
                    for kc in range(KD):
                        ln_apply(y1_sb, rstd_h, ms_h, g1, be1, z1_sb, kc,
                                 cols=hs)

            # ---- FFN1 + FFN2 (first output half interleaved) ----
            with tc.tile_pool(name="psa2", bufs=1, space="PSUM") as psa2:
                f_ps = [psa2.tile([P, 512], F32, name=f"f_ps{i}")
                        for i in range(4)]
                for fg in range(KD):
                    w1_tile = w1p.tile([P, KD, 512], BF16, name="w1_tile")
                    nc.sync.dma_start(w1_tile[:], w18[fg])
                    h_ps = [psa2.tile([P, 512], F32, name=f"h_ps{i}")
                            for i in range(4)]
                    if fg == 0:
                        # token-halved so the half-0 chains start as soon as
                        # the half-0 LN1 applies land
                        for half in range(2):
                            hs = slice(half * 256, (half + 1) * 256)
                            for kc in range(KD):
                                for i in range(4):
                                    nc.tensor.matmul(
                                        h_ps[i][:, hs],
                                        w1_tile[:, kc, i * P:(i + 1) * P],
                                        z1_sb[:, kc, hs],
                                        start=(kc == 0), stop=(kc == KD - 1))
                    else:
                        for kc in range(KD):
                            for i in range(4):
                                nc.tensor.matmul(
                                    h_ps[i][:], w1_tile[:, kc, i * P:(i + 1) * P],
                                    z1_sb[:, kc], start=(kc == 0),
                                    stop=(kc == KD - 1))
                    for i in range(4):
                        fm = fg * 4 + i
                        nc.scalar.activation(h_sb[:, fm], h_ps[i][:], AF.Relu,
                                             bias=b1c[fm])
                    w2_tile = w2p.tile([P, 4, 512], BF16, name="w2_tile")
                    nc.sync.dma_start(w2_tile[:], w2a8[fg])
                    nc.sync.dma_start(w2b_sb[:, fg], w2b8[fg])
                    for i in range(4):
                        fk = fg * 4 + i
                        for j in range(4):
                            nc.tensor.matmul(
                                f_ps[j][:], w2_tile[:, i, j * P:(j + 1) * P],
                                h_sb[:, fk], start=(fk == 0), stop=(fk == KF - 1))
                # y2 first half + squares; LN2 stats for it run behind
                # FFN2b's first chain
                st2_sum = psa2.tile([1, 512], F32, name="h_ps0", tag="h_ps0")
                st2_sq = psa2.tile([1, 512], F32, name="h_ps1", tag="h_ps1")
                sqs2 = {}

                def y2_add_sq(m, ps):
                    nc.vector.scalar_tensor_tensor(
                        out=y2_sb[:, m], in0=ps[:], scalar=b2c[m],
                        in1=z1_sb[:, m], op0=OP.add, op1=OP.add)
                    sq = sqp.tile([P, 512], BF16, name="sq", bufs=14)
                    nc.vector.tensor_mul(sq[:], y2_sb[:, m], y2_sb[:, m])
                    sqs2[m] = sq

                for j in range(4):
                    y2_add_sq(j, f_ps[j])

                # token-halved FFN2 second half: half 0's chains, stats,
                # and LN2 applies complete while half 1's chains run, so only
                # half 1's (narrower) LN2 epilogue remains on the tail
                f_ps2 = [psa2.tile([P, 512], F32, name=f"f_ps{i}")
                         for i in range(4)]
                for half in range(2):
                    hs = slice(half * 256, (half + 1) * 256)
                    for j in range(4):
                        m = 4 + j
                        for fg in range(KD):
                            for i in range(4):
                                nc.tensor.matmul(
                                    f_ps2[j][:, hs],
                                    w2b_sb[:, fg, i, j * P:(j + 1) * P],
                                    h_sb[:, fg * 4 + i, hs],
                                    start=(fg == 0 and i == 0),
                                    stop=(fg == KD - 1 and i == 3))
                        nc.vector.scalar_tensor_tensor(
                            out=y2_sb[:, m, hs], in0=f_ps2[j][:, hs],
                            scalar=b2c[m], in1=z1_sb[:, m, hs],
                            op0=OP.add, op1=OP.add)
                        sq = sqp.tile([P, 512], BF16, name="sq", bufs=14)
                        nc.vector.tensor_mul(sq[:, hs], y2_sb[:, m, hs],
                                             y2_sb[:, m, hs])
                        sqs2[(m, half)] = sq
                        if j == 0:
                            for kc in range(4):
                                nc.tensor.matmul(
                                    st2_sum[:, hs], ones[:],
                                    y2_sb[:, kc, hs],
                                    start=(kc == 0), stop=False)
                                nc.tensor.matmul(
                                    st2_sq[:, hs], ones[:],
                                    sqs2[kc][:, hs],
                                    start=(kc == 0), stop=False)
                        else:
                            mm1 = 4 + j - 1
                            nc.tensor.matmul(
                                st2_sum[:, hs], ones[:], y2_sb[:, mm1, hs],
                                start=False, stop=False)
                            nc.tensor.matmul(
                                st2_sq[:, hs], ones[:],
                                sqs2[(mm1, half)][:, hs],
                                start=False, stop=False)
                    nc.tensor.matmul(st2_sum[:, hs], ones[:],
                                     y2_sb[:, 7, hs], start=False, stop=True)
                    nc.tensor.matmul(st2_sq[:, hs], ones[:],
                                     sqs2[(7, half)][:, hs],
                                     start=False, stop=True)
                    rstd_h, ms_h = ln_finish(st2_sum[:, hs], st2_sq[:, hs],
                                             f"ln2h{half}", w=256)
                    for kc in range(KD):
                        ln_apply(y2_sb, rstd_h, ms_h, g2, be2, z2_sb, kc,
                                 cols=hs)
                        nc.sync.dma_start(out8[:, kc, hs], z2_sb[:, kc, hs])
    nc.compile()
    return nc


def _get(name, builder):
    if name not in _CACHE:
        _CACHE[name] = builder()
    return _CACHE[name]


def _prep_inputs(X, Wq, Wk, Wo, ln1_g, ln1_b, ln2_g, ln2_b, W1, b1, W2, b2):
    f = lambda a: np.ascontiguousarray(a)
    Xt = np.asarray(X, np.float32).reshape(N, D).T          # [D, N]
    WqT = np.asarray(Wq, np.float32).T                      # [D, D]
    WkT = np.asarray(Wk, np.float32).T
    WoT = np.asarray(Wo, np.float32).T
    W1T = np.asarray(W1, np.float32).T                      # [D, FF]
    W2T = np.asarray(W2, np.float32).T                      # [FF, D]
    vecP = lambda v, k: np.asarray(v, np.float32).reshape(k, P).T  # [P, k]

    # xt8: [idx, p, kc, q]
    xt8 = f(Xt.reshape(KD, P, NQ, 512).transpose(2, 1, 0, 3))
    id64b = np.tile(np.eye(DH, dtype=np.float32), (2, 1)).astype(BF)
    w_tile = lambda w, c: f(
        w[:, c * P:(c + 1) * P].reshape(KD, P, P).transpose(1, 0, 2))

    in_maps_a = [
        {
            "xt8": xt8,
            "id64b": id64b,
            "wqt8": w_tile(WqT, c),
            "wkt8": w_tile(WkT, c),
            "wvt8": w_tile(WoT, c),   # value projection uses W_o in this model
        }
        for c in range(N_CORES)
    ]

    wo8 = f(WoT.reshape(KD, P, D).transpose(1, 0, 2).astype(BF))
    w18 = f(W1T.reshape(KD, P, KD, 512).transpose(2, 1, 0, 3).astype(BF))
    w2a8 = f(W2T[:, 0:512].reshape(KD, 4, P, 512).transpose(0, 2, 1, 3)
             .astype(BF))
    w2b8 = f(W2T[:, 512:1024].reshape(KD, 4, P, 512).transpose(0, 2, 1, 3)
             .astype(BF))
    consts = f(np.hstack([vecP(ln1_g, KD), vecP(ln1_b, KD), vecP(ln2_g, KD),
                          vecP(ln2_b, KD), vecP(b1, KF), vecP(b2, KD)]))
    id128b = np.eye(P, dtype=BF)

    def in_maps_b(full_nat):
        maps = []
        for c in range(N_CORES):
            blk = full_nat[c * QC:(c + 1) * QC]             # [512, 1024]
            ctin = f(blk.reshape(4, P, KD, P).transpose(1, 2, 0, 3).astype(BF))
            xts = f(Xt[:, c * QC:(c + 1) * QC].reshape(KD, P, 512)
                    .transpose(1, 0, 2))
            maps.append({
                "ctin": ctin, "xts8": xts, "wo8": wo8, "w18": w18,
                "w2a8": w2a8, "w2b8": w2b8, "consts": consts,
                "id128b": id128b,
            })
        return maps

    return in_maps_a, in_maps_b


def kernel(X, Wq, Wk, Wo, ln1_g, ln1_b, ln2_g, ln2_b, W1, b1, W2, b2):
    in_maps_a, in_maps_b = _prep_inputs(
        X, Wq, Wk, Wo, ln1_g, ln1_b, ln2_g, ln2_b, W1, b1, W2, b2)

    nc_a = _get("a", _build_phase_a)
    res_a = run_bass_kernel_spmd(nc_a, in_maps_a, core_ids=list(range(N_CORES)))
    # ctxn [P, NQ, 4, P] per core -> natural [4096, 128] -> concat cols
    full_nat = np.concatenate(
        [res_a.results[c]["ctxn"].transpose(1, 2, 0, 3).reshape(N, P)
         for c in range(N_CORES)], axis=1)                  # [N, D]

    nc_b = _get("b", _build_phase_b)
    res_b = run_bass_kernel_spmd(nc_b, in_maps_b(full_nat),
                                 core_ids=list(range(N_CORES)))
    # out8 [P, KD, 512] per core -> [D, 512] col block of out^T
    out_t = np.concatenate(
        [res_b.results[c]["out8"].astype(np.float32).transpose(1, 0, 2)
         .reshape(D, QC) for c in range(N_CORES)], axis=1)  # [D, N]
    return np.ascontiguousarray(out_t.T).reshape(B, S, D).astype(np.float32)


# revision 39
# speedup vs baseline: 1.1596x; 1.0025x over previous
"""Trainium2 Bass kernel for nn_Encoder (dense transformer block), 8 NeuronCores.

Strategy (single chip, 8 cores), v3:
  Phase A (head-parallel): core c computes attention for heads {2c, 2c+1}.
    Projections run in t-layout; q/k land in bf16, V is PE-transposed into
    natural [keys, dims] bf16 layout. softmax(relu(s)) is p = max(exp(s/8), 1)
    with the softmax denominator taken from a ones column appended to V.
    The exp pass on the Activation engine is the critical resource (~134 us);
    a queue-based emitter keeps it saturated: score matmuls are emitted as
    early as their projections allow (wavefront), projection matmuls are
    spread between them in small pieces, and the context-accumulation chains
    (65-cycle bf16 matmuls in the fast [q,65] orientation) fill the PE's
    exp-paced slack. ctx leaves phase A in natural [token, dim] layout.
  Phase B (row-parallel): core c takes 512 of the 4096 token rows. It
    PE-transposes the incoming ctx back to t-layout fused with the Wo
    matmuls, then AddNorm1, FFN (ReLU, bf16 weights/activations), AddNorm2.
    All weights stream as a handful of large host-pre-tiled bf16 DMAs on the
    SP queue; LayerNorm statistics are accumulated in halves so their matmuls
    and squares overlap the surrounding GEMMs.
"""

import os
import sys

for _p in ("/opt/trn_rl_repo",):
    if _p not in sys.path:
        sys.path.insert(0, _p)

# The Bass SPMD path executes through jax/PJRT on the axon platform; make
# sure a caller-pinned JAX_PLATFORMS=cpu doesn't hide the NeuronCores.
_jp = os.environ.get("JAX_PLATFORMS")
if _jp is not None and "axon" not in _jp:
    os.environ["JAX_PLATFORMS"] = "axon," + _jp

import ml_dtypes
import numpy as np

import concourse.bass as bass
import concourse.mybir as mybir
import concourse.tile as tile
from concourse import bacc
from concourse.bass_utils import run_bass_kernel_spmd

F32 = mybir.dt.float32
F32R = mybir.dt.float32r
BF16 = mybir.dt.bfloat16
AF = mybir.ActivationFunctionType
OP = mybir.AluOpType
BF = ml_dtypes.bfloat16


def _mm(nc, out, lhsT, rhs, **kw):
    # fp32r: 1-pass FP22 matmul (1 cyc/row when the moving dim is >= 256)
    nc.tensor.matmul(out, lhsT.bitcast(F32R), rhs.bitcast(F32R), **kw)


N_CORES = 8
B, S, D, H, DH, FF = 2, 2048, 1024, 16, 64, 4096
N = B * S            # 4096 token rows
P = 128
QC = N // N_CORES    # 512 rows per core in phase B
KD = D // P          # 8 contraction chunks over D
KI = S // P          # 16 key chunks of 128 per batch
NO = S // 512        # 4 query chunks of 512 per batch
NQ = N // 512        # 8 query chunks overall
KF = FF // P         # 32
EPS = 1e-5

_CACHE = {}


# --------------------------------------------------------------------------
# Phase A: per-core head-parallel attention.
# Inputs (per core):
#   xt8  [NQ, P, KD, 512]  X^T tiled per 512-token chunk (replicated)
#   wqt8/wkt8/wvt8 [P, KD, P]  W^T columns for this core's two heads, tiled
#   id64b [P, DH] bf16 tiled identity (V transposes)
# Output:
#   ctxn [P, NQ, 4, P] f32: natural-layout ctx; token = idx*512 + j*128 + p,
#   col = the two heads' 64-dim blocks concatenated.
# --------------------------------------------------------------------------
def _build_phase_a():
    nc = bacc.Bacc("TRN2", target_bir_lowering=False, debug=False,
                   num_devices=N_CORES)
    xt8 = nc.dram_tensor("xt8", [NQ, P, KD, 512], F32R, kind="ExternalInput")
    wqt8 = nc.dram_tensor("wqt8", [P, KD, P], F32R, kind="ExternalInput")
    wkt8 = nc.dram_tensor("wkt8", [P, KD, P], F32R, kind="ExternalInput")
    wvt8 = nc.dram_tensor("wvt8", [P, KD, P], F32R, kind="ExternalInput")
    id64b = nc.dram_tensor("id64b", [P, DH], BF16, kind="ExternalInput")
    ctxn = nc.dram_tensor("ctxn", [P, NQ, 4, P], F32, kind="ExternalOutput")

    chunks = [(b_, o) for b_ in range(B) for o in range(NO)]

    with tile.TileContext(nc) as tc:
        with tc.tile_pool(name="persist", bufs=1) as persist:
            qt_sb = [persist.tile([P, S], BF16, name=f"qt{b_}") for b_ in range(B)]
            kt_sb = [persist.tile([P, S], BF16, name=f"kt{b_}") for b_ in range(B)]
            vt_sb = [persist.tile([P, S], BF16, name=f"vt{b_}") for b_ in range(B)]
            vp_sb = [persist.tile([P, KI, 2, DH + 1], BF16, name=f"vp{b_}")
                     for b_ in range(B)]
            wq_sb = persist.tile([P, KD, P], F32R)
            wk_sb = persist.tile([P, KD, P], F32R)
            wv_sb = persist.tile([P, KD, P], F32R)
            id64_sb = persist.tile([P, DH], BF16)

            for b_ in range(B):
                nc.vector.memset(vp_sb[b_][:, :, 0, DH:DH + 1], 1.0)
                nc.vector.memset(vp_sb[b_][:, :, 1, DH:DH + 1], 1.0)

            with (
                tc.tile_pool(name="xpool", bufs=2) as xpool,
                tc.tile_pool(name="accp", bufs=2, space="PSUM") as accp,
                tc.tile_pool(name="slabp", bufs=50) as slabp,
                tc.tile_pool(name="stagep", bufs=2) as stagep,
                tc.tile_pool(name="smallp", bufs=8) as smallp,
                tc.tile_pool(name="pss", bufs=2, space="PSUM") as pss,
                tc.tile_pool(name="psc", bufs=2, space="PSUM") as psc,
            ):
                xt_tiles = {}

                def issue_xt(ci):
                    t = xpool.tile([P, KD, 512], F32R, name="xt_tile")
                    # two half-DMAs so the first projection matmuls can start
                    # as soon as the front half lands (subtile deps)
                    nc.sync.dma_start(t[:, 0:4], xt8[ci, :, 0:4])
                    nc.sync.dma_start(t[:, 4:8], xt8[ci, :, 4:8])
                    xt_tiles[ci] = t

                def gen_proj_qk(ci):
                    """Generator: project chunk ci into qt/kt (bf16). For the
                    first chunk the q and k chains interleave so both finish
                    (and the first scores emit) as early as possible."""
                    b_, o = chunks[ci]
                    osl = slice(o * 512, (o + 1) * 512)
                    xt_tile = xt_tiles[ci]
                    if ci == 0:
                        accq = accp.tile([P, 512], F32, name="acc_ps",
                                         tag="acc")
                        acck = accp.tile([P, 512], F32, name="acc_ps",
                                         tag="acc")
                        for kc in range(KD):
                            _mm(nc, accq[:], wq_sb[:, kc], xt_tile[:, kc],
                                start=(kc == 0), stop=(kc == KD - 1))
                            _mm(nc, acck[:], wk_sb[:, kc], xt_tile[:, kc],
                                start=(kc == 0), stop=(kc == KD - 1))
                            if kc % 2 == 1:
                                yield
                        nc.vector.tensor_copy(qt_sb[b_][:, osl], accq[:])
                        nc.vector.tensor_copy(kt_sb[b_][:, osl], acck[:])
                        yield
                        return
                    for w_sb, dst in ((wq_sb, qt_sb[b_]), (wk_sb, kt_sb[b_])):
                        acc = accp.tile([P, 512], F32, name="acc_ps", tag="acc")
                        for kc in range(KD):
                            _mm(nc, acc[:], w_sb[:, kc], xt_tile[:, kc],
                                start=(kc == 0), stop=(kc == KD - 1))
                            if kc % 2 == 1:
                                yield
                        nc.vector.tensor_copy(dst[:, osl], acc[:])
                        yield

                def gen_proj_v(ci):
                    """Generator: V projection + natural-layout transposes."""
                    b_, o = chunks[ci]
                    osl = slice(o * 512, (o + 1) * 512)
                    xt_tile = xt_tiles.pop(ci)
                    acc = accp.tile([P, 512], F32, name="acc_ps", tag="acc")
                    for kc in range(KD):
                        _mm(nc, acc[:], wv_sb[:, kc], xt_tile[:, kc],
                            start=(kc == 0), stop=(kc == KD - 1))
                        if kc % 2 == 1:
                            yield
                    nc.vector.tensor_copy(vt_sb[b_][:, osl], acc[:])
                    yield
                    for t in range(4):
                        kc2 = o * 4 + t
                        for hh in range(2):
                            tp = accp.tile([P, DH], BF16, name="tp_ps",
                                           tag="acc")
                            nc.tensor.transpose(
                                tp[:, 0:DH],
                                vt_sb[b_][hh * DH:(hh + 1) * DH,
                                          kc2 * P:(kc2 + 1) * P],
                                id64_sb[hh * DH:(hh + 1) * DH, :])
                            nc.vector.tensor_copy(
                                vp_sb[b_][:, kc2, hh, 0:DH], tp[:, 0:DH])
                        yield

                slabs = {i: {} for i in range(NQ)}   # idx -> kc -> slab tile
                stages = {}

                def emit_scores(idx, kc):
                    b_, o = chunks[idx]
                    qs = slice(o * 512, (o + 1) * 512)
                    ks = slice(kc * P, (kc + 1) * P)
                    s_ps = pss.tile([P, 1024], F32, name="s_ps")
                    nc.tensor.matmul(s_ps[:, 0:512], kt_sb[b_][0:DH, ks],
                                     qt_sb[b_][0:DH, qs], start=True, stop=True)
                    nc.tensor.matmul(s_ps[:, 512:1024], kt_sb[b_][DH:2 * DH, ks],
                                     qt_sb[b_][DH:2 * DH, qs],
                                     start=True, stop=True)
                    slab = slabp.tile([P, 1024], BF16, name="slab")
                    nc.scalar.activation(slab[:], s_ps[:], AF.Exp, scale=0.125)
                    nc.vector.tensor_scalar_max(slab[:], slab[:], 1.0)
                    slabs[idx][kc] = slab

                def gen_chains(idx):
                    """Generator: the 8 ctx chains of idx + normalize + DMA,
                    yielding every couple of matmuls."""
                    b_, o = chunks[idx]
                    stage = stagep.tile([P, 4, P], F32, name="stage")
                    for ci in range(8):
                        j, h = ci // 2, ci % 2
                        acc = psc.tile([P, DH + 1], F32, name="ctx_ps")
                        for kc in range(KI):
                            nc.tensor.matmul(
                                acc[:],
                                slabs[idx][kc][:, h * 512 + j * P:
                                               h * 512 + (j + 1) * P],
                                vp_sb[b_][:, kc, h, :],
                                start=(kc == 0), stop=(kc == KI - 1))
                            if kc % 4 == 3:
                                yield
                        inv = smallp.tile([P, 1], F32, name="inv")
                        nc.vector.reciprocal(inv[:], acc[:, DH:DH + 1])
                        nc.vector.tensor_scalar(
                            out=stage[:, j, h * DH:(h + 1) * DH],
                            in0=acc[:, 0:DH], scalar1=inv[:], scalar2=None,
                            op0=OP.mult)
                        yield
                    nc.sync.dma_start(ctxn[:, idx], stage[:])
                    slabs[idx].clear()

                # ---------------- queue-based emitter ----------------
                emitted = set()           # (idx, kc) scores emitted
                score_q = []              # ordered pending scores
                qk_done = [False] * NQ
                v_done = [False] * NQ
                chains_done = 0           # count of fully-emitted chain idxs
                chain_gen = None
                chain_idx = 0             # next idx needing chains
                qk_idx = 0                # next chunk for q/k projection
                v_idx = 0                 # next chunk for v projection
                qkgen = None
                vgen = None

                def update_score_q():
                    for i in range(NQ):
                        bi, _ = chunks[i]
                        if not qk_done[i]:
                            continue
                        if i >= chains_done + 3:
                            continue
                        base = 4 * bi
                        kmax = sum(4 for c in range(base, base + NO)
                                   if qk_done[c])
                        for k in range(kmax):
                            if (i, k) not in emitted and (i, k) not in score_q:
                                score_q.append((i, k))

                # DMA order tuned so the first q-projection matmuls can
                # start at ~4.5us: wq, then the first xt half, then wk etc.
                nc.sync.dma_start(wq_sb[:], wqt8.ap())
                issue_xt(0)
                nc.sync.dma_start(wk_sb[:], wkt8.ap())
                nc.sync.dma_start(wv_sb[:], wvt8.ap())
                issue_xt(1)
                nc.sync.dma_start(id64_sb[:], id64b.ap())
                while (qk_idx < NQ or v_idx < NQ or score_q
                       or chain_idx < NQ or chain_gen is not None):
                    # 1. a slice of chain work (PE filler, no Act dependency)
                    if chain_gen is None and chain_idx < NQ:
                        bci, _ = chunks[chain_idx]
                        if (len(slabs[chain_idx]) == KI
                                and all(v_done[c] for c in
                                        range(4 * bci, 4 * bci + NO))):
                            chain_gen = gen_chains(chain_idx)
                    if chain_gen is not None:
                        for _ in range(3 if score_q else 8):
                            try:
                                next(chain_gen)
                            except StopIteration:
                                chain_gen = None
                                chain_idx += 1
                                chains_done += 1
                                update_score_q()
                                break
                    # 2. q/k projection pieces (gate scores)
                    if qkgen is None and qk_idx < NQ and qk_idx <= v_idx:
                        qkgen = gen_proj_qk(qk_idx)
                    if qkgen is not None:
                        steps = 1 if score_q else 4
                        for _ in range(steps):
                            try:
                                next(qkgen)
                            except StopIteration:
                                qk_done[qk_idx] = True
                                qk_idx += 1
                                if qk_idx + 1 < NQ:
                                    issue_xt(qk_idx + 1)
                                qkgen = None
                                update_score_q()
                                break
                    # 3. v projection + transposes (gate chains only)
                    if vgen is None and v_idx < NQ and v_idx < qk_idx:
                        vgen = gen_proj_v(v_idx)
                    if vgen is not None:
                        # boost only when idle or when chains are starved on v
                        chain_starved = (
                            chain_gen is None and chain_idx < NQ
                            and len(slabs[chain_idx]) == KI)
                        steps = 2 if (chain_starved or not score_q
                                      or v_idx < qk_idx - 1) else 1
                        for _ in range(steps):
                            try:
                                next(vgen)
                            except StopIteration:
                                v_done[v_idx] = True
                                v_idx += 1
                                vgen = None
                                break
                    # 4. one score (the Act engine's food)
                    if score_q:
                        i, k = score_q.pop(0)
                        emit_scores(i, k)
                        emitted.add((i, k))
                        update_score_q()
    nc.compile()
    return nc


# --------------------------------------------------------------------------
# Phase B: per-core row-parallel transpose + Wo-proj + AddNorm1 + FFN + AddNorm2.
# Inputs (per core, qi = this core's 512 token rows):
#   ctin [P, 4, KD, P] bf16   natural-layout ctx blocks for these rows
#   wo8  [P, KD, D]    bf16   Wo^T tiled
#   w18  [KD, P, KD, 512] bf16  W1^T tiled per 512-wide ffn-col group
#   w2a8/w2b8 [KD, P, 4, 512] bf16  W2^T tiled, first/second output half
#   xts8 [P, KD, 512] f32     X^T slice (residual 1)
#   consts [P, 72] f32        g1|be1|g2|be2|b1t|b2t feature-on-partition
#   id128b [P, P] bf16
# Output: out8 [P, KD, 512] f32 (t-layout output slice, tiled)
# --------------------------------------------------------------------------
def _build_phase_b():
    nc = bacc.Bacc("TRN2", target_bir_lowering=False, debug=False,
                   num_devices=N_CORES)
    ctin = nc.dram_tensor("ctin", [P, KD, 4, P], BF16, kind="ExternalInput")
    wo8 = nc.dram_tensor("wo8", [P, KD, D], BF16, kind="ExternalInput")
    w18 = nc.dram_tensor("w18", [KD, P, KD, 512], BF16, kind="ExternalInput")
    w2a8 = nc.dram_tensor("w2a8", [KD, P, 4, 512], BF16, kind="ExternalInput")
    w2b8 = nc.dram_tensor("w2b8", [KD, P, 4, 512], BF16, kind="ExternalInput")
    xts8 = nc.dram_tensor("xts8", [P, KD, 512], F32, kind="ExternalInput")
    consts = nc.dram_tensor("consts", [P, 72], F32, kind="ExternalInput")
    id128b = nc.dram_tensor("id128b", [P, P], BF16, kind="ExternalInput")
    out8 = nc.dram_tensor("out8", [P, KD, 512], BF16, kind="ExternalOutput")

    with tile.TileContext(nc) as tc:
        with (
            tc.tile_pool(name="persist", bufs=1) as persist,
            tc.tile_pool(name="w1p", bufs=3) as w1p,
            tc.tile_pool(name="w2p", bufs=3) as w2p,
            tc.tile_pool(name="sqp", bufs=3) as sqp,
            tc.tile_pool(name="smallp", bufs=2) as smallp,
            tc.tile_pool(name="bcp", bufs=2) as bcp,
        ):
            ctin_sb = persist.tile([P, KD, 4, P], BF16)
            ct_sb = persist.tile([P, KD, 4, P], BF16)
            wo_sb = persist.tile([P, KD, D], BF16)
            xts_sb = persist.tile([P, KD, 512], F32)
            y1_sb = persist.tile([P, KD, 512], BF16)
            z1_sb = persist.tile([P, KD, 512], BF16)
            h_sb = persist.tile([P, KF, 512], BF16)
            w2b_sb = persist.tile([P, KD, 4, 512], BF16)
            y2_sb = persist.tile([P, KD, 512], BF16, tag="y1_sb")
            z2_sb = persist.tile([P, KD, 512], BF16, tag="xts_sb")
            consts_sb = persist.tile([P, 72], F32)
            id128_sb = persist.tile([P, P], BF16)
            ones = persist.tile([P, 1], BF16)

            nc.sync.dma_start(ctin_sb[:, 0:2], ctin[:, 0:2])
            nc.sync.dma_start(id128_sb[:], id128b.ap())
            nc.sync.dma_start(wo_sb[:, 0:2], wo8[:, 0:2])
            nc.sync.dma_start(ctin_sb[:, 2:8], ctin[:, 2:8])
            nc.sync.dma_start(wo_sb[:, 2:8], wo8[:, 2:8])
            nc.sync.dma_start(consts_sb[:], consts.ap())
            nc.sync.dma_start(xts_sb[:], xts8.ap())
            nc.vector.memset(ones[:], 1.0)

            g1 = [consts_sb[:, kc:kc + 1] for kc in range(KD)]
            be1 = [consts_sb[:, 8 + kc:9 + kc] for kc in range(KD)]
            g2 = [consts_sb[:, 16 + kc:17 + kc] for kc in range(KD)]
            be2 = [consts_sb[:, 24 + kc:25 + kc] for kc in range(KD)]
            b1c = [consts_sb[:, 32 + fm:33 + fm] for fm in range(KF)]
            b2c = [consts_sb[:, 64 + kc:65 + kc] for kc in range(KD)]

            def ln_finish(st_sum, st_sq, tag, w=512):
                """Stats (over w tokens) -> (rstd_b, ms_b) broadcast tiles."""
                mean = smallp.tile([1, w], F32, name="mean")
                ex2 = smallp.tile([1, w], F32, name="ex2")
                nc.vector.tensor_scalar(out=mean[:], in0=st_sum,
                                        scalar1=1.0 / D, scalar2=None,
                                        op0=OP.mult)
                nc.vector.tensor_scalar(out=ex2[:], in0=st_sq,
                                        scalar1=1.0 / D, scalar2=None,
                                        op0=OP.mult)
                msq = smallp.tile([1, w], F32, name="msq")
                nc.vector.tensor_mul(msq[:], mean[:], mean[:])
                var = smallp.tile([1, w], F32, name="var")
                nc.vector.tensor_sub(var[:], ex2[:], msq[:])
                nc.vector.tensor_scalar_add(var[:], var[:], EPS)
                std = smallp.tile([1, w], F32, name="std")
                nc.scalar.activation(std[:], var[:], AF.Sqrt)
                rstd = smallp.tile([1, w], BF16, name="rstd")
                ms = smallp.tile([1, w], BF16, name="ms")
                with nc.allow_low_precision(reason="bf16 LN scale factors"):
                    nc.vector.reciprocal(rstd[:], std[:])
                nc.vector.tensor_mul(ms[:], mean[:], rstd[:])
                rstd_b = bcp.tile([P, w], BF16, name="rstd_b", bufs=3)
                ms_b = bcp.tile([P, w], BF16, name="ms_b", bufs=3)
                nc.gpsimd.partition_broadcast(rstd_b[:], rstd[:])
                nc.gpsimd.partition_broadcast(ms_b[:], ms[:])
                return rstd_b, ms_b

            def ln_apply(y_sb, rstd_b, ms_b, g_c, be_c, z_sb, kc, cols=None):
                cols = cols or slice(0, 512)
                w = cols.stop - cols.start
                t = sqp.tile([P, 512], BF16, name="t_ln")
                nc.vector.tensor_mul(t[:, 0:w], y_sb[:, kc, cols], rstd_b[:])
                nc.vector.tensor_sub(t[:, 0:w], t[:, 0:w], ms_b[:])
                nc.vector.tensor_scalar(out=z_sb[:, kc, cols], in0=t[:, 0:w],
                                        scalar1=g_c[kc], scalar2=be_c[kc],
                                        op0=OP.mult, op1=OP.add)

            # ---- B0+B1: transpose ctx to t-layout, fused with Wo matmuls ----
            with (
                tc.tile_pool(name="tpp", bufs=2, space="PSUM") as tpp,
                tc.tile_pool(name="psa", bufs=1, space="PSUM") as psa,
                tc.tile_pool(name="psst1", bufs=1, space="PSUM") as psst1,
            ):
                st1_sum = psst1.tile([1, 512], F32, name="st1_sum")
                st1_sq = psst1.tile([1, 512], F32, name="st1_sq")
                # chain-major Wo: each output chain stops early so its
                # residual add + square overlap the following chains
                a_ps = [psa.tile([P, 512], F32, name=f"mm_ps{i}")
                        for i in range(4)]
                sqs = {}

                def y1_add_sq(m, ps):
                    nc.vector.tensor_add(y1_sb[:, m], ps[:], xts_sb[:, m])
                    sq = sqp.tile([P, 512], BF16, name="sq", bufs=14)
                    nc.vector.tensor_mul(sq[:], y1_sb[:, m], y1_sb[:, m])
                    sqs[m] = sq

                for kc in range(KD):
                    tp = tpp.tile([P, 4, P], BF16, name="tp_ps")
                    for jb in range(4):
                        nc.tensor.transpose(tp[:, jb, :],
                                            ctin_sb[:, kc, jb, :],
                                            id128_sb[:])
                    nc.vector.tensor_copy(ct_sb[:, kc], tp[:])
                    nc.tensor.matmul(a_ps[0][:], wo_sb[:, kc, 0:P],
                                     ct_sb[:, kc],
                                     start=(kc == 0), stop=(kc == KD - 1))
                y1_add_sq(0, a_ps[0])
                for i in range(1, 4):
                    for kc in range(KD):
                        nc.tensor.matmul(
                            a_ps[i][:], wo_sb[:, kc, i * P:(i + 1) * P],
                            ct_sb[:, kc], start=(kc == 0), stop=(kc == KD - 1))
                    y1_add_sq(i, a_ps[i])
                # mg1 token-halved: half 0's LN1 stats/broadcast hide under
                # half 1's chains, and FFN1's first group (also token-halved,
                # below) starts right after the half-0 applies
                a_ps2 = [psa.tile([P, 512], F32, name=f"mm_ps{i}")
                         for i in range(4)]
                ln1_bh = {}
                for half in range(2):
                    hs = slice(half * 256, (half + 1) * 256)
                    jbs = slice(half * 2, half * 2 + 2)
                    for i in range(4):
                        m = 4 + i
                        for kc in range(KD):
                            nc.tensor.matmul(
                                a_ps2[i][:, hs],
                                wo_sb[:, kc, 512 + i * P:512 + (i + 1) * P],
                                ct_sb[:, kc, jbs, :],
                                start=(kc == 0), stop=(kc == KD - 1))
                        nc.vector.tensor_add(y1_sb[:, m, hs],
                                             a_ps2[i][:, hs],
                                             xts_sb[:, m, hs])
                        sq = sqp.tile([P, 512], BF16, name="sq", bufs=14)
                        nc.vector.tensor_mul(sq[:, hs], y1_sb[:, m, hs],
                                             y1_sb[:, m, hs])
                        sqs[(m, half)] = sq
                    for kc in range(KD):
                        nc.tensor.matmul(st1_sum[:, hs], ones[:],
                                         y1_sb[:, kc, hs],
                                         start=(kc == 0), stop=(kc == KD - 1))
                    for kc in range(KD):
                        s = sqs[kc] if kc < 4 else sqs[(kc, half)]
                        nc.tensor.matmul(st1_sq[:, hs], ones[:], s[:, hs],
                                         start=(kc == 0), stop=(kc == KD - 1))
                    ln1_bh[half] = ln_finish(st1_sum[:, hs], st1_sq[:, hs],
                                             f"ln1h{half}", w=256)
                sqs.clear()
                for half in range(2):
                    hs = slice(half * 256, (half + 1) * 256)
                    rstd_h, ms_h = ln1_bh[half]
                    for kc in range(KD):
                        ln_apply(y1_sb, rstd_h, ms_h, g1, be1, z1_sb, kc,
                                 cols=hs)

            # ---- FFN1 + FFN2 (first output half interleaved) ----
            with tc.tile_pool(name="psa2", bufs=1, space="PSUM") as psa2:
                f_ps = [psa2.tile([P, 512], F32, name=f"f_ps{i}")
                        for i in range(4)]
                for fg in range(KD):
                    w1_tile = w1p.tile([P, KD, 512], BF16, name="w1_tile")
                    nc.sync.dma_start(w1_tile[:], w18[fg])
                    h_ps = [psa2.tile([P, 512], F32, name=f"h_ps{i}")
                            for i in range(4)]
                    if fg == 0:
                        # token-halved so the half-0 chains start as soon as
                        # the half-0 LN1 applies land
                        for half in range(2):
                            hs = slice(half * 256, (half + 1) * 256)
                            for kc in range(KD):
                                for i in range(4):
                                    nc.tensor.matmul(
                                        h_ps[i][:, hs],
                                        w1_tile[:, kc, i * P:(i + 1) * P],
                                        z1_sb[:, kc, hs],
                                        start=(kc == 0), stop=(kc == KD - 1))
                    else:
                        for kc in range(KD):
                            for i in range(4):
                                nc.tensor.matmul(
                                    h_ps[i][:], w1_tile[:, kc, i * P:(i + 1) * P],
                                    z1_sb[:, kc], start=(kc == 0),
                                    stop=(kc == KD - 1))
                    for i in range(4):
                        fm = fg * 4 + i
                        nc.scalar.activation(h_sb[:, fm], h_ps[i][:], AF.Relu,
                                             bias=b1c[fm])
                    w2_tile = w2p.tile([P, 4, 512], BF16, name="w2_tile")
                    nc.sync.dma_start(w2_tile[:], w2a8[fg])
                    nc.sync.dma_start(w2b_sb[:, fg], w2b8[fg])
                    for i in range(4):
                        fk = fg * 4 + i
                        for j in range(4):
                            nc.tensor.matmul(
                                f_ps[j][:], w2_tile[:, i, j * P:(j + 1) * P],
                                h_sb[:, fk], start=(fk == 0), stop=(fk == KF - 1))
                # y2 first half + squares; LN2 stats for it run behind
                # FFN2b's first chain
                st2_sum = psa2.tile([1, 512], F32, name="h_ps0", tag="h_ps0")
                st2_sq = psa2.tile([1, 512], F32, name="h_ps1", tag="h_ps1")
                sqs2 = {}

                def y2_add_sq(m, ps):
                    nc.vector.scalar_tensor_tensor(
                        out=y2_sb[:, m], in0=ps[:], scalar=b2c[m],
                        in1=z1_sb[:, m], op0=OP.add, op1=OP.add)
                    sq = sqp.tile([P, 512], BF16, name="sq", bufs=14)
                    nc.vector.tensor_mul(sq[:], y2_sb[:, m], y2_sb[:, m])
                    sqs2[m] = sq

                for j in range(4):
                    y2_add_sq(j, f_ps[j])

                # token-halved FFN2 second half: half 0's chains, stats,
                # and LN2 applies complete while half 1's chains run, so only
                # half 1's (narrower) LN2 epilogue remains on the tail
                f_ps2 = [psa2.tile([P, 512], F32, name=f"f_ps{i}")
                         for i in range(4)]
                segs = [(0, 256), (256, 384), (384, 512)]
                for si, (s0, s1) in enumerate(segs):
                    hs = slice(s0, s1)
                    for j in range(4):
                        m = 4 + j
                        for fg in range(KD):
                            for i in range(4):
                                nc.tensor.matmul(
                                    f_ps2[j][:, hs],
                                    w2b_sb[:, fg, i, j * P:(j + 1) * P],
                                    h_sb[:, fg * 4 + i, hs],
                                    start=(fg == 0 and i == 0),
                                    stop=(fg == KD - 1 and i == 3))
                        nc.vector.scalar_tensor_tensor(
                            out=y2_sb[:, m, hs], in0=f_ps2[j][:, hs],
                            scalar=b2c[m], in1=z1_sb[:, m, hs],
                            op0=OP.add, op1=OP.add)
                        sq = sqp.tile([P, 512], BF16, name="sq", bufs=14)
                        nc.vector.tensor_mul(sq[:, hs], y2_sb[:, m, hs],
                                             y2_sb[:, m, hs])
                        sqs2[(m, si)] = sq
                        if j == 0:
                            for kc in range(4):
                                nc.tensor.matmul(
                                    st2_sum[:, hs], ones[:],
                                    y2_sb[:, kc, hs],
                                    start=(kc == 0), stop=False)
                                nc.tensor.matmul(
                                    st2_sq[:, hs], ones[:],
                                    sqs2[kc][:, hs],
                                    start=(kc == 0), stop=False)
                        else:
                            mm1 = 4 + j - 1
                            nc.tensor.matmul(
                                st2_sum[:, hs], ones[:], y2_sb[:, mm1, hs],
                                start=False, stop=False)
                            nc.tensor.matmul(
                                st2_sq[:, hs], ones[:],
                                sqs2[(mm1, si)][:, hs],
                                start=False, stop=False)
                    nc.tensor.matmul(st2_sum[:, hs], ones[:],
                                     y2_sb[:, 7, hs], start=False, stop=True)
                    nc.tensor.matmul(st2_sq[:, hs], ones[:],
                                     sqs2[(7, si)][:, hs],
                                     start=False, stop=True)
                    rstd_h, ms_h = ln_finish(st2_sum[:, hs], st2_sq[:, hs],
                                             f"ln2s{si}", w=s1 - s0)
                    for kc in range(KD):
                        ln_apply(y2_sb, rstd_h, ms_h, g2, be2, z2_sb, kc,
                                 cols=hs)
                        nc.sync.dma_start(out8[:, kc, hs], z2_sb[:, kc, hs])
    nc.compile()
    return nc


def _get(name, builder):
    if name not in _CACHE:
        _CACHE[name] = builder()
    return _CACHE[name]


def _prep_inputs(X, Wq, Wk, Wo, ln1_g, ln1_b, ln2_g, ln2_b, W1, b1, W2, b2):
    f = lambda a: np.ascontiguousarray(a)
    Xt = np.asarray(X, np.float32).reshape(N, D).T          # [D, N]
    WqT = np.asarray(Wq, np.float32).T                      # [D, D]
    WkT = np.asarray(Wk, np.float32).T
    WoT = np.asarray(Wo, np.float32).T
    W1T = np.asarray(W1, np.float32).T                      # [D, FF]
    W2T = np.asarray(W2, np.float32).T                      # [FF, D]
    vecP = lambda v, k: np.asarray(v, np.float32).reshape(k, P).T  # [P, k]

    # xt8: [idx, p, kc, q]
    xt8 = f(Xt.reshape(KD, P, NQ, 512).transpose(2, 1, 0, 3))
    id64b = np.tile(np.eye(DH, dtype=np.float32), (2, 1)).astype(BF)
    w_tile = lambda w, c: f(
        w[:, c * P:(c + 1) * P].reshape(KD, P, P).transpose(1, 0, 2))

    in_maps_a = [
        {
            "xt8": xt8,
            "id64b": id64b,
            "wqt8": w_tile(WqT, c),
            "wkt8": w_tile(WkT, c),
            "wvt8": w_tile(WoT, c),   # value projection uses W_o in this model
        }
        for c in range(N_CORES)
    ]

    wo8 = f(WoT.reshape(KD, P, D).transpose(1, 0, 2).astype(BF))
    w18 = f(W1T.reshape(KD, P, KD, 512).transpose(2, 1, 0, 3).astype(BF))
    w2a8 = f(W2T[:, 0:512].reshape(KD, 4, P, 512).transpose(0, 2, 1, 3)
             .astype(BF))
    w2b8 = f(W2T[:, 512:1024].reshape(KD, 4, P, 512).transpose(0, 2, 1, 3)
             .astype(BF))
    consts = f(np.hstack([vecP(ln1_g, KD), vecP(ln1_b, KD), vecP(ln2_g, KD),
                          vecP(ln2_b, KD), vecP(b1, KF), vecP(b2, KD)]))
    id128b = np.eye(P, dtype=BF)

    def in_maps_b(full_nat):
        maps = []
        for c in range(N_CORES):
            blk = full_nat[c * QC:(c + 1) * QC]             # [512, 1024]
            ctin = f(blk.reshape(4, P, KD, P).transpose(1, 2, 0, 3).astype(BF))
            xts = f(Xt[:, c * QC:(c + 1) * QC].reshape(KD, P, 512)
                    .transpose(1, 0, 2))
            maps.append({
                "ctin": ctin, "xts8": xts, "wo8": wo8, "w18": w18,
                "w2a8": w2a8, "w2b8": w2b8, "consts": consts,
                "id128b": id128b,
            })
        return maps

    return in_maps_a, in_maps_b


def kernel(X, Wq, Wk, Wo, ln1_g, ln1_b, ln2_g, ln2_b, W1, b1, W2, b2):
    in_maps_a, in_maps_b = _prep_inputs(
        X, Wq, Wk, Wo, ln1_g, ln1_b, ln2_g, ln2_b, W1, b1, W2, b2)

    nc_a = _get("a", _build_phase_a)
    res_a = run_bass_kernel_spmd(nc_a, in_maps_a, core_ids=list(range(N_CORES)))
    # ctxn [P, NQ, 4, P] per core -> natural [4096, 128] -> concat cols
    full_nat = np.concatenate(
        [res_a.results[c]["ctxn"].transpose(1, 2, 0, 3).reshape(N, P)
         for c in range(N_CORES)], axis=1)                  # [N, D]

    nc_b = _get("b", _build_phase_b)
    res_b = run_bass_kernel_spmd(nc_b, in_maps_b(full_nat),
                                 core_ids=list(range(N_CORES)))
    # out8 [P, KD, 512] per core -> [D, 512] col block of out^T
    out_t = np.concatenate(
        [res_b.results[c]["out8"].astype(np.float32).transpose(1, 0, 2)
         .reshape(D, QC) for c in range(N_CORES)], axis=1)  # [D, N]
    return np.ascontiguousarray(out_t.T).reshape(B, S, D).astype(np.float32)


# revision 46
# speedup vs baseline: 1.1669x; 1.0063x over previous
"""Trainium2 Bass kernel for nn_Encoder (dense transformer block), 8 NeuronCores.

Strategy (single chip, 8 cores), v3:
  Phase A (head-parallel): core c computes attention for heads {2c, 2c+1}.
    Projections run in t-layout; q/k land in bf16, V is PE-transposed into
    natural [keys, dims] bf16 layout. softmax(relu(s)) is p = max(exp(s/8), 1)
    with the softmax denominator taken from a ones column appended to V.
    The exp pass on the Activation engine is the critical resource (~134 us);
    a queue-based emitter keeps it saturated: score matmuls are emitted as
    early as their projections allow (wavefront), projection matmuls are
    spread between them in small pieces, and the context-accumulation chains
    (65-cycle bf16 matmuls in the fast [q,65] orientation) fill the PE's
    exp-paced slack. ctx leaves phase A in natural [token, dim] layout.
  Phase B (row-parallel): core c takes 512 of the 4096 token rows. It
    PE-transposes the incoming ctx back to t-layout fused with the Wo
    matmuls, then AddNorm1, FFN (ReLU, bf16 weights/activations), AddNorm2.
    All weights stream as a handful of large host-pre-tiled bf16 DMAs on the
    SP queue; LayerNorm statistics are accumulated in halves so their matmuls
    and squares overlap the surrounding GEMMs.
"""

import os
import sys

for _p in ("/opt/trn_rl_repo",):
    if _p not in sys.path:
        sys.path.insert(0, _p)

# The Bass SPMD path executes through jax/PJRT on the axon platform; make
# sure a caller-pinned JAX_PLATFORMS=cpu doesn't hide the NeuronCores.
_jp = os.environ.get("JAX_PLATFORMS")
if _jp is not None and "axon" not in _jp:
    os.environ["JAX_PLATFORMS"] = "axon," + _jp

import ml_dtypes
import numpy as np

import concourse.bass as bass
import concourse.mybir as mybir
import concourse.tile as tile
from concourse import bacc
from concourse.bass_utils import run_bass_kernel_spmd

F32 = mybir.dt.float32
F32R = mybir.dt.float32r
BF16 = mybir.dt.bfloat16
AF = mybir.ActivationFunctionType
OP = mybir.AluOpType
BF = ml_dtypes.bfloat16


def _mm(nc, out, lhsT, rhs, **kw):
    # fp32r: 1-pass FP22 matmul (1 cyc/row when the moving dim is >= 256)
    nc.tensor.matmul(out, lhsT.bitcast(F32R), rhs.bitcast(F32R), **kw)


N_CORES = 8
B, S, D, H, DH, FF = 2, 2048, 1024, 16, 64, 4096
N = B * S            # 4096 token rows
P = 128
QC = N // N_CORES    # 512 rows per core in phase B
KD = D // P          # 8 contraction chunks over D
KI = S // P          # 16 key chunks of 128 per batch
NO = S // 512        # 4 query chunks of 512 per batch
NQ = N // 512        # 8 query chunks overall
KF = FF // P         # 32
EPS = 1e-5

_CACHE = {}


# --------------------------------------------------------------------------
# Phase A: per-core head-parallel attention.
# Inputs (per core):
#   xt8  [NQ, P, KD, 512]  X^T tiled per 512-token chunk (replicated)
#   wqt8/wkt8/wvt8 [P, KD, P]  W^T columns for this core's two heads, tiled
#   id64b [P, DH] bf16 tiled identity (V transposes)
# Output:
#   ctxn [P, NQ, 4, P] f32: natural-layout ctx; token = idx*512 + j*128 + p,
#   col = the two heads' 64-dim blocks concatenated.
# --------------------------------------------------------------------------
def _build_phase_a():
    nc = bacc.Bacc("TRN2", target_bir_lowering=False, debug=False,
                   num_devices=N_CORES)
    xt8 = nc.dram_tensor("xt8", [NQ, P, KD, 512], F32R, kind="ExternalInput")
    wqt8 = nc.dram_tensor("wqt8", [P, KD, P], F32R, kind="ExternalInput")
    wkt8 = nc.dram_tensor("wkt8", [P, KD, P], F32R, kind="ExternalInput")
    wvt8 = nc.dram_tensor("wvt8", [P, KD, P], F32R, kind="ExternalInput")
    id64b = nc.dram_tensor("id64b", [P, DH], BF16, kind="ExternalInput")
    ctxn = nc.dram_tensor("ctxn", [P, NQ, 4, P], F32, kind="ExternalOutput")

    chunks = [(b_, o) for b_ in range(B) for o in range(NO)]

    with tile.TileContext(nc) as tc:
        with tc.tile_pool(name="persist", bufs=1) as persist:
            qt_sb = [persist.tile([P, S], BF16, name=f"qt{b_}") for b_ in range(B)]
            kt_sb = [persist.tile([P, S], BF16, name=f"kt{b_}") for b_ in range(B)]
            vt_sb = [persist.tile([P, S], BF16, name=f"vt{b_}") for b_ in range(B)]
            vp_sb = [persist.tile([P, KI, 2, DH + 1], BF16, name=f"vp{b_}")
                     for b_ in range(B)]
            wq_sb = persist.tile([P, KD, P], F32R)
            wk_sb = persist.tile([P, KD, P], F32R)
            wv_sb = persist.tile([P, KD, P], F32R)
            id64_sb = persist.tile([P, DH], BF16)

            for b_ in range(B):
                nc.vector.memset(vp_sb[b_][:, :, 0, DH:DH + 1], 1.0)
                nc.vector.memset(vp_sb[b_][:, :, 1, DH:DH + 1], 1.0)

            with (
                tc.tile_pool(name="xpool", bufs=2) as xpool,
                tc.tile_pool(name="accp", bufs=2, space="PSUM") as accp,
                tc.tile_pool(name="slabp", bufs=50) as slabp,
                tc.tile_pool(name="stagep", bufs=2) as stagep,
                tc.tile_pool(name="smallp", bufs=8) as smallp,
                tc.tile_pool(name="pss", bufs=2, space="PSUM") as pss,
                tc.tile_pool(name="psc", bufs=2, space="PSUM") as psc,
            ):
                xt_tiles = {}

                def issue_xt(ci):
                    t = xpool.tile([P, KD, 512], F32R, name="xt_tile")
                    # two half-DMAs so the first projection matmuls can start
                    # as soon as the front half lands (subtile deps)
                    nc.sync.dma_start(t[:, 0:4], xt8[ci, :, 0:4])
                    nc.sync.dma_start(t[:, 4:8], xt8[ci, :, 4:8])
                    xt_tiles[ci] = t

                def gen_proj_qk(ci):
                    """Generator: project chunk ci into qt/kt (bf16). For the
                    first chunk the q and k chains interleave so both finish
                    (and the first scores emit) as early as possible."""
                    b_, o = chunks[ci]
                    osl = slice(o * 512, (o + 1) * 512)
                    xt_tile = xt_tiles[ci]
                    if ci == 0:
                        accq = accp.tile([P, 512], F32, name="acc_ps",
                                         tag="acc")
                        acck = accp.tile([P, 512], F32, name="acc_ps",
                                         tag="acc")
                        for kc in range(KD):
                            _mm(nc, accq[:], wq_sb[:, kc], xt_tile[:, kc],
                                start=(kc == 0), stop=(kc == KD - 1))
                            _mm(nc, acck[:], wk_sb[:, kc], xt_tile[:, kc],
                                start=(kc == 0), stop=(kc == KD - 1))
                            if kc % 2 == 1:
                                yield
                        nc.vector.tensor_copy(qt_sb[b_][:, osl], accq[:])
                        nc.vector.tensor_copy(kt_sb[b_][:, osl], acck[:])
                        yield
                        return
                    for w_sb, dst in ((wq_sb, qt_sb[b_]), (wk_sb, kt_sb[b_])):
                        acc = accp.tile([P, 512], F32, name="acc_ps", tag="acc")
                        for kc in range(KD):
                            _mm(nc, acc[:], w_sb[:, kc], xt_tile[:, kc],
                                start=(kc == 0), stop=(kc == KD - 1))
                            if kc % 2 == 1:
                                yield
                        nc.vector.tensor_copy(dst[:, osl], acc[:])
                        yield

                def gen_proj_v(ci):
                    """Generator: V projection + natural-layout transposes."""
                    b_, o = chunks[ci]
                    osl = slice(o * 512, (o + 1) * 512)
                    xt_tile = xt_tiles.pop(ci)
                    acc = accp.tile([P, 512], F32, name="acc_ps", tag="acc")
                    for kc in range(KD):
                        _mm(nc, acc[:], wv_sb[:, kc], xt_tile[:, kc],
                            start=(kc == 0), stop=(kc == KD - 1))
                        if kc % 2 == 1:
                            yield
                    nc.vector.tensor_copy(vt_sb[b_][:, osl], acc[:])
                    yield
                    for t in range(4):
                        kc2 = o * 4 + t
                        for hh in range(2):
                            tp = accp.tile([P, DH], BF16, name="tp_ps",
                                           tag="acc")
                            nc.tensor.transpose(
                                tp[:, 0:DH],
                                vt_sb[b_][hh * DH:(hh + 1) * DH,
                                          kc2 * P:(kc2 + 1) * P],
                                id64_sb[hh * DH:(hh + 1) * DH, :])
                            nc.vector.tensor_copy(
                                vp_sb[b_][:, kc2, hh, 0:DH], tp[:, 0:DH])
                        yield

                slabs = {i: {} for i in range(NQ)}   # idx -> kc -> slab tile

                def emit_scores(idx, kc):
                    b_, o = chunks[idx]
                    qs = slice(o * 512, (o + 1) * 512)
                    ks = slice(kc * P, (kc + 1) * P)
                    s_ps = pss.tile([P, 1024], F32, name="s_ps")
                    nc.tensor.matmul(s_ps[:, 0:512], kt_sb[b_][0:DH, ks],
                                     qt_sb[b_][0:DH, qs], start=True, stop=True)
                    nc.tensor.matmul(s_ps[:, 512:1024], kt_sb[b_][DH:2 * DH, ks],
                                     qt_sb[b_][DH:2 * DH, qs],
                                     start=True, stop=True)
                    slab = slabp.tile([P, 1024], BF16, name="slab")
                    nc.scalar.activation(slab[:], s_ps[:], AF.Exp, scale=0.125)
                    nc.vector.tensor_scalar_max(slab[:], slab[:], 1.0)
                    slabs[idx][kc] = slab

                def gen_chains(idx):
                    """Generator: the 8 ctx chains of idx + normalize + DMA,
                    yielding every couple of matmuls."""
                    b_, o = chunks[idx]
                    stage = stagep.tile([P, 4, P], F32, name="stage")
                    for ci in range(8):
                        j, h = ci // 2, ci % 2
                        acc = psc.tile([P, DH + 1], F32, name="ctx_ps")
                        for kc in range(KI):
                            nc.tensor.matmul(
                                acc[:],
                                slabs[idx][kc][:, h * 512 + j * P:
                                               h * 512 + (j + 1) * P],
                                vp_sb[b_][:, kc, h, :],
                                start=(kc == 0), stop=(kc == KI - 1))
                            if kc % 4 == 3:
                                yield
                        inv = smallp.tile([P, 1], F32, name="inv")
                        nc.vector.reciprocal(inv[:], acc[:, DH:DH + 1])
                        nc.vector.tensor_scalar(
                            out=stage[:, j, h * DH:(h + 1) * DH],
                            in0=acc[:, 0:DH], scalar1=inv[:], scalar2=None,
                            op0=OP.mult)
                        yield
                    nc.sync.dma_start(ctxn[:, idx], stage[:])
                    slabs[idx].clear()

                # ---------------- queue-based emitter ----------------
                emitted = set()           # (idx, kc) scores emitted
                score_q = []              # ordered pending scores
                qk_done = [False] * NQ
                v_done = [False] * NQ
                chains_done = 0           # count of fully-emitted chain idxs
                chain_gen = None
                chain_idx = 0             # next idx needing chains
                qk_idx = 0                # next chunk for q/k projection
                v_idx = 0                 # next chunk for v projection
                qkgen = None
                vgen = None

                def update_score_q():
                    for i in range(NQ):
                        bi, _ = chunks[i]
                        if not qk_done[i]:
                            continue
                        if i >= chains_done + 3:
                            continue
                        base = 4 * bi
                        kmax = sum(4 for c in range(base, base + NO)
                                   if qk_done[c])
                        for k in range(kmax):
                            if (i, k) not in emitted and (i, k) not in score_q:
                                score_q.append((i, k))

                # DMA order tuned so the first q-projection matmuls can
                # start at ~4.5us: wq, then the first xt half, then wk etc.
                nc.sync.dma_start(wq_sb[:], wqt8.ap())
                issue_xt(0)
                nc.sync.dma_start(wk_sb[:], wkt8.ap())
                nc.sync.dma_start(wv_sb[:], wvt8.ap())
                issue_xt(1)
                nc.sync.dma_start(id64_sb[:], id64b.ap())
                while (qk_idx < NQ or v_idx < NQ or score_q
                       or chain_idx < NQ or chain_gen is not None):
                    # 1. a slice of chain work (PE filler, no Act dependency)
                    if chain_gen is None and chain_idx < NQ:
                        bci, _ = chunks[chain_idx]
                        if (len(slabs[chain_idx]) == KI
                                and all(v_done[c] for c in
                                        range(4 * bci, 4 * bci + NO))):
                            chain_gen = gen_chains(chain_idx)
                    if chain_gen is not None:
                        for _ in range(3 if score_q else 8):
                            try:
                                next(chain_gen)
                            except StopIteration:
                                chain_gen = None
                                chain_idx += 1
                                chains_done += 1
                                update_score_q()
                                break
                    # 2. q/k projection pieces (gate scores)
                    if qkgen is None and qk_idx < NQ and qk_idx <= v_idx:
                        qkgen = gen_proj_qk(qk_idx)
                    if qkgen is not None:
                        steps = 1 if score_q else 4
                        for _ in range(steps):
                            try:
                                next(qkgen)
                            except StopIteration:
                                qk_done[qk_idx] = True
                                qk_idx += 1
                                if qk_idx + 1 < NQ:
                                    issue_xt(qk_idx + 1)
                                qkgen = None
                                update_score_q()
                                break
                    # 3. v projection + transposes (gate chains only)
                    if vgen is None and v_idx < NQ and v_idx < qk_idx:
                        vgen = gen_proj_v(v_idx)
                    if vgen is not None:
                        # boost only when idle or when chains are starved on v
                        chain_starved = (
                            chain_gen is None and chain_idx < NQ
                            and len(slabs[chain_idx]) == KI)
                        steps = 2 if (chain_starved or not score_q
                                      or v_idx < qk_idx - 1) else 1
                        for _ in range(steps):
                            try:
                                next(vgen)
                            except StopIteration:
                                v_done[v_idx] = True
                                v_idx += 1
                                vgen = None
                                break
                    # 4. one score (the Act engine's food)
                    if score_q:
                        i, k = score_q.pop(0)
                        emit_scores(i, k)
                        emitted.add((i, k))
                        update_score_q()
    nc.compile()
    return nc


# --------------------------------------------------------------------------
# Phase B: per-core row-parallel transpose + Wo-proj + AddNorm1 + FFN + AddNorm2.
# Inputs (per core, qi = this core's 512 token rows):
#   ctin [P, 4, KD, P] bf16   natural-layout ctx blocks for these rows
#   wo8  [P, KD, D]    bf16   Wo^T tiled
#   w18  [KD, P, KD, 512] bf16  W1^T tiled per 512-wide ffn-col group
#   w2a8/w2b8 [KD, P, 4, 512] bf16  W2^T tiled, first/second output half
#   xts8 [P, KD, 512] f32     X^T slice (residual 1)
#   consts [P, 72] f32        g1|be1|g2|be2|b1t|b2t feature-on-partition
#   id128b [P, P] bf16
# Output: out8 [P, KD, 512] f32 (t-layout output slice, tiled)
# --------------------------------------------------------------------------
def _build_phase_b():
    nc = bacc.Bacc("TRN2", target_bir_lowering=False, debug=False,
                   num_devices=N_CORES)
    ctin = nc.dram_tensor("ctin", [P, KD, 4, P], BF16, kind="ExternalInput")
    wo8 = nc.dram_tensor("wo8", [P, KD, D], BF16, kind="ExternalInput")
    w18 = nc.dram_tensor("w18", [KD, P, KD, 512], BF16, kind="ExternalInput")
    w2a8 = nc.dram_tensor("w2a8", [KD, P, 4, 512], BF16, kind="ExternalInput")
    w2b8 = nc.dram_tensor("w2b8", [KD, P, 4, 512], BF16, kind="ExternalInput")
    xts8 = nc.dram_tensor("xts8", [P, KD, 512], F32, kind="ExternalInput")
    consts = nc.dram_tensor("consts", [P, 72], F32, kind="ExternalInput")
    id128b = nc.dram_tensor("id128b", [P, P], BF16, kind="ExternalInput")
    outs = [nc.dram_tensor(f"outs{i}", [P, KD, w], BF16,
                       kind="ExternalOutput")
        for i, w in enumerate((256, 128, 128))]

    with tile.TileContext(nc) as tc:
        with (
            tc.tile_pool(name="persist", bufs=1) as persist,
            tc.tile_pool(name="w1p", bufs=3) as w1p,
            tc.tile_pool(name="w2p", bufs=3) as w2p,
            tc.tile_pool(name="sqp", bufs=3) as sqp,
            tc.tile_pool(name="smallp", bufs=2) as smallp,
            tc.tile_pool(name="bcp", bufs=2) as bcp,
        ):
            ctin_sb = persist.tile([P, KD, 4, P], BF16)
            ct_sb = persist.tile([P, KD, 4, P], BF16)
            wo_sb = persist.tile([P, KD, D], BF16)
            xts_sb = persist.tile([P, KD, 512], F32)
            y1_sb = persist.tile([P, KD, 512], BF16)
            z1_sb = persist.tile([P, KD, 512], BF16)
            h_sb = persist.tile([P, KF, 512], BF16)
            w2b_sb = persist.tile([P, KD, 4, 512], BF16)
            y2_sb = persist.tile([P, KD, 512], BF16, tag="y1_sb")
            z2_seg = [persist.tile([P, KD, w], BF16, name=f"z2s{i}", tag="xts_sb",
                       bufs=1)
          for i, w in enumerate((256, 128, 128))]
            consts_sb = persist.tile([P, 72], F32)
            id128_sb = persist.tile([P, P], BF16)
            ones = persist.tile([P, 1], BF16)

            nc.sync.dma_start(ctin_sb[:, 0:2], ctin[:, 0:2])
            nc.sync.dma_start(id128_sb[:], id128b.ap())
            nc.sync.dma_start(wo_sb[:, 0:2], wo8[:, 0:2])
            nc.sync.dma_start(ctin_sb[:, 2:8], ctin[:, 2:8])
            nc.sync.dma_start(wo_sb[:, 2:8], wo8[:, 2:8])
            nc.sync.dma_start(consts_sb[:], consts.ap())
            nc.sync.dma_start(xts_sb[:], xts8.ap())
            nc.vector.memset(ones[:], 1.0)

            g1 = [consts_sb[:, kc:kc + 1] for kc in range(KD)]
            be1 = [consts_sb[:, 8 + kc:9 + kc] for kc in range(KD)]
            g2 = [consts_sb[:, 16 + kc:17 + kc] for kc in range(KD)]
            be2 = [consts_sb[:, 24 + kc:25 + kc] for kc in range(KD)]
            b1c = [consts_sb[:, 32 + fm:33 + fm] for fm in range(KF)]
            b2c = [consts_sb[:, 64 + kc:65 + kc] for kc in range(KD)]

            def ln_finish(st_sum, st_sq, tag, w=512):
                """Stats (over w tokens) -> (rstd_b, ms_b) broadcast tiles."""
                mean = smallp.tile([1, w], F32, name="mean")
                ex2 = smallp.tile([1, w], F32, name="ex2")
                nc.vector.tensor_scalar(out=mean[:], in0=st_sum,
                                        scalar1=1.0 / D, scalar2=None,
                                        op0=OP.mult)
                nc.vector.tensor_scalar(out=ex2[:], in0=st_sq,
                                        scalar1=1.0 / D, scalar2=None,
                                        op0=OP.mult)
                msq = smallp.tile([1, w], F32, name="msq")
                nc.vector.tensor_mul(msq[:], mean[:], mean[:])
                var = smallp.tile([1, w], F32, name="var")
                nc.vector.tensor_sub(var[:], ex2[:], msq[:])
                nc.vector.tensor_scalar_add(var[:], var[:], EPS)
                std = smallp.tile([1, w], F32, name="std")
                nc.scalar.activation(std[:], var[:], AF.Sqrt)
                rstd = smallp.tile([1, w], BF16, name="rstd")
                ms = smallp.tile([1, w], BF16, name="ms")
                with nc.allow_low_precision(reason="bf16 LN scale factors"):
                    nc.vector.reciprocal(rstd[:], std[:])
                nc.vector.tensor_mul(ms[:], mean[:], rstd[:])
                rstd_b = bcp.tile([P, w], BF16, name="rstd_b", bufs=3)
                ms_b = bcp.tile([P, w], BF16, name="ms_b", bufs=3)
                nc.gpsimd.partition_broadcast(rstd_b[:], rstd[:])
                nc.gpsimd.partition_broadcast(ms_b[:], ms[:])
                return rstd_b, ms_b

            def ln_apply_seg(y_sb, rstd_b, ms_b, g_c, be_c, z_t, kc, cols):
                w = cols.stop - cols.start
                t = sqp.tile([P, 512], BF16, name="t_ln")
                nc.vector.tensor_mul(t[:, 0:w], y_sb[:, kc, cols], rstd_b[:])
                nc.vector.tensor_sub(t[:, 0:w], t[:, 0:w], ms_b[:])
                nc.vector.tensor_scalar(out=z_t[:, kc], in0=t[:, 0:w],
                                        scalar1=g_c[kc], scalar2=be_c[kc],
                                        op0=OP.mult, op1=OP.add)

            def ln_apply(y_sb, rstd_b, ms_b, g_c, be_c, z_sb, kc, cols=None):
                cols = cols or slice(0, 512)
                w = cols.stop - cols.start
                t = sqp.tile([P, 512], BF16, name="t_ln")
                nc.vector.tensor_mul(t[:, 0:w], y_sb[:, kc, cols], rstd_b[:])
                nc.vector.tensor_sub(t[:, 0:w], t[:, 0:w], ms_b[:])
                nc.vector.tensor_scalar(out=z_sb[:, kc, cols], in0=t[:, 0:w],
                                        scalar1=g_c[kc], scalar2=be_c[kc],
                                        op0=OP.mult, op1=OP.add)

            # ---- B0+B1: transpose ctx to t-layout, fused with Wo matmuls ----
            with (
                tc.tile_pool(name="tpp", bufs=2, space="PSUM") as tpp,
                tc.tile_pool(name="psa", bufs=1, space="PSUM") as psa,
                tc.tile_pool(name="psst1", bufs=1, space="PSUM") as psst1,
            ):
                st1_sum = psst1.tile([1, 512], F32, name="st1_sum")
                st1_sq = psst1.tile([1, 512], F32, name="st1_sq")
                # chain-major Wo: each output chain stops early so its
                # residual add + square overlap the following chains
                a_ps = [psa.tile([P, 512], F32, name=f"mm_ps{i}")
                        for i in range(4)]
                sqs = {}

                def y1_add_sq(m, ps):
                    nc.vector.tensor_add(y1_sb[:, m], ps[:], xts_sb[:, m])
                    sq = sqp.tile([P, 512], BF16, name="sq", bufs=14)
                    nc.vector.tensor_mul(sq[:], y1_sb[:, m], y1_sb[:, m])
                    sqs[m] = sq

                for kc in range(KD):
                    tp = tpp.tile([P, 4, P], BF16, name="tp_ps")
                    for jb in range(4):
                        nc.tensor.transpose(tp[:, jb, :],
                                            ctin_sb[:, kc, jb, :],
                                            id128_sb[:])
                    nc.vector.tensor_copy(ct_sb[:, kc], tp[:])
                    nc.tensor.matmul(a_ps[0][:], wo_sb[:, kc, 0:P],
                                     ct_sb[:, kc],
                                     start=(kc == 0), stop=(kc == KD - 1))
                y1_add_sq(0, a_ps[0])
                for i in range(1, 4):
                    for kc in range(KD):
                        nc.tensor.matmul(
                            a_ps[i][:], wo_sb[:, kc, i * P:(i + 1) * P],
                            ct_sb[:, kc], start=(kc == 0), stop=(kc == KD - 1))
                    y1_add_sq(i, a_ps[i])
                # mg1 token-halved: half 0's LN1 stats/broadcast hide under
                # half 1's chains, and FFN1's first group (also token-halved,
                # below) starts right after the half-0 applies
                a_ps2 = [psa.tile([P, 512], F32, name=f"mm_ps{i}")
                         for i in range(4)]
                ln1_bh = {}
                for half in range(2):
                    hs = slice(half * 256, (half + 1) * 256)
                    jbs = slice(half * 2, half * 2 + 2)
                    for i in range(4):
                        m = 4 + i
                        for kc in range(KD):
                            nc.tensor.matmul(
                                a_ps2[i][:, hs],
                                wo_sb[:, kc, 512 + i * P:512 + (i + 1) * P],
                                ct_sb[:, kc, jbs, :],
                                start=(kc == 0), stop=(kc == KD - 1))
                        nc.vector.tensor_add(y1_sb[:, m, hs],
                                             a_ps2[i][:, hs],
                                             xts_sb[:, m, hs])
                        sq = sqp.tile([P, 512], BF16, name="sq", bufs=14)
                        nc.vector.tensor_mul(sq[:, hs], y1_sb[:, m, hs],
                                             y1_sb[:, m, hs])
                        sqs[(m, half)] = sq
                    for kc in range(KD):
                        nc.tensor.matmul(st1_sum[:, hs], ones[:],
                                         y1_sb[:, kc, hs],
                                         start=(kc == 0), stop=(kc == KD - 1))
                    for kc in range(KD):
                        s = sqs[kc] if kc < 4 else sqs[(kc, half)]
                        nc.tensor.matmul(st1_sq[:, hs], ones[:], s[:, hs],
                                         start=(kc == 0), stop=(kc == KD - 1))
                    ln1_bh[half] = ln_finish(st1_sum[:, hs], st1_sq[:, hs],
                                             f"ln1h{half}", w=256)
                sqs.clear()
                for half in range(2):
                    hs = slice(half * 256, (half + 1) * 256)
                    rstd_h, ms_h = ln1_bh[half]
                    for kc in range(KD):
                        ln_apply(y1_sb, rstd_h, ms_h, g1, be1, z1_sb, kc,
                                 cols=hs)

            # ---- FFN1 + FFN2 (first output half interleaved) ----
            with tc.tile_pool(name="psa2", bufs=1, space="PSUM") as psa2:
                f_ps = [psa2.tile([P, 512], F32, name=f"f_ps{i}")
                        for i in range(4)]
                for fg in range(KD):
                    w1_tile = w1p.tile([P, KD, 512], BF16, name="w1_tile")
                    nc.sync.dma_start(w1_tile[:], w18[fg])
                    h_ps = [psa2.tile([P, 512], F32, name=f"h_ps{i}")
                            for i in range(4)]
                    if fg == 0:
                        # token-halved so the half-0 chains start as soon as
                        # the half-0 LN1 applies land
                        for half in range(2):
                            hs = slice(half * 256, (half + 1) * 256)
                            for kc in range(KD):
                                for i in range(4):
                                    nc.tensor.matmul(
                                        h_ps[i][:, hs],
                                        w1_tile[:, kc, i * P:(i + 1) * P],
                                        z1_sb[:, kc, hs],
                                        start=(kc == 0), stop=(kc == KD - 1))
                    else:
                        for kc in range(KD):
                            for i in range(4):
                                nc.tensor.matmul(
                                    h_ps[i][:], w1_tile[:, kc, i * P:(i + 1) * P],
                                    z1_sb[:, kc], start=(kc == 0),
                                    stop=(kc == KD - 1))
                    for i in range(4):
                        fm = fg * 4 + i
                        nc.scalar.activation(h_sb[:, fm], h_ps[i][:], AF.Relu,
                                             bias=b1c[fm])
                    w2_tile = w2p.tile([P, 4, 512], BF16, name="w2_tile")
                    nc.sync.dma_start(w2_tile[:], w2a8[fg])
                    nc.sync.dma_start(w2b_sb[:, fg], w2b8[fg])
                    for i in range(4):
                        fk = fg * 4 + i
                        for j in range(4):
                            nc.tensor.matmul(
                                f_ps[j][:], w2_tile[:, i, j * P:(j + 1) * P],
                                h_sb[:, fk], start=(fk == 0), stop=(fk == KF - 1))
                # y2 first half + squares; LN2 stats for it run behind
                # FFN2b's first chain
                st2_sum = psa2.tile([1, 512], F32, name="h_ps0", tag="h_ps0")
                st2_sq = psa2.tile([1, 512], F32, name="h_ps1", tag="h_ps1")
                sqs2 = {}

                def y2_add_sq(m, ps):
                    nc.vector.scalar_tensor_tensor(
                        out=y2_sb[:, m], in0=ps[:], scalar=b2c[m],
                        in1=z1_sb[:, m], op0=OP.add, op1=OP.add)
                    sq = sqp.tile([P, 512], BF16, name="sq", bufs=14)
                    nc.vector.tensor_mul(sq[:], y2_sb[:, m], y2_sb[:, m])
                    sqs2[m] = sq

                for j in range(4):
                    y2_add_sq(j, f_ps[j])

                # token-halved FFN2 second half: half 0's chains, stats,
                # and LN2 applies complete while half 1's chains run, so only
                # half 1's (narrower) LN2 epilogue remains on the tail
                f_ps2 = [psa2.tile([P, 512], F32, name=f"f_ps{i}")
                         for i in range(4)]
                segs = [(0, 256), (256, 384), (384, 512)]
                for si, (s0, s1) in enumerate(segs):
                    hs = slice(s0, s1)
                    for j in range(4):
                        m = 4 + j
                        for fg in range(KD):
                            for i in range(4):
                                nc.tensor.matmul(
                                    f_ps2[j][:, hs],
                                    w2b_sb[:, fg, i, j * P:(j + 1) * P],
                                    h_sb[:, fg * 4 + i, hs],
                                    start=(fg == 0 and i == 0),
                                    stop=(fg == KD - 1 and i == 3))
                        nc.vector.scalar_tensor_tensor(
                            out=y2_sb[:, m, hs], in0=f_ps2[j][:, hs],
                            scalar=b2c[m], in1=z1_sb[:, m, hs],
                            op0=OP.add, op1=OP.add)
                        sq = sqp.tile([P, 512], BF16, name="sq", bufs=14)
                        nc.vector.tensor_mul(sq[:, hs], y2_sb[:, m, hs],
                                             y2_sb[:, m, hs])
                        sqs2[(m, si)] = sq
                        if j == 0:
                            for kc in range(4):
                                nc.tensor.matmul(
                                    st2_sum[:, hs], ones[:],
                                    y2_sb[:, kc, hs],
                                    start=(kc == 0), stop=False)
                                nc.tensor.matmul(
                                    st2_sq[:, hs], ones[:],
                                    sqs2[kc][:, hs],
                                    start=(kc == 0), stop=False)
                        else:
                            mm1 = 4 + j - 1
                            nc.tensor.matmul(
                                st2_sum[:, hs], ones[:], y2_sb[:, mm1, hs],
                                start=False, stop=False)
                            nc.tensor.matmul(
                                st2_sq[:, hs], ones[:],
                                sqs2[(mm1, si)][:, hs],
                                start=False, stop=False)
                    nc.tensor.matmul(st2_sum[:, hs], ones[:],
                                     y2_sb[:, 7, hs], start=False, stop=True)
                    nc.tensor.matmul(st2_sq[:, hs], ones[:],
                                     sqs2[(7, si)][:, hs],
                                     start=False, stop=True)
                    rstd_h, ms_h = ln_finish(st2_sum[:, hs], st2_sq[:, hs],
                                             f"ln2s{si}", w=s1 - s0)
                    for kc in range(KD):
                        ln_apply_seg(y2_sb, rstd_h, ms_h, g2, be2,
                                     z2_seg[si], kc, hs)
                    nc.sync.dma_start(outs[si].ap(), z2_seg[si][:])
    nc.compile()
    return nc


def _get(name, builder):
    if name not in _CACHE:
        _CACHE[name] = builder()
    return _CACHE[name]


def _prep_inputs(X, Wq, Wk, Wo, ln1_g, ln1_b, ln2_g, ln2_b, W1, b1, W2, b2):
    f = lambda a: np.ascontiguousarray(a)
    Xt = np.asarray(X, np.float32).reshape(N, D).T          # [D, N]
    WqT = np.asarray(Wq, np.float32).T                      # [D, D]
    WkT = np.asarray(Wk, np.float32).T
    WoT = np.asarray(Wo, np.float32).T
    W1T = np.asarray(W1, np.float32).T                      # [D, FF]
    W2T = np.asarray(W2, np.float32).T                      # [FF, D]
    vecP = lambda v, k: np.asarray(v, np.float32).reshape(k, P).T  # [P, k]

    # xt8: [idx, p, kc, q]
    xt8 = f(Xt.reshape(KD, P, NQ, 512).transpose(2, 1, 0, 3))
    id64b = np.tile(np.eye(DH, dtype=np.float32), (2, 1)).astype(BF)
    w_tile = lambda w, c: f(
        w[:, c * P:(c + 1) * P].reshape(KD, P, P).transpose(1, 0, 2))

    in_maps_a = [
        {
            "xt8": xt8,
            "id64b": id64b,
            "wqt8": w_tile(WqT, c),
            "wkt8": w_tile(WkT, c),
            "wvt8": w_tile(WoT, c),   # value projection uses W_o in this model
        }
        for c in range(N_CORES)
    ]

    wo8 = f(WoT.reshape(KD, P, D).transpose(1, 0, 2).astype(BF))
    w18 = f(W1T.reshape(KD, P, KD, 512).transpose(2, 1, 0, 3).astype(BF))
    w2a8 = f(W2T[:, 0:512].reshape(KD, 4, P, 512).transpose(0, 2, 1, 3)
             .astype(BF))
    w2b8 = f(W2T[:, 512:1024].reshape(KD, 4, P, 512).transpose(0, 2, 1, 3)
             .astype(BF))
    consts = f(np.hstack([vecP(ln1_g, KD), vecP(ln1_b, KD), vecP(ln2_g, KD),
                          vecP(ln2_b, KD), vecP(b1, KF), vecP(b2, KD)]))
    id128b = np.eye(P, dtype=BF)

    def in_maps_b(full_nat):
        maps = []
        for c in range(N_CORES):
            blk = full_nat[c * QC:(c + 1) * QC]             # [512, 1024]
            ctin = f(blk.reshape(4, P, KD, P).transpose(1, 2, 0, 3).astype(BF))
            xts = f(Xt[:, c * QC:(c + 1) * QC].reshape(KD, P, 512)
                    .transpose(1, 0, 2))
            maps.append({
                "ctin": ctin, "xts8": xts, "wo8": wo8, "w18": w18,
                "w2a8": w2a8, "w2b8": w2b8, "consts": consts,
                "id128b": id128b,
            })
        return maps

    return in_maps_a, in_maps_b


def kernel(X, Wq, Wk, Wo, ln1_g, ln1_b, ln2_g, ln2_b, W1, b1, W2, b2):
    in_maps_a, in_maps_b = _prep_inputs(
        X, Wq, Wk, Wo, ln1_g, ln1_b, ln2_g, ln2_b, W1, b1, W2, b2)

    nc_a = _get("a", _build_phase_a)
    res_a = run_bass_kernel_spmd(nc_a, in_maps_a, core_ids=list(range(N_CORES)))
    # ctxn [P, NQ, 4, P] per core -> natural [4096, 128] -> concat cols
    full_nat = np.concatenate(
        [res_a.results[c]["ctxn"].transpose(1, 2, 0, 3).reshape(N, P)
         for c in range(N_CORES)], axis=1)                  # [N, D]

    nc_b = _get("b", _build_phase_b)
    res_b = run_bass_kernel_spmd(nc_b, in_maps_b(full_nat),
                                 core_ids=list(range(N_CORES)))
    # outs{0,1,2} [P, KD, w] per core (token segments) -> [D, 512] col block
    out_t = np.concatenate(
        [np.concatenate(
            [res_b.results[c][f"outs{i}"].astype(np.float32) for i in range(3)],
            axis=2).transpose(1, 0, 2).reshape(D, QC)
         for c in range(N_CORES)], axis=1)                  # [D, N]
    return np.ascontiguousarray(out_t.T).reshape(B, S, D).astype(np.float32)


# revision 48
# speedup vs baseline: 1.1766x; 1.0083x over previous
"""Trainium2 Bass kernel for nn_Encoder (dense transformer block), 8 NeuronCores.

Strategy (single chip, 8 cores), v3:
  Phase A (head-parallel): core c computes attention for heads {2c, 2c+1}.
    Projections run in t-layout; q/k land in bf16, V is PE-transposed into
    natural [keys, dims] bf16 layout. softmax(relu(s)) is p = max(exp(s/8), 1)
    with the softmax denominator taken from a ones column appended to V.
    The exp pass on the Activation engine is the critical resource (~134 us);
    a queue-based emitter keeps it saturated: score matmuls are emitted as
    early as their projections allow (wavefront), projection matmuls are
    spread between them in small pieces, and the context-accumulation chains
    (65-cycle bf16 matmuls in the fast [q,65] orientation) fill the PE's
    exp-paced slack. ctx leaves phase A in natural [token, dim] layout.
  Phase B (row-parallel): core c takes 512 of the 4096 token rows. It
    PE-transposes the incoming ctx back to t-layout fused with the Wo
    matmuls, then AddNorm1, FFN (ReLU, bf16 weights/activations), AddNorm2.
    All weights stream as a handful of large host-pre-tiled bf16 DMAs on the
    SP queue; LayerNorm statistics are accumulated in halves so their matmuls
    and squares overlap the surrounding GEMMs.
"""

import os
import sys

for _p in ("/opt/trn_rl_repo",):
    if _p not in sys.path:
        sys.path.insert(0, _p)

# The Bass SPMD path executes through jax/PJRT on the axon platform; make
# sure a caller-pinned JAX_PLATFORMS=cpu doesn't hide the NeuronCores.
_jp = os.environ.get("JAX_PLATFORMS")
if _jp is not None and "axon" not in _jp:
    os.environ["JAX_PLATFORMS"] = "axon," + _jp

import ml_dtypes
import numpy as np

import concourse.bass as bass
import concourse.mybir as mybir
import concourse.tile as tile
from concourse import bacc
from concourse.bass_utils import run_bass_kernel_spmd

F32 = mybir.dt.float32
F32R = mybir.dt.float32r
BF16 = mybir.dt.bfloat16
AF = mybir.ActivationFunctionType
OP = mybir.AluOpType
BF = ml_dtypes.bfloat16


def _mm(nc, out, lhsT, rhs, **kw):
    # fp32r: 1-pass FP22 matmul (1 cyc/row when the moving dim is >= 256)
    nc.tensor.matmul(out, lhsT.bitcast(F32R), rhs.bitcast(F32R), **kw)


N_CORES = 8
B, S, D, H, DH, FF = 2, 2048, 1024, 16, 64, 4096
N = B * S            # 4096 token rows
P = 128
QC = N // N_CORES    # 512 rows per core in phase B
KD = D // P          # 8 contraction chunks over D
KI = S // P          # 16 key chunks of 128 per batch
NO = S // 512        # 4 query chunks of 512 per batch
NQ = N // 512        # 8 query chunks overall
KF = FF // P         # 32
EPS = 1e-5

_CACHE = {}


# --------------------------------------------------------------------------
# Phase A: per-core head-parallel attention.
# Inputs (per core):
#   xt8  [NQ, P, KD, 512]  X^T tiled per 512-token chunk (replicated)
#   wqt8/wkt8/wvt8 [P, KD, P]  W^T columns for this core's two heads, tiled
#   id64b [P, DH] bf16 tiled identity (V transposes)
# Output:
#   ctxn [P, NQ, 4, P] f32: natural-layout ctx; token = idx*512 + j*128 + p,
#   col = the two heads' 64-dim blocks concatenated.
# --------------------------------------------------------------------------
def _build_phase_a():
    nc = bacc.Bacc("TRN2", target_bir_lowering=False, debug=False,
                   num_devices=N_CORES)
    xt8 = nc.dram_tensor("xt8", [NQ, P, KD, 512], F32R, kind="ExternalInput")
    wqt8 = nc.dram_tensor("wqt8", [P, KD, P], F32R, kind="ExternalInput")
    wkt8 = nc.dram_tensor("wkt8", [P, KD, P], F32R, kind="ExternalInput")
    wvt8 = nc.dram_tensor("wvt8", [P, KD, P], F32R, kind="ExternalInput")
    id64b = nc.dram_tensor("id64b", [P, DH], BF16, kind="ExternalInput")
    ctxn = nc.dram_tensor("ctxn", [P, NQ, 4, P], F32, kind="ExternalOutput")

    chunks = [(b_, o) for b_ in range(B) for o in range(NO)]

    with tile.TileContext(nc) as tc:
        with tc.tile_pool(name="persist", bufs=1) as persist:
            qt_sb = [persist.tile([P, S], BF16, name=f"qt{b_}") for b_ in range(B)]
            kt_sb = [persist.tile([P, S], BF16, name=f"kt{b_}") for b_ in range(B)]
            vt_sb = [persist.tile([P, S], BF16, name=f"vt{b_}") for b_ in range(B)]
            vp_sb = [persist.tile([P, KI, 2, DH + 1], BF16, name=f"vp{b_}")
                     for b_ in range(B)]
            wq_sb = persist.tile([P, KD, P], F32R)
            wk_sb = persist.tile([P, KD, P], F32R)
            wv_sb = persist.tile([P, KD, P], F32R)
            id64_sb = persist.tile([P, DH], BF16)

            for b_ in range(B):
                nc.vector.memset(vp_sb[b_][:, :, 0, DH:DH + 1], 1.0)
                nc.vector.memset(vp_sb[b_][:, :, 1, DH:DH + 1], 1.0)

            with (
                tc.tile_pool(name="xpool", bufs=2) as xpool,
                tc.tile_pool(name="accp", bufs=2, space="PSUM") as accp,
                tc.tile_pool(name="slabp", bufs=50) as slabp,
                tc.tile_pool(name="stagep", bufs=2) as stagep,
                tc.tile_pool(name="smallp", bufs=8) as smallp,
                tc.tile_pool(name="pss", bufs=2, space="PSUM") as pss,
                tc.tile_pool(name="psc", bufs=2, space="PSUM") as psc,
            ):
                xt_tiles = {}

                def issue_xt(ci):
                    t = xpool.tile([P, KD, 512], F32R, name="xt_tile")
                    # split DMAs so the first projection matmuls start as
                    # soon as the front piece lands (subtile deps); the cold
                    # first chunk is quartered to shave the pipeline fill
                    if ci == 0:
                        for q4 in range(4):
                            nc.sync.dma_start(t[:, 2 * q4:2 * q4 + 2],
                                              xt8[ci, :, 2 * q4:2 * q4 + 2])
                    else:
                        nc.sync.dma_start(t[:, 0:4], xt8[ci, :, 0:4])
                        nc.sync.dma_start(t[:, 4:8], xt8[ci, :, 4:8])
                    xt_tiles[ci] = t

                def gen_proj_qk(ci):
                    """Generator: project chunk ci into qt/kt (bf16). For the
                    first chunk the q and k chains interleave so both finish
                    (and the first scores emit) as early as possible."""
                    b_, o = chunks[ci]
                    osl = slice(o * 512, (o + 1) * 512)
                    xt_tile = xt_tiles[ci]
                    if ci == 0:
                        accq = accp.tile([P, 512], F32, name="acc_ps",
                                         tag="acc")
                        acck = accp.tile([P, 512], F32, name="acc_ps",
                                         tag="acc")
                        for kc in range(KD):
                            _mm(nc, accq[:], wq_sb[:, kc], xt_tile[:, kc],
                                start=(kc == 0), stop=(kc == KD - 1))
                            _mm(nc, acck[:], wk_sb[:, kc], xt_tile[:, kc],
                                start=(kc == 0), stop=(kc == KD - 1))
                            if kc % 2 == 1:
                                yield
                        nc.vector.tensor_copy(qt_sb[b_][:, osl], accq[:])
                        nc.vector.tensor_copy(kt_sb[b_][:, osl], acck[:])
                        yield
                        return
                    for w_sb, dst in ((wq_sb, qt_sb[b_]), (wk_sb, kt_sb[b_])):
                        acc = accp.tile([P, 512], F32, name="acc_ps", tag="acc")
                        for kc in range(KD):
                            _mm(nc, acc[:], w_sb[:, kc], xt_tile[:, kc],
                                start=(kc == 0), stop=(kc == KD - 1))
                            if kc % 2 == 1:
                                yield
                        nc.vector.tensor_copy(dst[:, osl], acc[:])
                        yield

                def gen_proj_v(ci):
                    """Generator: V projection + natural-layout transposes."""
                    b_, o = chunks[ci]
                    osl = slice(o * 512, (o + 1) * 512)
                    xt_tile = xt_tiles.pop(ci)
                    acc = accp.tile([P, 512], F32, name="acc_ps", tag="acc")
                    for kc in range(KD):
                        _mm(nc, acc[:], wv_sb[:, kc], xt_tile[:, kc],
                            start=(kc == 0), stop=(kc == KD - 1))
                        if kc % 2 == 1:
                            yield
                    nc.vector.tensor_copy(vt_sb[b_][:, osl], acc[:])
                    yield
                    for t in range(4):
                        kc2 = o * 4 + t
                        for hh in range(2):
                            tp = accp.tile([P, DH], BF16, name="tp_ps",
                                           tag="acc")
                            nc.tensor.transpose(
                                tp[:, 0:DH],
                                vt_sb[b_][hh * DH:(hh + 1) * DH,
                                          kc2 * P:(kc2 + 1) * P],
                                id64_sb[hh * DH:(hh + 1) * DH, :])
                            nc.vector.tensor_copy(
                                vp_sb[b_][:, kc2, hh, 0:DH], tp[:, 0:DH])
                        yield

                slabs = {i: {} for i in range(NQ)}   # idx -> kc -> slab tile

                def emit_scores(idx, kc):
                    b_, o = chunks[idx]
                    qs = slice(o * 512, (o + 1) * 512)
                    ks = slice(kc * P, (kc + 1) * P)
                    s_ps = pss.tile([P, 1024], F32, name="s_ps")
                    nc.tensor.matmul(s_ps[:, 0:512], kt_sb[b_][0:DH, ks],
                                     qt_sb[b_][0:DH, qs], start=True, stop=True)
                    nc.tensor.matmul(s_ps[:, 512:1024], kt_sb[b_][DH:2 * DH, ks],
                                     qt_sb[b_][DH:2 * DH, qs],
                                     start=True, stop=True)
                    slab = slabp.tile([P, 1024], BF16, name="slab")
                    nc.scalar.activation(slab[:], s_ps[:], AF.Exp, scale=0.125)
                    nc.vector.tensor_scalar_max(slab[:], slab[:], 1.0)
                    slabs[idx][kc] = slab

                def gen_chains(idx):
                    """Generator: the 8 ctx chains of idx + normalize + DMA,
                    yielding every couple of matmuls."""
                    b_, o = chunks[idx]
                    stage = stagep.tile([P, 4, P], F32, name="stage")
                    for ci in range(8):
                        j, h = ci // 2, ci % 2
                        acc = psc.tile([P, DH + 1], F32, name="ctx_ps")
                        for kc in range(KI):
                            nc.tensor.matmul(
                                acc[:],
                                slabs[idx][kc][:, h * 512 + j * P:
                                               h * 512 + (j + 1) * P],
                                vp_sb[b_][:, kc, h, :],
                                start=(kc == 0), stop=(kc == KI - 1))
                            if kc % 4 == 3:
                                yield
                        inv = smallp.tile([P, 1], F32, name="inv")
                        nc.vector.reciprocal(inv[:], acc[:, DH:DH + 1])
                        nc.vector.tensor_scalar(
                            out=stage[:, j, h * DH:(h + 1) * DH],
                            in0=acc[:, 0:DH], scalar1=inv[:], scalar2=None,
                            op0=OP.mult)
                        yield
                    nc.sync.dma_start(ctxn[:, idx], stage[:])
                    slabs[idx].clear()

                # ---------------- queue-based emitter ----------------
                emitted = set()           # (idx, kc) scores emitted
                score_q = []              # ordered pending scores
                qk_done = [False] * NQ
                v_done = [False] * NQ
                chains_done = 0           # count of fully-emitted chain idxs
                chain_gen = None
                chain_idx = 0             # next idx needing chains
                qk_idx = 0                # next chunk for q/k projection
                v_idx = 0                 # next chunk for v projection
                qkgen = None
                vgen = None

                def update_score_q():
                    for i in range(NQ):
                        bi, _ = chunks[i]
                        if not qk_done[i]:
                            continue
                        if i >= chains_done + 3:
                            continue
                        base = 4 * bi
                        kmax = sum(4 for c in range(base, base + NO)
                                   if qk_done[c])
                        for k in range(kmax):
                            if (i, k) not in emitted and (i, k) not in score_q:
                                score_q.append((i, k))

                # DMA order tuned so the first q-projection matmuls can
                # start at ~4.5us: wq, then the first xt half, then wk etc.
                nc.sync.dma_start(wq_sb[:], wqt8.ap())
                issue_xt(0)
                nc.sync.dma_start(wk_sb[:], wkt8.ap())
                nc.sync.dma_start(wv_sb[:], wvt8.ap())
                issue_xt(1)
                nc.sync.dma_start(id64_sb[:], id64b.ap())
                while (qk_idx < NQ or v_idx < NQ or score_q
                       or chain_idx < NQ or chain_gen is not None):
                    # 1. a slice of chain work (PE filler, no Act dependency)
                    if chain_gen is None and chain_idx < NQ:
                        bci, _ = chunks[chain_idx]
                        if (len(slabs[chain_idx]) == KI
                                and all(v_done[c] for c in
                                        range(4 * bci, 4 * bci + NO))):
                            chain_gen = gen_chains(chain_idx)
                    if chain_gen is not None:
                        for _ in range(3 if score_q else 8):
                            try:
                                next(chain_gen)
                            except StopIteration:
                                chain_gen = None
                                chain_idx += 1
                                chains_done += 1
                                update_score_q()
                                break
                    # 2. q/k projection pieces (gate scores)
                    if qkgen is None and qk_idx < NQ and qk_idx <= v_idx:
                        qkgen = gen_proj_qk(qk_idx)
                    if qkgen is not None:
                        steps = (4 if not score_q
                                 else (2 if len(score_q) < 6 else 1))
                        for _ in range(steps):
                            try:
                                next(qkgen)
                            except StopIteration:
                                qk_done[qk_idx] = True
                                qk_idx += 1
                                if qk_idx + 1 < NQ:
                                    issue_xt(qk_idx + 1)
                                qkgen = None
                                update_score_q()
                                break
                    # 3. v projection + transposes (gate chains only)
                    if vgen is None and v_idx < NQ and v_idx < qk_idx:
                        vgen = gen_proj_v(v_idx)
                    if vgen is not None:
                        # boost only when idle or when chains are starved on v
                        chain_starved = (
                            chain_gen is None and chain_idx < NQ
                            and len(slabs[chain_idx]) == KI)
                        steps = 2 if (chain_starved or not score_q
                                      or v_idx < qk_idx - 1) else 1
                        for _ in range(steps):
                            try:
                                next(vgen)
                            except StopIteration:
                                v_done[v_idx] = True
                                v_idx += 1
                                vgen = None
                                break
                    # 4. one score (the Act engine's food)
                    if score_q:
                        i, k = score_q.pop(0)
                        emit_scores(i, k)
                        emitted.add((i, k))
                        update_score_q()
    nc.compile()
    return nc


# --------------------------------------------------------------------------
# Phase B: per-core row-parallel transpose + Wo-proj + AddNorm1 + FFN + AddNorm2.
# Inputs (per core, qi = this core's 512 token rows):
#   ctin [P, 4, KD, P] bf16   natural-layout ctx blocks for these rows
#   wo8  [P, KD, D]    bf16   Wo^T tiled
#   w18  [KD, P, KD, 512] bf16  W1^T tiled per 512-wide ffn-col group
#   w2a8/w2b8 [KD, P, 4, 512] bf16  W2^T tiled, first/second output half
#   xts8 [P, KD, 512] f32     X^T slice (residual 1)
#   consts [P, 72] f32        g1|be1|g2|be2|b1t|b2t feature-on-partition
#   id128b [P, P] bf16
# Output: out8 [P, KD, 512] f32 (t-layout output slice, tiled)
# --------------------------------------------------------------------------
def _build_phase_b():
    nc = bacc.Bacc("TRN2", target_bir_lowering=False, debug=False,
                   num_devices=N_CORES)
    ctin = nc.dram_tensor("ctin", [P, KD, 4, P], BF16, kind="ExternalInput")
    wo8 = nc.dram_tensor("wo8", [P, KD, D], BF16, kind="ExternalInput")
    w18 = nc.dram_tensor("w18", [KD, P, KD, 512], BF16, kind="ExternalInput")
    w2a8 = nc.dram_tensor("w2a8", [KD, P, 4, 512], BF16, kind="ExternalInput")
    w2b8 = nc.dram_tensor("w2b8", [KD, P, 4, 512], BF16, kind="ExternalInput")
    xts8 = nc.dram_tensor("xts8", [P, KD, 512], F32, kind="ExternalInput")
    consts = nc.dram_tensor("consts", [P, 72], F32, kind="ExternalInput")
    id128b = nc.dram_tensor("id128b", [P, P], BF16, kind="ExternalInput")
    outs = [nc.dram_tensor(f"outs{i}", [P, KD, w], BF16,
                       kind="ExternalOutput")
        for i, w in enumerate((256, 128, 128))]

    with tile.TileContext(nc) as tc:
        with (
            tc.tile_pool(name="persist", bufs=1) as persist,
            tc.tile_pool(name="w1p", bufs=3) as w1p,
            tc.tile_pool(name="w2p", bufs=3) as w2p,
            tc.tile_pool(name="sqp", bufs=3) as sqp,
            tc.tile_pool(name="smallp", bufs=2) as smallp,
            tc.tile_pool(name="bcp", bufs=2) as bcp,
        ):
            ctin_sb = persist.tile([P, KD, 4, P], BF16)
            ct_sb = persist.tile([P, KD, 4, P], BF16)
            wo_sb = persist.tile([P, KD, D], BF16)
            xts_sb = persist.tile([P, KD, 512], F32)
            y1_sb = persist.tile([P, KD, 512], BF16)
            z1_sb = persist.tile([P, KD, 512], BF16)
            h_sb = persist.tile([P, KF, 512], BF16)
            w2b_sb = persist.tile([P, KD, 4, 512], BF16)
            y2_sb = persist.tile([P, KD, 512], BF16, tag="y1_sb")
            z2_seg = [persist.tile([P, KD, w], BF16, name=f"z2s{i}", tag="xts_sb",
                       bufs=1)
          for i, w in enumerate((256, 128, 128))]
            consts_sb = persist.tile([P, 72], F32)
            id128_sb = persist.tile([P, P], BF16)
            ones = persist.tile([P, 1], BF16)

            nc.sync.dma_start(ctin_sb[:, 0:2], ctin[:, 0:2])
            nc.sync.dma_start(id128_sb[:], id128b.ap())
            nc.sync.dma_start(wo_sb[:, 0:2], wo8[:, 0:2])
            nc.sync.dma_start(ctin_sb[:, 2:8], ctin[:, 2:8])
            nc.sync.dma_start(wo_sb[:, 2:8], wo8[:, 2:8])
            nc.sync.dma_start(consts_sb[:], consts.ap())
            nc.sync.dma_start(xts_sb[:], xts8.ap())
            nc.vector.memset(ones[:], 1.0)

            g1 = [consts_sb[:, kc:kc + 1] for kc in range(KD)]
            be1 = [consts_sb[:, 8 + kc:9 + kc] for kc in range(KD)]
            g2 = [consts_sb[:, 16 + kc:17 + kc] for kc in range(KD)]
            be2 = [consts_sb[:, 24 + kc:25 + kc] for kc in range(KD)]
            b1c = [consts_sb[:, 32 + fm:33 + fm] for fm in range(KF)]
            b2c = [consts_sb[:, 64 + kc:65 + kc] for kc in range(KD)]

            def ln_finish(st_sum, st_sq, tag, w=512):
                """Stats (over w tokens) -> (rstd_b, ms_b) broadcast tiles."""
                mean = smallp.tile([1, w], F32, name="mean")
                ex2 = smallp.tile([1, w], F32, name="ex2")
                nc.vector.tensor_scalar(out=mean[:], in0=st_sum,
                                        scalar1=1.0 / D, scalar2=None,
                                        op0=OP.mult)
                nc.vector.tensor_scalar(out=ex2[:], in0=st_sq,
                                        scalar1=1.0 / D, scalar2=None,
                                        op0=OP.mult)
                msq = smallp.tile([1, w], F32, name="msq")
                nc.vector.tensor_mul(msq[:], mean[:], mean[:])
                var = smallp.tile([1, w], F32, name="var")
                nc.vector.tensor_sub(var[:], ex2[:], msq[:])
                nc.vector.tensor_scalar_add(var[:], var[:], EPS)
                std = smallp.tile([1, w], F32, name="std")
                nc.scalar.activation(std[:], var[:], AF.Sqrt)
                rstd = smallp.tile([1, w], BF16, name="rstd")
                ms = smallp.tile([1, w], BF16, name="ms")
                with nc.allow_low_precision(reason="bf16 LN scale factors"):
                    nc.vector.reciprocal(rstd[:], std[:])
                nc.vector.tensor_mul(ms[:], mean[:], rstd[:])
                rstd_b = bcp.tile([P, w], BF16, name="rstd_b", bufs=3)
                ms_b = bcp.tile([P, w], BF16, name="ms_b", bufs=3)
                nc.gpsimd.partition_broadcast(rstd_b[:], rstd[:])
                nc.gpsimd.partition_broadcast(ms_b[:], ms[:])
                return rstd_b, ms_b

            def ln_apply_seg(y_sb, rstd_b, ms_b, g_c, be_c, z_t, kc, cols):
                w = cols.stop - cols.start
                t = sqp.tile([P, 512], BF16, name="t_ln")
                nc.vector.tensor_mul(t[:, 0:w], y_sb[:, kc, cols], rstd_b[:])
                nc.vector.tensor_sub(t[:, 0:w], t[:, 0:w], ms_b[:])
                nc.vector.tensor_scalar(out=z_t[:, kc], in0=t[:, 0:w],
                                        scalar1=g_c[kc], scalar2=be_c[kc],
                                        op0=OP.mult, op1=OP.add)

            def ln_apply(y_sb, rstd_b, ms_b, g_c, be_c, z_sb, kc, cols=None):
                cols = cols or slice(0, 512)
                w = cols.stop - cols.start
                t = sqp.tile([P, 512], BF16, name="t_ln")
                nc.vector.tensor_mul(t[:, 0:w], y_sb[:, kc, cols], rstd_b[:])
                nc.vector.tensor_sub(t[:, 0:w], t[:, 0:w], ms_b[:])
                nc.vector.tensor_scalar(out=z_sb[:, kc, cols], in0=t[:, 0:w],
                                        scalar1=g_c[kc], scalar2=be_c[kc],
                                        op0=OP.mult, op1=OP.add)

            # ---- B0+B1: transpose ctx to t-layout, fused with Wo matmuls ----
            with (
                tc.tile_pool(name="tpp", bufs=2, space="PSUM") as tpp,
                tc.tile_pool(name="psa", bufs=1, space="PSUM") as psa,
                tc.tile_pool(name="psst1", bufs=1, space="PSUM") as psst1,
            ):
                st1_sum = psst1.tile([1, 512], F32, name="st1_sum")
                st1_sq = psst1.tile([1, 512], F32, name="st1_sq")
                # chain-major Wo: each output chain stops early so its
                # residual add + square overlap the following chains
                a_ps = [psa.tile([P, 512], F32, name=f"mm_ps{i}")
                        for i in range(4)]
                sqs = {}

                def y1_add_sq(m, ps):
                    nc.vector.tensor_add(y1_sb[:, m], ps[:], xts_sb[:, m])
                    sq = sqp.tile([P, 512], BF16, name="sq", bufs=14)
                    nc.vector.tensor_mul(sq[:], y1_sb[:, m], y1_sb[:, m])
                    sqs[m] = sq

                for kc in range(KD):
                    tp = tpp.tile([P, 4, P], BF16, name="tp_ps")
                    for jb in range(4):
                        nc.tensor.transpose(tp[:, jb, :],
                                            ctin_sb[:, kc, jb, :],
                                            id128_sb[:])
                    nc.vector.tensor_copy(ct_sb[:, kc], tp[:])
                    nc.tensor.matmul(a_ps[0][:], wo_sb[:, kc, 0:P],
                                     ct_sb[:, kc],
                                     start=(kc == 0), stop=(kc == KD - 1))
                y1_add_sq(0, a_ps[0])
                for i in range(1, 4):
                    for kc in range(KD):
                        nc.tensor.matmul(
                            a_ps[i][:], wo_sb[:, kc, i * P:(i + 1) * P],
                            ct_sb[:, kc], start=(kc == 0), stop=(kc == KD - 1))
                    y1_add_sq(i, a_ps[i])
                # mg1 token-halved: half 0's LN1 stats/broadcast hide under
                # half 1's chains, and FFN1's first group (also token-halved,
                # below) starts right after the half-0 applies
                a_ps2 = [psa.tile([P, 512], F32, name=f"mm_ps{i}")
                         for i in range(4)]
                ln1_bh = {}
                for half in range(2):
                    hs = slice(half * 256, (half + 1) * 256)
                    jbs = slice(half * 2, half * 2 + 2)
                    for i in range(4):
                        m = 4 + i
                        for kc in range(KD):
                            nc.tensor.matmul(
                                a_ps2[i][:, hs],
                                wo_sb[:, kc, 512 + i * P:512 + (i + 1) * P],
                                ct_sb[:, kc, jbs, :],
                                start=(kc == 0), stop=(kc == KD - 1))
                        nc.vector.tensor_add(y1_sb[:, m, hs],
                                             a_ps2[i][:, hs],
                                             xts_sb[:, m, hs])
                        sq = sqp.tile([P, 512], BF16, name="sq", bufs=14)
                        nc.vector.tensor_mul(sq[:, hs], y1_sb[:, m, hs],
                                             y1_sb[:, m, hs])
                        sqs[(m, half)] = sq
                    for kc in range(KD):
                        nc.tensor.matmul(st1_sum[:, hs], ones[:],
                                         y1_sb[:, kc, hs],
                                         start=(kc == 0), stop=(kc == KD - 1))
                    for kc in range(KD):
                        s = sqs[kc] if kc < 4 else sqs[(kc, half)]
                        nc.tensor.matmul(st1_sq[:, hs], ones[:], s[:, hs],
                                         start=(kc == 0), stop=(kc == KD - 1))
                    ln1_bh[half] = ln_finish(st1_sum[:, hs], st1_sq[:, hs],
                                             f"ln1h{half}", w=256)
                sqs.clear()
                for half in range(2):
                    hs = slice(half * 256, (half + 1) * 256)
                    rstd_h, ms_h = ln1_bh[half]
                    for kc in range(KD):
                        ln_apply(y1_sb, rstd_h, ms_h, g1, be1, z1_sb, kc,
                                 cols=hs)

            # ---- FFN1 + FFN2 (first output half interleaved) ----
            with tc.tile_pool(name="psa2", bufs=1, space="PSUM") as psa2:
                f_ps = [psa2.tile([P, 512], F32, name=f"f_ps{i}")
                        for i in range(4)]
                for fg in range(KD):
                    w1_tile = w1p.tile([P, KD, 512], BF16, name="w1_tile")
                    nc.sync.dma_start(w1_tile[:], w18[fg])
                    h_ps = [psa2.tile([P, 512], F32, name=f"h_ps{i}")
                            for i in range(4)]
                    if fg == 0:
                        # token-halved so the half-0 chains start as soon as
                        # the half-0 LN1 applies land
                        for half in range(2):
                            hs = slice(half * 256, (half + 1) * 256)
                            for kc in range(KD):
                                for i in range(4):
                                    nc.tensor.matmul(
                                        h_ps[i][:, hs],
                                        w1_tile[:, kc, i * P:(i + 1) * P],
                                        z1_sb[:, kc, hs],
                                        start=(kc == 0), stop=(kc == KD - 1))
                    else:
                        for kc in range(KD):
                            for i in range(4):
                                nc.tensor.matmul(
                                    h_ps[i][:], w1_tile[:, kc, i * P:(i + 1) * P],
                                    z1_sb[:, kc], start=(kc == 0),
                                    stop=(kc == KD - 1))
                    for i in range(4):
                        fm = fg * 4 + i
                        nc.scalar.activation(h_sb[:, fm], h_ps[i][:], AF.Relu,
                                             bias=b1c[fm])
                    w2_tile = w2p.tile([P, 4, 512], BF16, name="w2_tile")
                    nc.sync.dma_start(w2_tile[:], w2a8[fg])
                    nc.sync.dma_start(w2b_sb[:, fg], w2b8[fg])
                    for i in range(4):
                        fk = fg * 4 + i
                        for j in range(4):
                            nc.tensor.matmul(
                                f_ps[j][:], w2_tile[:, i, j * P:(j + 1) * P],
                                h_sb[:, fk], start=(fk == 0), stop=(fk == KF - 1))
                # y2 first half + squares; LN2 stats for it run behind
                # FFN2b's first chain
                st2_sum = psa2.tile([1, 512], F32, name="h_ps0", tag="h_ps0")
                st2_sq = psa2.tile([1, 512], F32, name="h_ps1", tag="h_ps1")
                sqs2 = {}

                def y2_add_sq(m, ps):
                    nc.vector.scalar_tensor_tensor(
                        out=y2_sb[:, m], in0=ps[:], scalar=b2c[m],
                        in1=z1_sb[:, m], op0=OP.add, op1=OP.add)
                    sq = sqp.tile([P, 512], BF16, name="sq", bufs=14)
                    nc.vector.tensor_mul(sq[:], y2_sb[:, m], y2_sb[:, m])
                    sqs2[m] = sq

                for j in range(4):
                    y2_add_sq(j, f_ps[j])

                # token-halved FFN2 second half: half 0's chains, stats,
                # and LN2 applies complete while half 1's chains run, so only
                # half 1's (narrower) LN2 epilogue remains on the tail
                f_ps2 = [psa2.tile([P, 512], F32, name=f"f_ps{i}")
                         for i in range(4)]
                segs = [(0, 256), (256, 384), (384, 512)]
                for si, (s0, s1) in enumerate(segs):
                    hs = slice(s0, s1)
                    for j in range(4):
                        m = 4 + j
                        for fg in range(KD):
                            for i in range(4):
                                nc.tensor.matmul(
                                    f_ps2[j][:, hs],
                                    w2b_sb[:, fg, i, j * P:(j + 1) * P],
                                    h_sb[:, fg * 4 + i, hs],
                                    start=(fg == 0 and i == 0),
                                    stop=(fg == KD - 1 and i == 3))
                        nc.vector.scalar_tensor_tensor(
                            out=y2_sb[:, m, hs], in0=f_ps2[j][:, hs],
                            scalar=b2c[m], in1=z1_sb[:, m, hs],
                            op0=OP.add, op1=OP.add)
                        sq = sqp.tile([P, 512], BF16, name="sq", bufs=14)
                        nc.vector.tensor_mul(sq[:, hs], y2_sb[:, m, hs],
                                             y2_sb[:, m, hs])
                        sqs2[(m, si)] = sq
                        if j == 0:
                            for kc in range(4):
                                nc.tensor.matmul(
                                    st2_sum[:, hs], ones[:],
                                    y2_sb[:, kc, hs],
                                    start=(kc == 0), stop=False)
                                nc.tensor.matmul(
                                    st2_sq[:, hs], ones[:],
                                    sqs2[kc][:, hs],
                                    start=(kc == 0), stop=False)
                        else:
                            mm1 = 4 + j - 1
                            nc.tensor.matmul(
                                st2_sum[:, hs], ones[:], y2_sb[:, mm1, hs],
                                start=False, stop=False)
                            nc.tensor.matmul(
                                st2_sq[:, hs], ones[:],
                                sqs2[(mm1, si)][:, hs],
                                start=False, stop=False)
                    nc.tensor.matmul(st2_sum[:, hs], ones[:],
                                     y2_sb[:, 7, hs], start=False, stop=True)
                    nc.tensor.matmul(st2_sq[:, hs], ones[:],
                                     sqs2[(7, si)][:, hs],
                                     start=False, stop=True)
                    rstd_h, ms_h = ln_finish(st2_sum[:, hs], st2_sq[:, hs],
                                             f"ln2s{si}", w=s1 - s0)
                    for kc in range(KD):
                        ln_apply_seg(y2_sb, rstd_h, ms_h, g2, be2,
                                     z2_seg[si], kc, hs)
                    nc.sync.dma_start(outs[si].ap(), z2_seg[si][:])
    nc.compile()
    return nc


def _get(name, builder):
    if name not in _CACHE:
        _CACHE[name] = builder()
    return _CACHE[name]


def _prep_inputs(X, Wq, Wk, Wo, ln1_g, ln1_b, ln2_g, ln2_b, W1, b1, W2, b2):
    f = lambda a: np.ascontiguousarray(a)
    Xt = np.asarray(X, np.float32).reshape(N, D).T          # [D, N]
    WqT = np.asarray(Wq, np.float32).T                      # [D, D]
    WkT = np.asarray(Wk, np.float32).T
    WoT = np.asarray(Wo, np.float32).T
    W1T = np.asarray(W1, np.float32).T                      # [D, FF]
    W2T = np.asarray(W2, np.float32).T                      # [FF, D]
    vecP = lambda v, k: np.asarray(v, np.float32).reshape(k, P).T  # [P, k]

    # xt8: [idx, p, kc, q]
    xt8 = f(Xt.reshape(KD, P, NQ, 512).transpose(2, 1, 0, 3))
    id64b = np.tile(np.eye(DH, dtype=np.float32), (2, 1)).astype(BF)
    w_tile = lambda w, c: f(
        w[:, c * P:(c + 1) * P].reshape(KD, P, P).transpose(1, 0, 2))

    in_maps_a = [
        {
            "xt8": xt8,
            "id64b": id64b,
            "wqt8": w_tile(WqT, c),
            "wkt8": w_tile(WkT, c),
            "wvt8": w_tile(WoT, c),   # value projection uses W_o in this model
        }
        for c in range(N_CORES)
    ]

    wo8 = f(WoT.reshape(KD, P, D).transpose(1, 0, 2).astype(BF))
    w18 = f(W1T.reshape(KD, P, KD, 512).transpose(2, 1, 0, 3).astype(BF))
    w2a8 = f(W2T[:, 0:512].reshape(KD, 4, P, 512).transpose(0, 2, 1, 3)
             .astype(BF))
    w2b8 = f(W2T[:, 512:1024].reshape(KD, 4, P, 512).transpose(0, 2, 1, 3)
             .astype(BF))
    consts = f(np.hstack([vecP(ln1_g, KD), vecP(ln1_b, KD), vecP(ln2_g, KD),
                          vecP(ln2_b, KD), vecP(b1, KF), vecP(b2, KD)]))
    id128b = np.eye(P, dtype=BF)

    def in_maps_b(full_nat):
        maps = []
        for c in range(N_CORES):
            blk = full_nat[c * QC:(c + 1) * QC]             # [512, 1024]
            ctin = f(blk.reshape(4, P, KD, P).transpose(1, 2, 0, 3).astype(BF))
            xts = f(Xt[:, c * QC:(c + 1) * QC].reshape(KD, P, 512)
                    .transpose(1, 0, 2))
            maps.append({
                "ctin": ctin, "xts8": xts, "wo8": wo8, "w18": w18,
                "w2a8": w2a8, "w2b8": w2b8, "consts": consts,
                "id128b": id128b,
            })
        return maps

    return in_maps_a, in_maps_b


def kernel(X, Wq, Wk, Wo, ln1_g, ln1_b, ln2_g, ln2_b, W1, b1, W2, b2):
    in_maps_a, in_maps_b = _prep_inputs(
        X, Wq, Wk, Wo, ln1_g, ln1_b, ln2_g, ln2_b, W1, b1, W2, b2)

    nc_a = _get("a", _build_phase_a)
    res_a = run_bass_kernel_spmd(nc_a, in_maps_a, core_ids=list(range(N_CORES)))
    # ctxn [P, NQ, 4, P] per core -> natural [4096, 128] -> concat cols
    full_nat = np.concatenate(
        [res_a.results[c]["ctxn"].transpose(1, 2, 0, 3).reshape(N, P)
         for c in range(N_CORES)], axis=1)                  # [N, D]

    nc_b = _get("b", _build_phase_b)
    res_b = run_bass_kernel_spmd(nc_b, in_maps_b(full_nat),
                                 core_ids=list(range(N_CORES)))
    # outs{0,1,2} [P, KD, w] per core (token segments) -> [D, 512] col block
    out_t = np.concatenate(
        [np.concatenate(
            [res_b.results[c][f"outs{i}"].astype(np.float32) for i in range(3)],
            axis=2).transpose(1, 0, 2).reshape(D, QC)
         for c in range(N_CORES)], axis=1)                  # [D, N]
    return np.ascontiguousarray(out_t.T).reshape(B, S, D).astype(np.float32)
